# revision 1
# baseline (speedup 1.0000x reference)
"""Trainium2 Bass kernel for the sparse submanifold 3D CNN (nn_Net_38963943309313).

Network: 7 blocks of 2 submanifold 3x3x3 convs on a 64^3 grid, 2x2x2 sparse
max-pools between blocks, channels 3->64->...->256, output [1,1,1,1,256].

Strategy (8 NeuronCores):
 - Shard z-slabs across cores for levels 0-2 (grids 64/32/16), AllGather the
   pooled activations between levels (z-padded gather buffers so per-core
   reads are a single dynamic-offset DMA). Levels 3-6 (grids 8/4/2/1) are
   replicated on every core.
 - Convs are fp32r matmuls: activations channel-major [C, z, y, x] in SBUF
   (y/x zero-padded), 27 shifted-window matmuls accumulated in PSUM.
 - conv1 of block 0 uses a host-side im2col (81 contract rows, masked
   columns so the submanifold mask is free).
 - 64-channel contractions (L0 conv2, L1 conv1) pack z-pairs into K=128 via
   duplicated storage; L0 conv2 additionally pairs two output slices into
   the two 64-column halves of the PE array.
 - Submanifold masking: conv1 evictions multiply by a broadcast mask (also
   zeroes the out-of-grid halo slices); conv2 evictions add (mask-1)*BIG so
   the following max-pool ignores inactive voxels; pool result is multiplied
   by the pooled mask.
"""

import sys

sys.path.insert(0, "/opt/trn_rl_repo")

import numpy as np
import concourse.bass as bass
import concourse.tile as tile
from concourse.tile import add_dep_helper
from concourse import bacc, mybir
from concourse.bass_utils import run_bass_kernel_spmd

NC = 8
GRID = 64
BIG = 1.0e30
CHANNELS = [(3, 64), (64, 64), (64, 96), (96, 96), (96, 128), (128, 128),
            (128, 160), (160, 160), (160, 192), (192, 192), (192, 224),
            (224, 224), (224, 256), (256, 256)]
F32 = mybir.dt.float32
F32R = mybir.dt.float32r

OFFSETS = [(dz, dy, dx) for dz in (-1, 0, 1) for dy in (-1, 0, 1) for dx in (-1, 0, 1)]
# 9 (dy,dx) pairs for z-pair-packed layers
DYDX = [(dy, dx) for dy in (-1, 0, 1) for dx in (-1, 0, 1)]


def _pool_np(x, m):
    # x: [D,D,D,C] or [D,D,D]; max over active voxels of 2x2x2 windows
    D = x.shape[0]
    if x.ndim == 3:
        xr = x.reshape(D // 2, 2, D // 2, 2, D // 2, 2)
        return xr.max(axis=(1, 3, 5))
    neg = np.where(m[..., None] > 0, x, -np.inf)
    xr = neg.reshape(D // 2, 2, D // 2, 2, D // 2, 2, -1)
    p = xr.max(axis=(1, 3, 5))
    mp = m.reshape(D // 2, 2, D // 2, 2, D // 2, 2).max(axis=(1, 3, 5))
    return np.where(mp[..., None] > 0, p, 0.0), mp


def _ceil_div(a, b):
    return (a + b - 1) // b


def build_host_inputs(features, coors, Ws):
    """All host-side data marshalling. Returns (in_maps, meta)."""
    z, y, x = coors[:, 0], coors[:, 1], coors[:, 2]
    dense = np.zeros((GRID, GRID, GRID, 3), np.float32)
    mask0 = np.zeros((GRID, GRID, GRID), np.float32)
    dense[z, y, x] = features  # last write wins (matches XLA CPU scatter)
    mask0[z, y, x] = 1.0

    # mask pyramid
    masks = [mask0]
    m = mask0
    for _ in range(6):
        mr = m.reshape(m.shape[0] // 2, 2, m.shape[1] // 2, 2, m.shape[2] // 2, 2)
        m = mr.max(axis=(1, 3, 5))
        masks.append(m)

    # ---- X1col: host im2col for conv1 of block 0, column-masked ----
    # padded dense [3, 66, 66, 66]
    dpad = np.zeros((3, GRID + 2, GRID + 2, GRID + 2), np.float32)
    dpad[:, 1:-1, 1:-1, 1:-1] = dense.transpose(3, 0, 1, 2)
    # X1col_full[(off*3+ci), zglob, y, x] ; z in [-1, 65) handled per-core
    # build per-core slabs directly: core k conv1-out slices global [8k-1, 8k+9)
    x1cols = []
    for k in range(NC):
        xc = np.zeros((10, 81, GRID * GRID), np.float32)
        for sl in range(10):
            zg = 8 * k - 1 + sl
            if zg < 0 or zg >= GRID:
                continue
            cols = np.zeros((81, GRID, GRID), np.float32)
            for o, (dz, dy, dx) in enumerate(OFFSETS):
                # padded coords: (zg+dz+1, y+dy+1, x+dx+1) over y,x in [0,64)
                cols[o * 3:(o + 1) * 3] = dpad[:, zg + dz + 1,
                                               1 + dy:GRID + 1 + dy,
                                               1 + dx:GRID + 1 + dx]
            cols *= mask0[zg][None, :, :]
            xc[sl] = cols.reshape(81, -1)
        x1cols.append(xc)

    # ---- weight packs ----
    # W0 for im2col conv1: [81, 128] (co=64 duplicated for col-pairing)
    W0 = Ws[0]  # [3,3,3,3,64]
    w1col = np.zeros((81, 128), np.float32)
    for o, (dz, dy, dx) in enumerate(OFFSETS):
        w1col[o * 3:(o + 1) * 3, 0:64] = W0[dz + 1, dy + 1, dx + 1]
        w1col[o * 3:(o + 1) * 3, 64:128] = W0[dz + 1, dy + 1, dx + 1]

    def pack_pair(W):  # [3,3,3,cin,co] -> pair [2*cin, 9, co] + left [cin, 9, co]
        cin, co = W.shape[3], W.shape[4]
        wp = np.zeros((2 * cin, 9, co), np.float32)
        wl = np.zeros((cin, 9, co), np.float32)
        for j, (dy, dx) in enumerate(DYDX):
            wp[0:cin, j] = W[0, dy + 1, dx + 1]      # dz=-1
            wp[cin:2 * cin, j] = W[1, dy + 1, dx + 1]  # dz=0
            wl[:, j] = W[2, dy + 1, dx + 1]          # dz=+1
        return wp, wl

    w0p, w0l = pack_pair(Ws[1])   # L0 conv2 64->64
    w1p, w1l = pack_pair(Ws[2])   # L1 conv1 64->96
    w0l = np.concatenate([w0l, w0l], axis=0)  # [128, 9, 64] both halves
    w1l = np.concatenate([w1l, w1l], axis=0)  # [128, 9, 96]

    def pack_generic(W):  # -> list of [kchunk, 27, co] arrays
        cin, co = W.shape[3], W.shape[4]
        wf = W.reshape(27, cin, co)
        out = []
        for k0 in range(0, cin, 128):
            kc = min(128, cin - k0)
            out.append(np.ascontiguousarray(
                wf[:, k0:k0 + kc, :].transpose(1, 0, 2)))  # [kc, 27, co]
        return out

    gen_w = {}
    for li, wi in [("w1c2", 3), ("w2c1", 4), ("w2c2", 5), ("w3c1", 6),
                   ("w3c2", 7), ("w4c1", 8), ("w4c2", 9), ("w5c1", 10),
                   ("w5c2", 11)]:
        gen_w[li] = pack_generic(Ws[wi])
    # L6: center tap only (1^3 grid)
    for li, wi in [("w6c1", 12), ("w6c2", 13)]:
        W = Ws[wi]
        cin, co = W.shape[3], W.shape[4]
        wc = W[1, 1, 1]  # [cin, co]
        gen_w[li] = [np.ascontiguousarray(wc[k0:k0 + min(128, cin - k0)][:, None, :])
                     for k0 in range(0, cin, 128)]

    # ---- per-core mask arrays ----
    # L0 maskneg for conv2-evict: [8, 4096]
    mn0 = [((masks[0][8 * k:8 * k + 8] - 1.0) * BIG).reshape(8, -1).astype(np.float32)
           for k in range(NC)]
    # L0 pool-out multiply: m1 on core's L1 slices [4, 1024]
    m1p = [masks[1][4 * k:4 * k + 4].reshape(4, -1).astype(np.float32)
           for k in range(NC)]

    def slab_mask(mask, z0, nsl):
        D2 = mask.shape[1] * mask.shape[2]
        out = np.zeros((nsl, D2), np.float32)
        for i in range(nsl):
            zg = z0 + i
            if 0 <= zg < mask.shape[0]:
                out[i] = mask[zg].reshape(-1)
        return out

    # L1 conv1-evict multiply mask (m1 x ingrid): slices [4k-1, 4k+5)
    m1mul = [slab_mask(masks[1], 4 * k - 1, 6) for k in range(NC)]
    # L1 conv2-evict maskneg: slices [4k, 4k+4)
    mn1 = [((slab_mask(masks[1], 4 * k, 4) - 1.0) * BIG).astype(np.float32)
           for k in range(NC)]
    # L1 pool-out multiply: m2 on core's L2 slices [2, 256]
    m2p = [slab_mask(masks[2], 2 * k, 2) for k in range(NC)]
    # L2 conv1-evict multiply (m2 x ingrid): slices [2k-1, 2k+3)
    m2mul = [slab_mask(masks[2], 2 * k - 1, 4) for k in range(NC)]
    # L2 conv2-evict maskneg: slices [2k, 2k+2)
    mn2 = [((slab_mask(masks[2], 2 * k, 2) - 1.0) * BIG).astype(np.float32)
           for k in range(NC)]
    # L2 pool-out multiply: m3 on core's L3 slice [1, 64]
    m3p = [slab_mask(masks[3], k, 1) for k in range(NC)]
    # L3 (replicated): conv1-evict mul (m3 x ingrid) slices [-1, 9)
    m3mul_r = slab_mask(masks[3], -1, 10)
    mn3_r = ((slab_mask(masks[3], 0, 8) - 1.0) * BIG).astype(np.float32)
    m4p_r = slab_mask(masks[4], 0, 4)       # [4, 16]
    m4mul_r = slab_mask(masks[4], 0, 4)     # L4 out all valid (full grid)
    mn4_r = ((slab_mask(masks[4], 0, 4) - 1.0) * BIG).astype(np.float32)
    m5p_r = slab_mask(masks[5], 0, 2)
    m5mul_r = slab_mask(masks[5], 0, 2)
    mn5_r = ((slab_mask(masks[5], 0, 2) - 1.0) * BIG).astype(np.float32)
    m6p_r = slab_mask(masks[6], 0, 1)

    meta = {
        "mask_flags": {
            # whether the real mask (not just ingrid) has zeros at each level
            1: not np.all(masks[1] == 1.0),
            2: not np.all(masks[2] == 1.0),
            3: not np.all(masks[3] == 1.0),
            4: not np.all(masks[4] == 1.0),
            5: not np.all(masks[5] == 1.0),
            6: not np.all(masks[6] == 1.0),
        },
    }

    in_maps = []
    for k in range(NC):
        im = {
            "x1col": x1cols[k],
            "w1col": w1col,
            "w0p": w0p, "w0l": w0l, "w1p": w1p, "w1l": w1l,
            "mn0": mn0[k], "m1p": m1p[k],
            "m1mul": m1mul[k], "mn1": mn1[k], "m2p": m2p[k],
            "m2mul": m2mul[k], "mn2": mn2[k], "m3p": m3p[k],
            "m3mul": m3mul_r, "mn3": mn3_r, "m4p": m4p_r,
            "m4mul": m4mul_r, "mn4": mn4_r, "m5p": m5p_r,
            "m5mul": m5mul_r, "mn5": mn5_r, "m6p": m6p_r,
        }
        for name, chunks in gen_w.items():
            for ci, arr in enumerate(chunks):
                im[f"{name}_{ci}"] = arr
        in_maps.append(im)
    return in_maps, meta


def build_kernel(meta):
    import contextlib
    nc = bacc.Bacc("TRN2", target_bir_lowering=False, debug=False, num_devices=NC)
    mf = meta["mask_flags"]

    # ---------- DRAM I/O declarations ----------
    def din(name, shape):
        return nc.dram_tensor(name, list(shape), F32, kind="ExternalInput")

    x1col = din("x1col", (10, 81, 4096))
    w1col_d = din("w1col", (81, 128))
    w0p_d = din("w0p", (128, 9, 64)); w0l_d = din("w0l", (128, 9, 64))
    w1p_d = din("w1p", (128, 9, 96)); w1l_d = din("w1l", (128, 9, 96))
    mn0_d = din("mn0", (8, 4096)); m1p_d = din("m1p", (4, 1024))
    m1mul_d = din("m1mul", (6, 1024)); mn1_d = din("mn1", (4, 1024))
    m2p_d = din("m2p", (2, 256))
    m2mul_d = din("m2mul", (4, 256)); mn2_d = din("mn2", (2, 256))
    m3p_d = din("m3p", (1, 64))
    m3mul_d = din("m3mul", (10, 64)); mn3_d = din("mn3", (8, 64))
    m4p_d = din("m4p", (4, 16)); m4mul_d = din("m4mul", (4, 16))
    mn4_d = din("mn4", (4, 16))
    m5p_d = din("m5p", (2, 4)); m5mul_d = din("m5mul", (2, 4))
    mn5_d = din("mn5", (2, 4)); m6p_d = din("m6p", (1, 1))

    genw_d = {}
    genw_shapes = {
        "w1c2": [(96, 27, 96)], "w2c1": [(96, 27, 128)], "w2c2": [(128, 27, 128)],
        "w3c1": [(128, 27, 160)], "w3c2": [(128, 27, 160), (32, 27, 160)],
        "w4c1": [(128, 27, 192), (32, 27, 192)],
        "w4c2": [(128, 27, 192), (64, 27, 192)],
        "w5c1": [(128, 27, 224), (64, 27, 224)],
        "w5c2": [(128, 27, 224), (96, 27, 224)],
        "w6c1": [(128, 1, 256), (96, 1, 256)],
        "w6c2": [(128, 1, 256), (128, 1, 256)],
    }
    for name, shl in genw_shapes.items():
        genw_d[name] = [din(f"{name}_{i}", s) for i, s in enumerate(shl)]

    out_d = nc.dram_tensor("out", [1, 256], F32, kind="ExternalOutput")
    import os as _os
    DBG = bool(_os.environ.get("K_DEBUG"))
    dbg_d = {}
    if DBG:
        for nm, sh in [("dP0", (64, 4, 1156)), ("dA1", (128, 8, 1156)),
                       ("dB1", (96, 6, 1156)), ("dC1", (96, 4, 1024)),
                       ("dP1", (96, 2, 324)), ("dA2", (96, 6, 324)),
                       ("dB2", (128, 4, 324)), ("dC2", (128, 2, 256)),
                       ("dP2", (128, 1, 100)), ("dA3", (128, 12, 100)),
                       ("dB3a", (128, 10, 100)), ("dC3a", (128, 512)),
                       ("dP4a", (128, 216)), ("dB4a", (128, 216)),
                       ("dC4a", (128, 64)), ("dP5a", (128, 64)),
                       ("dB5a", (128, 64)), ("dP6a", (128, 27)),
                       ("dX6a", (128, 1)), ("dC0", (64, 2, 4096))]:
            dbg_d[nm] = nc.dram_tensor(nm, list(sh), F32, kind="ExternalOutput")

    with tile.TileContext(nc) as tc:
        ctx = contextlib.ExitStack()
        with ctx:
            pst = ctx.enter_context(tc.tile_pool(name="ps", bufs=4, space="PSUM"))
            drm = ctx.enter_context(tc.tile_pool(name="dram", bufs=1, space="DRAM"))
            glob = ctx.enter_context(tc.tile_pool(name="glob", bufs=1))

            pid = nc.sync.partition_id()

            def wload(pool, d, shape=None, name=None, dt=F32R):
                sh = shape or d.shape
                t = pool.tile(list(sh), dt, name=name or f"sb_{d.name}")
                nc.sync.dma_start(t[:], d[:].bitcast(dt) if dt is F32R else d[:])
                return t

            # zero tile for G-pad zeroing
            zt = glob.tile([128, 1156], F32)
            nc.vector.memset(zt[:], 0.0)

            # DRAM gather buffers
            c1_d = drm.tile([4, 64, 1156], F32)
            G1 = drm.tile([36, 64, 1156], F32)
            c2_d = drm.tile([2, 96, 324], F32)
            G2 = drm.tile([20, 96, 324], F32)
            c3_d = drm.tile([1, 128, 100], F32)
            G3 = drm.tile([12, 128, 100], F32)
            gpad_insts = []
            for G, csz, npad in ((G1, (64, 1156), 2), (G2, (96, 324), 2),
                                 (G3, (128, 100), 2)):
                n = G.shape[0]
                for s in list(range(npad)) + list(range(n - npad, n)):
                    gpad_insts.append(
                        nc.sync.dma_start(G[s], zt[0:csz[0], 0:csz[1]]))

            # persistent tail tensors (small; cross level boundaries)
            P4a = glob.tile([128, 216], F32); P4b = glob.tile([32, 216], F32)
            P5a = glob.tile([128, 64], F32); P5b = glob.tile([64, 64], F32)
            P6a = glob.tile([128, 27], F32); P6b = glob.tile([96, 27], F32)
            X6a = glob.tile([128, 1], F32); X6b = glob.tile([128, 1], F32)
            outt = glob.tile([128, 2], F32)
            for t in (P4a, P4b, P5a, P5b, P6a, P6b):
                nc.vector.memset(t[:].bitcast(F32), 0.0)

            # ================ LEVEL 0 ================
            with tc.tile_pool(name="l0w", bufs=1) as wp, \
                 tc.tile_pool(name="l0p", bufs=1) as pp, \
                 tc.tile_pool(name="l0s", bufs=2) as ss, \
                 tc.tile_pool(name="l0m", bufs=4) as sm:
                w1col_t = wload(wp, w1col_d)
                w0p_t = wload(wp, w0p_d)
                w0l_t = wload(wp, w0l_d)

                A0 = pp.tile([128, 4, 4356], F32R)
                C0 = pp.tile([64, 2, 4096], F32R)
                P0 = pp.tile([64, 4, 1156], F32R)
                for _s in range(4):
                    nc.vector.memset(A0[:, _s, :].bitcast(F32), 0.0)
                nc.vector.memset(P0[:].bitcast(F32), 0.0)

                def l0_conv1(sl):
                    xs = ss.tile([81, 4096], F32R, tag="x1s")
                    nc.sync.dma_start(xs[:], x1col[sl].bitcast(F32R))
                    for chunk in range(8):
                        ps = pst.tile([64, 512], F32, tag="ps")
                        nc.tensor.matmul(ps[:], w1col_t[:, 0:64],
                                         xs[:, chunk * 512:chunk * 512 + 512],
                                         start=True, stop=True)
                        r0, r1 = sl % 4, (sl - 1) % 4
                        yb = chunk * 8
                        src = ps[:].rearrange("p (a b) -> p a b", b=64)
                        d0 = A0[0:64, r0, :].rearrange("p (a b) -> p a b", b=66)
                        d1 = A0[64:128, r1, :].rearrange("p (a b) -> p a b", b=66)
                        nc.scalar.copy(d0[:, yb + 1:yb + 9, 1:65], src)
                        nc.gpsimd.tensor_copy(d1[:, yb + 1:yb + 9, 1:65],
                                              d0[:, yb + 1:yb + 9, 1:65])

                def l0_conv2(z):
                    # ring r: rows0 = h1[local r mod 4 writer], i.e.
                    # conv1(sl) wrote rows0@sl%4 and rows64@(sl-1)%4.
                    # out z needs h1 locals (z, z+1, z+2); out z+1 one more.
                    rA = z % 4         # rows0=h1[z], rows64=h1[z+1]
                    rB = (z + 1) % 4   # rows0=h1[z+1], rows64=h1[z+2]
                    rD = (z + 3) % 4   # rows0=h1[z+3]
                    for chunk in range(8):
                        yb = chunk * 8
                        psA = pst.tile([64, 512], F32, tag="ps")
                        psB = pst.tile([64, 512], F32, tag="ps")
                        wA = A0[:, rA, :].rearrange("p (a b) -> p a b", b=66)
                        wB = A0[:, rB, :].rearrange("p (a b) -> p a b", b=66)
                        wD = A0[:, rD, :].rearrange("p (a b) -> p a b", b=66)
                        for j, (dy, dx) in enumerate(DYDX):
                            first, last = (j == 0), (j == 8)
                            ys = slice(yb + 1 + dy, yb + 9 + dy)
                            xsl = slice(1 + dx, 65 + dx)
                            vA = psA[:].rearrange("p (a b) -> p a b", b=64)
                            vB = psB[:].rearrange("p (a b) -> p a b", b=64)
                            # K=128 z-pair mms (full rows)
                            nc.tensor.matmul(vA, w0p_t[:, j, :],
                                             wA[:, ys, xsl],
                                             start=first, stop=False)
                            nc.tensor.matmul(vB, w0p_t[:, j, :],
                                             wB[:, ys, xsl],
                                             start=first, stop=False)
                            # K=64 leftovers, row-group paired:
                            # out z reads h1[z+1] at rows64 of rB;
                            # out z+1 reads h1[z+2] at rows0 of rD.
                            nc.tensor.matmul(vA, w0l_t[64:128, j, :],
                                             wB[64:128, ys, xsl],
                                             start=False, stop=last)
                            nc.tensor.matmul(vB, w0l_t[0:64, j, :],
                                             wD[0:64, ys, xsl],
                                             start=False, stop=last)
                        for ps_, zz, h in ((psA, z, 0), (psB, z + 1, 1)):
                            mt = sm.tile([64, 512], F32, tag="mn0")
                            nc.sync.dma_start(
                                mt[:], mn0_d[zz, yb * 64:yb * 64 + 512]
                                .unsqueeze(0).to_broadcast((64, 512)))
                            nc.vector.tensor_add(
                                C0[:, h, yb * 64:yb * 64 + 512], ps_[:], mt[:])

                def l0_pool(z):
                    zp = z // 2
                    nc.vector.tensor_max(C0[:, 0, :], C0[:, 0, :], C0[:, 1, :])
                    v = C0[:, 0, :].rearrange("p (a b) -> p a b", b=64)
                    t2 = ss.tile([64, 32, 64], F32R, tag="pool0b", bufs=1)
                    nc.vector.tensor_max(t2[:], v[:, 0::2, :], v[:, 1::2, :])
                    t3 = ss.tile([64, 32, 32], F32R, tag="pool0c", bufs=1)
                    nc.vector.tensor_max(t3[:], t2[:, :, 0::2], t2[:, :, 1::2])
                    mt = sm.tile([64, 1024], F32, tag="m1p", bufs=2)
                    nc.sync.dma_start(mt[:], m1p_d[zp].unsqueeze(0)
                                      .to_broadcast((64, 1024)))
                    dst = P0[:, zp, :].rearrange("p (a b) -> p a b", b=34)
                    nc.vector.tensor_mul(
                        dst[:, 1:33, 1:33], t3[:],
                        mt[:].rearrange("p (a b) -> p a b", b=32))

                for sl in range(10):
                    l0_conv1(sl)
                    if sl >= 3 and (sl - 3) % 2 == 0:
                        zz = sl - 3
                        l0_conv2(zz)
                        l0_pool(zz)

                if DBG:
                    nc.sync.dma_start(dbg_d["dP0"][:], P0[:].bitcast(F32))
                    nc.sync.dma_start(dbg_d["dC0"][:], C0[:].bitcast(F32))
                nc.sync.dma_start(c1_d[:].rearrange("z c v -> c z v"),
                                  P0[:].bitcast(F32))

            # ---- AllGather L0 -> L1 ----
            ag1 = nc.gpsimd.collective_compute(
                "AllGather", mybir.AluOpType.bypass,
                replica_groups=[list(range(NC))],
                ins=[c1_d[:].opt()], outs=[G1[2:34].opt()])
            for gi in gpad_insts:
                add_dep_helper(ag1.ins, gi.ins, reason="G pads zeroed before gathers")

            # ================ LEVEL 1 ================
            with tc.tile_pool(name="l1w", bufs=1) as wp, \
                 tc.tile_pool(name="l1p", bufs=1) as pp, \
                 tc.tile_pool(name="l1s", bufs=2) as ss, \
                 tc.tile_pool(name="l1m", bufs=4) as sm:
                w1p_t = wload(wp, w1p_d)
                w1l_t = wload(wp, w1l_d)
                w1c2_t = wload(wp, genw_d["w1c2"][0])

                A1 = pp.tile([128, 8, 1156], F32R)
                B1 = pp.tile([96, 6, 1156], F32R)
                C1 = pp.tile([96, 4, 1024], F32R)
                P1 = pp.tile([96, 2, 324], F32R)
                nc.vector.memset(B1[:].bitcast(F32), 0.0)
                nc.vector.memset(P1[:].bitcast(F32), 0.0)
                _r1 = nc.sync.dma_start(A1[0:64, :, :],
                                  G1[bass.ds(pid * 4, 8)].rearrange("z c v -> c z v").bitcast(F32R))
                _r2 = nc.sync.dma_start(A1[64:128, 0:7, :],
                                  G1[bass.ds(pid * 4 + 1, 7)].rearrange("z c v -> c z v").bitcast(F32R))
                add_dep_helper(_r1.ins, ag1.ins, reason="gather before dynamic read")
                add_dep_helper(_r2.ins, ag1.ins, reason="gather before dynamic read")

                def l1_conv1(sl):
                    # A1 rows0 idx i = x1[4k-2+i]; rows64 idx i = x1[4k-1+i].
                    # out sl (global 4k-1+sl): pair = A1[:, sl] (dz=-1,0);
                    # leftover dz=+1 = rows64 idx sl+1 == rows0 idx sl+2.
                    mt = sm.tile([96, 1024], F32, tag="m1mul")
                    nc.sync.dma_start(mt[:], m1mul_d[sl].unsqueeze(0)
                                      .to_broadcast((96, 1024)))
                    pss = [pst.tile([96, 512], F32, tag="ps", name=f"ps_l1_{sl}_{_c}") for _c in range(2)]
                    wA = A1[:, sl, :].rearrange("p (a b) -> p a b", b=34)
                    wB = A1[64:128, sl + 1, :].rearrange("p (a b) -> p a b", b=34)
                    wC = A1[0:64, sl + 2, :].rearrange("p (a b) -> p a b", b=34)
                    for j, (dy, dx) in enumerate(DYDX):
                        xsl = slice(1 + dx, 33 + dx)
                        for chunk in range(2):
                            yb = chunk * 16
                            ys = slice(yb + 1 + dy, yb + 17 + dy)
                            nc.tensor.matmul(
                                pss[chunk][:].rearrange("p (a b) -> p a b", b=32),
                                w1p_t[:, j, :], wA[:, ys, xsl],
                                start=(j == 0), stop=False)
                        # row-paired leftovers: chunk0 on rows 64:128,
                        # chunk1 on rows 0:64 (concurrent row groups)
                        ys0 = slice(1 + dy, 17 + dy)
                        ys1 = slice(17 + dy, 33 + dy)
                        nc.tensor.matmul(
                            pss[0][:].rearrange("p (a b) -> p a b", b=32),
                            w1l_t[64:128, j, :], wB[:, ys0, xsl],
                            start=False, stop=(j == 8))
                        nc.tensor.matmul(
                            pss[1][:].rearrange("p (a b) -> p a b", b=32),
                            w1l_t[0:64, j, :], wC[:, ys1, xsl],
                            start=False, stop=(j == 8))
                    for chunk in range(2):
                        yb = chunk * 16
                        dst = B1[:, sl, :].rearrange("p (a b) -> p a b", b=34)
                        nc.vector.tensor_mul(
                            dst[:, yb + 1:yb + 17, 1:33],
                            pss[chunk][:].rearrange("p (a b) -> p a b", b=32),
                            mt[:, yb * 32:yb * 32 + 512].rearrange(
                                "p (a b) -> p a b", b=32))

                def l1_conv2(sl):
                    mt = sm.tile([96, 1024], F32, tag="mn1")
                    nc.sync.dma_start(mt[:], mn1_d[sl].unsqueeze(0)
                                      .to_broadcast((96, 1024)))
                    for chunk in range(2):
                        yb = chunk * 16
                        ps = pst.tile([96, 512], F32, tag="ps")
                        for o, (dz, dy, dx) in enumerate(OFFSETS):
                            w = B1[:, sl + 1 + dz, :].rearrange(
                                "p (a b) -> p a b", b=34)
                            nc.tensor.matmul(
                                ps[:].rearrange("p (a b) -> p a b", b=32),
                                w1c2_t[:, o, :],
                                w[:, yb + 1 + dy:yb + 17 + dy, 1 + dx:33 + dx],
                                start=(o == 0), stop=(o == 26))
                        nc.vector.tensor_add(C1[:, sl, yb * 32:yb * 32 + 512],
                                             ps[:],
                                             mt[:, yb * 32:yb * 32 + 512])

                def l1_pool(zz):
                    zp = zz // 2
                    nc.vector.tensor_max(C1[:, zz, :], C1[:, zz, :], C1[:, zz + 1, :])
                    v = C1[:, zz, :].rearrange("p (a b) -> p a b", b=32)
                    t2 = ss.tile([96, 16, 32], F32R, tag="pool1b")
                    nc.vector.tensor_max(t2[:], v[:, 0::2, :], v[:, 1::2, :])
                    t3 = ss.tile([96, 16, 16], F32R, tag="pool1c")
                    nc.vector.tensor_max(t3[:], t2[:, :, 0::2], t2[:, :, 1::2])
                    mt = sm.tile([96, 256], F32, tag="m2p")
                    nc.sync.dma_start(mt[:], m2p_d[zp].unsqueeze(0)
                                      .to_broadcast((96, 256)))
                    dst = P1[:, zp, :].rearrange("p (a b) -> p a b", b=18)
                    nc.vector.tensor_mul(
                        dst[:, 1:17, 1:17], t3[:],
                        mt[:].rearrange("p (a b) -> p a b", b=16))

                for sl in range(6):
                    l1_conv1(sl)
                    if sl >= 2:
                        l1_conv2(sl - 2)
                        if sl >= 3 and (sl - 3) % 2 == 0:
                            l1_pool(sl - 3)

                if DBG:
                    nc.sync.dma_start(dbg_d["dA1"][:], A1[:].bitcast(F32))
                    nc.sync.dma_start(dbg_d["dB1"][:], B1[:].bitcast(F32))
                    nc.sync.dma_start(dbg_d["dC1"][:], C1[:].bitcast(F32))
                    nc.sync.dma_start(dbg_d["dP1"][:], P1[:].bitcast(F32))
                nc.sync.dma_start(c2_d[:].rearrange("z c v -> c z v"),
                                  P1[:].bitcast(F32))

            # ---- AllGather L1 -> L2 ----
            ag2 = nc.gpsimd.collective_compute(
                "AllGather", mybir.AluOpType.bypass,
                replica_groups=[list(range(NC))],
                ins=[c2_d[:].opt()], outs=[G2[2:18].opt()])
            for gi in gpad_insts:
                add_dep_helper(ag2.ins, gi.ins, reason="G pads zeroed before gathers")

            # ================ LEVEL 2 ================
            with tc.tile_pool(name="l2w", bufs=1) as wp, \
                 tc.tile_pool(name="l2p", bufs=1) as pp, \
                 tc.tile_pool(name="l2s", bufs=2) as ss, \
                 tc.tile_pool(name="l2m", bufs=4) as sm:
                w2c1_t = wload(wp, genw_d["w2c1"][0])
                w2c2_t = wload(wp, genw_d["w2c2"][0])
                A2 = pp.tile([96, 6, 324], F32R)
                B2 = pp.tile([128, 4, 324], F32R)
                C2 = pp.tile([128, 2, 256], F32R)
                P2 = pp.tile([128, 1, 100], F32R)
                nc.vector.memset(B2[:].bitcast(F32), 0.0)
                nc.vector.memset(P2[:].bitcast(F32), 0.0)
                _r3 = nc.sync.dma_start(A2[:], G2[bass.ds(pid * 2, 6)].rearrange("z c v -> c z v").bitcast(F32R))
                add_dep_helper(_r3.ins, ag2.ins, reason="gather before dynamic read")

                for s0 in (0, 2):
                    ps = pst.tile([128, 512], F32, tag="ps")
                    for o, (dz, dy, dx) in enumerate(OFFSETS):
                        w = A2[:].rearrange("p z (a b) -> p z a b", b=18)
                        nc.tensor.matmul(
                            ps[:].rearrange("p (z a b) -> p z a b", z=2, a=16),
                            w2c1_t[:, o, :],
                            w[:, s0 + dz + 1:s0 + dz + 3,
                              1 + dy:17 + dy, 1 + dx:17 + dx],
                            start=(o == 0), stop=(o == 26))
                    mt = sm.tile([128, 512], F32, tag="m2mul")
                    nc.sync.dma_start(
                        mt[:], m2mul_d[s0:s0 + 2].flatten().unsqueeze(0)
                        .to_broadcast((128, 512)))
                    dst = B2[:].rearrange("p z (a b) -> p z a b", b=18)
                    nc.vector.tensor_mul(
                        dst[:, s0:s0 + 2, 1:17, 1:17],
                        ps[:].rearrange("p (z a b) -> p z a b", z=2, a=16),
                        mt[:].rearrange("p (z a b) -> p z a b", z=2, a=16))

                ps = pst.tile([128, 512], F32, tag="ps")
                for o, (dz, dy, dx) in enumerate(OFFSETS):
                    w = B2[:].rearrange("p z (a b) -> p z a b", b=18)
                    nc.tensor.matmul(
                        ps[:].rearrange("p (z a b) -> p z a b", z=2, a=16),
                        w2c2_t[:, o, :],
                        w[:, dz + 1:dz + 3, 1 + dy:17 + dy, 1 + dx:17 + dx],
                        start=(o == 0), stop=(o == 26))
                if mf[2]:
                    mt = sm.tile([128, 512], F32, tag="mn2")
                    nc.sync.dma_start(mt[:], mn2_d[:].flatten().unsqueeze(0)
                                      .to_broadcast((128, 512)))
                    nc.vector.tensor_add(C2[:].rearrange("p a b -> p (a b)"),
                                         ps[:], mt[:])
                else:
                    nc.scalar.copy(C2[:].rearrange("p a b -> p (a b)"), ps[:])

                # L2 pool
                nc.vector.tensor_max(C2[:, 0, :], C2[:, 0, :], C2[:, 1, :])
                v = C2[:, 0, :].rearrange("p (a b) -> p a b", b=16)
                t2 = ss.tile([128, 8, 16], F32R, tag="pool2b")
                nc.vector.tensor_max(t2[:], v[:, 0::2, :], v[:, 1::2, :])
                dst = P2[:, 0, :].rearrange("p (a b) -> p a b", b=10)
                if mf[3]:
                    t3 = ss.tile([128, 8, 8], F32R, tag="pool2c")
                    nc.vector.tensor_max(t3[:], t2[:, :, 0::2], t2[:, :, 1::2])
                    mt = sm.tile([128, 64], F32, tag="m3p")
                    nc.sync.dma_start(mt[:], m3p_d[0].unsqueeze(0)
                                      .to_broadcast((128, 64)))
                    nc.vector.tensor_mul(
                        dst[:, 1:9, 1:9], t3[:],
                        mt[:].rearrange("p (a b) -> p a b", b=8))
                else:
                    nc.vector.tensor_max(dst[:, 1:9, 1:9],
                                         t2[:, :, 0::2], t2[:, :, 1::2])

                if DBG:
                    nc.sync.dma_start(dbg_d["dA2"][:], A2[:].bitcast(F32))
                    nc.sync.dma_start(dbg_d["dB2"][:], B2[:].bitcast(F32))
                    nc.sync.dma_start(dbg_d["dC2"][:], C2[:].bitcast(F32))
                    nc.sync.dma_start(dbg_d["dP2"][:], P2[:].bitcast(F32))
                nc.sync.dma_start(c3_d[:].rearrange("z c v -> c z v"),
                                  P2[:].bitcast(F32))

            # ---- AllGather L2 -> L3 ----
            ag3 = nc.gpsimd.collective_compute(
                "AllGather", mybir.AluOpType.bypass,
                replica_groups=[list(range(NC))],
                ins=[c3_d[:].opt()], outs=[G3[2:10].opt()])
            for gi in gpad_insts:
                add_dep_helper(ag3.ins, gi.ins, reason="G pads zeroed before gathers")

            # ================ LEVEL 3 (replicated) ================
            with tc.tile_pool(name="l3w", bufs=1) as wp, \
                 tc.tile_pool(name="l3p", bufs=1) as pp, \
                 tc.tile_pool(name="l3s", bufs=2) as ss, \
                 tc.tile_pool(name="l3m", bufs=4) as sm:
                w3c1_t = wload(wp, genw_d["w3c1"][0])
                w3c2_t = [wload(wp, d) for d in genw_d["w3c2"]]
                A3 = pp.tile([128, 12, 100], F32R)
                B3a = pp.tile([128, 10, 100], F32R)
                B3b = pp.tile([32, 10, 100], F32R)
                C3a = pp.tile([128, 512], F32R)
                C3b = pp.tile([32, 512], F32R)
                nc.vector.memset(B3a[:].bitcast(F32), 0.0)
                nc.vector.memset(B3b[:].bitcast(F32), 0.0)
                _r4 = nc.sync.dma_start(A3[:], G3[:].rearrange("z c v -> c z v").bitcast(F32R))
                add_dep_helper(_r4.ins, ag3.ins, reason="gather before read")

                # conv1
                for (z0, nz) in ((0, 8), (2, 8)):
                    N = nz * 64
                    for (c0, co_n) in ((0, 128), (128, 32)):
                        ps = pst.tile([co_n, 512], F32, tag="ps")
                        for o, (dz, dy, dx) in enumerate(OFFSETS):
                            w = A3[:].rearrange("p z (a b) -> p z a b", b=10)
                            nc.tensor.matmul(
                                ps[:, 0:N].rearrange(
                                    "p (z a b) -> p z a b", z=nz, a=8),
                                w3c1_t[:, o, c0:c0 + co_n],
                                w[:, z0 + dz + 1:z0 + dz + 1 + nz,
                                  1 + dy:9 + dy, 1 + dx:9 + dx],
                                start=(o == 0), stop=(o == 26))
                        mt = sm.tile([co_n, 512], F32, tag="m3mul")
                        nc.sync.dma_start(
                            mt[:, 0:N],
                            m3mul_d[z0:z0 + nz].flatten().unsqueeze(0)
                            .to_broadcast((co_n, N)))
                        B3 = B3a if c0 == 0 else B3b
                        dst = B3[:].rearrange("p z (a b) -> p z a b", b=10)
                        nc.vector.tensor_mul(
                            dst[:, z0:z0 + nz, 1:9, 1:9],
                            ps[:, 0:N].rearrange(
                                "p (z a b) -> p z a b", z=nz, a=8),
                            mt[:, 0:N].rearrange(
                                "p (z a b) -> p z a b", z=nz, a=8))

                # conv2
                for (c0, co_n) in ((0, 128), (128, 32)):
                    ps = pst.tile([co_n, 512], F32, tag="ps")
                    for o, (dz, dy, dx) in enumerate(OFFSETS):
                        for ki, B3 in enumerate((B3a, B3b)):
                            w = B3[:].rearrange("p z (a b) -> p z a b", b=10)
                            nc.tensor.matmul(
                                ps[:].rearrange("p (z a b) -> p z a b",
                                                z=8, a=8),
                                w3c2_t[ki][:, o, c0:c0 + co_n],
                                w[:, dz + 1:dz + 9, 1 + dy:9 + dy,
                                  1 + dx:9 + dx],
                                start=(o == 0 and ki == 0),
                                stop=(o == 26 and ki == 1))
                    C3 = C3a if c0 == 0 else C3b
                    if mf[3]:
                        mt = sm.tile([co_n, 512], F32, tag="mn3")
                        nc.sync.dma_start(mt[:], mn3_d[:].flatten().unsqueeze(0)
                                          .to_broadcast((co_n, 512)))
                        nc.vector.tensor_add(C3[:], ps[:], mt[:])
                    else:
                        nc.scalar.copy(C3[:], ps[:])

                # pool -> P4
                for C3, P4, cn in ((C3a, P4a, 128), (C3b, P4b, 32)):
                    v = C3[:].rearrange("p (z v) -> p z v", v=64)
                    t1 = ss.tile([cn, 4, 64], F32R, tag="pool3a")
                    nc.vector.tensor_max(t1[:], v[:, 0::2, :], v[:, 1::2, :])
                    u = t1[:].rearrange("p z (a b) -> p z a b", b=8)
                    t2 = ss.tile([cn, 4, 4, 8], F32R, tag="pool3b")
                    nc.vector.tensor_max(t2[:], u[:, :, 0::2, :],
                                         u[:, :, 1::2, :])
                    dst = P4[:].rearrange("p (z a b) -> p z a b", z=6, a=6)
                    if mf[4]:
                        t3 = ss.tile([cn, 4, 4, 4], F32R, tag="pool3c")
                        nc.vector.tensor_max(t3[:], t2[:, :, :, 0::2],
                                             t2[:, :, :, 1::2])
                        mt = sm.tile([cn, 64], F32, tag="m4p")
                        nc.sync.dma_start(mt[:], m4p_d[:].flatten().unsqueeze(0)
                                          .to_broadcast((cn, 64)))
                        nc.vector.tensor_mul(
                            dst[:, 1:5, 1:5, 1:5], t3[:],
                            mt[:].rearrange("p (z a b) -> p z a b", z=4, a=4))
                    else:
                        nc.vector.tensor_max(dst[:, 1:5, 1:5, 1:5],
                                             t2[:, :, :, 0::2],
                                             t2[:, :, :, 1::2])

                if DBG:
                    nc.sync.dma_start(dbg_d["dA3"][:], A3[:].bitcast(F32))
                    nc.sync.dma_start(dbg_d["dB3a"][:], B3a[:].bitcast(F32))
                    nc.sync.dma_start(dbg_d["dC3a"][:], C3a[:].bitcast(F32))

            # ================ TAIL (levels 4-6, replicated) ================
            def tail_conv(sm, wts, ins, outs, pg, og, mode, mdram, mname):
                N = og * og * og
                noff = wts[0].shape[1]
                offs = OFFSETS if noff == 27 else [(0, 0, 0)]
                for (ot, c0, co_n, padded) in outs:
                    ps = pst.tile([co_n, max(N, 8)], F32, tag="ps")
                    nmm = len(offs) * len(ins)
                    i = 0
                    for o, (dz, dy, dx) in enumerate(offs):
                        for ki, it in enumerate(ins):
                            w = it[:].rearrange("p (z a b) -> p z a b",
                                                z=pg, a=pg)
                            nc.tensor.matmul(
                                ps[:, 0:N].rearrange(
                                    "p (z a b) -> p z a b", z=og, a=og),
                                wts[ki][:, o, c0:c0 + co_n],
                                w[:, 1 + dz:1 + dz + og, 1 + dy:1 + dy + og,
                                  1 + dx:1 + dx + og],
                                start=(i == 0), stop=(i == nmm - 1))
                            i += 1
                    if padded:
                        opg = og + 2
                        dst = ot[:].rearrange("p (z a b) -> p z a b",
                                              z=opg, a=opg)[:, 1:1 + og,
                                                            1:1 + og, 1:1 + og]
                    else:
                        dst = ot[:, 0:N].rearrange("p (z a b) -> p z a b",
                                                   z=og, a=og)
                    src = ps[:, 0:N].rearrange("p (z a b) -> p z a b",
                                               z=og, a=og)
                    if mode == "copy":
                        nc.scalar.copy(dst, src)
                    else:
                        mt = sm.tile([co_n, N], F32, tag=mname)
                        nc.sync.dma_start(
                            mt[:], mdram[:].flatten().unsqueeze(0)
                            .to_broadcast((co_n, N)))
                        mm = mt[:].rearrange("p (z a b) -> p z a b", z=og, a=og)
                        if mode == "mul":
                            nc.vector.tensor_mul(dst, src, mm)
                        else:
                            nc.vector.tensor_add(dst, src, mm)

            def tail_pool(sm, ss, cs, ps_out, g, has_mask, mdram):
                go = g // 2
                for (ct, cn), (pt, _) in zip(cs, ps_out):
                    v = ct[:, 0:g * g * g].rearrange("p (z v) -> p z v",
                                                     v=g * g)
                    t1 = ss.tile([cn, go, g * g], F32, tag=f"tp{g}a")
                    nc.vector.tensor_max(t1[:], v[:, 0::2, :], v[:, 1::2, :])
                    u = t1[:].rearrange("p z (a b) -> p z a b", b=g)
                    t2 = ss.tile([cn, go, go, g], F32, tag=f"tp{g}b")
                    nc.vector.tensor_max(t2[:], u[:, :, 0::2, :],
                                         u[:, :, 1::2, :])
                    gp = go + 2
                    dst = pt[:].rearrange("p (z a b) -> p z a b", z=gp, a=gp)
                    if has_mask:
                        t3 = ss.tile([cn, go, go, go], F32, tag=f"tp{g}c")
                        nc.vector.tensor_max(t3[:], t2[:, :, :, 0::2],
                                             t2[:, :, :, 1::2])
                        mt = sm.tile([cn, go * go * go], F32, tag=f"tp{g}m")
                        nc.sync.dma_start(
                            mt[:], mdram[:].flatten().unsqueeze(0)
                            .to_broadcast((cn, go * go * go)))
                        nc.vector.tensor_mul(
                            dst[:, 1:1 + go, 1:1 + go, 1:1 + go], t3[:],
                            mt[:].rearrange("p (z a b) -> p z a b",
                                            z=go, a=go))
                    else:
                        nc.vector.tensor_max(
                            dst[:, 1:1 + go, 1:1 + go, 1:1 + go],
                            t2[:, :, :, 0::2], t2[:, :, :, 1::2])

            # ---- L4 ----
            with tc.tile_pool(name="l4w", bufs=1) as wp, \
                 tc.tile_pool(name="l4p", bufs=1) as pp, \
                 tc.tile_pool(name="l4s", bufs=2) as ss, \
                 tc.tile_pool(name="l4m", bufs=2) as sm:
                w4c1_t = [wload(wp, d, dt=F32) for d in genw_d["w4c1"]]
                w4c2_t = [wload(wp, d, dt=F32) for d in genw_d["w4c2"]]
                B4a = pp.tile([128, 216], F32); B4b = pp.tile([64, 216], F32)
                C4a = pp.tile([128, 64], F32); C4b = pp.tile([64, 64], F32)
                nc.vector.memset(B4a[:].bitcast(F32), 0.0)
                nc.vector.memset(B4b[:].bitcast(F32), 0.0)
                tail_conv(sm, w4c1_t, [P4a, P4b],
                          [(B4a, 0, 128, True), (B4b, 128, 64, True)], 6, 4,
                          "mul" if mf[4] else "copy", m4mul_d, "m4mul")
                tail_conv(sm, w4c2_t, [B4a, B4b],
                          [(C4a, 0, 128, False), (C4b, 128, 64, False)], 6, 4,
                          "add" if mf[4] else "copy", mn4_d, "mn4")
                tail_pool(sm, ss, [(C4a, 128), (C4b, 64)],
                          [(P5a, 128), (P5b, 64)], 4, mf[5], m5p_d)

                if DBG:
                    nc.sync.dma_start(dbg_d["dP4a"][:], P4a[:])
                    nc.sync.dma_start(dbg_d["dB4a"][:], B4a[:])
                    nc.sync.dma_start(dbg_d["dC4a"][:], C4a[:])

            # ---- L5 ----
            with tc.tile_pool(name="l5w", bufs=1) as wp, \
                 tc.tile_pool(name="l5p", bufs=1) as pp, \
                 tc.tile_pool(name="l5s", bufs=2) as ss, \
                 tc.tile_pool(name="l5m", bufs=2) as sm:
                w5c1_t = [wload(wp, d, dt=F32) for d in genw_d["w5c1"]]
                w5c2_t = [wload(wp, d, dt=F32) for d in genw_d["w5c2"]]
                B5a = pp.tile([128, 64], F32); B5b = pp.tile([96, 64], F32)
                C5a = pp.tile([128, 8], F32); C5b = pp.tile([96, 8], F32)
                nc.vector.memset(B5a[:].bitcast(F32), 0.0)
                nc.vector.memset(B5b[:].bitcast(F32), 0.0)
                tail_conv(sm, w5c1_t, [P5a, P5b],
                          [(B5a, 0, 128, True), (B5b, 128, 96, True)], 4, 2,
                          "mul" if mf[5] else "copy", m5mul_d, "m5mul")
                tail_conv(sm, w5c2_t, [B5a, B5b],
                          [(C5a, 0, 128, False), (C5b, 128, 96, False)], 4, 2,
                          "add" if mf[5] else "copy", mn5_d, "mn5")
                tail_pool(sm, ss, [(C5a, 128), (C5b, 96)],
                          [(P6a, 128), (P6b, 96)], 2, mf[6], m6p_d)

                if DBG:
                    nc.sync.dma_start(dbg_d["dP5a"][:], P5a[:])
                    nc.sync.dma_start(dbg_d["dB5a"][:], B5a[:])
                    nc.sync.dma_start(dbg_d["dP6a"][:], P6a[:])

            # ---- L6 (1^3, center tap only) ----
            with tc.tile_pool(name="l6w", bufs=1) as wp:
                w6c1_t = [wload(wp, d, dt=F32) for d in genw_d["w6c1"]]
                w6c2_t = [wload(wp, d, dt=F32) for d in genw_d["w6c2"]]
                for (ot, c0) in ((X6a, 0), (X6b, 128)):
                    ps = pst.tile([128, 8], F32, tag="ps")
                    nc.tensor.matmul(ps[:, 0:1], w6c1_t[0][:, 0, c0:c0 + 128],
                                     P6a[:, 13:14], start=True, stop=False)
                    nc.tensor.matmul(ps[:, 0:1], w6c1_t[1][:, 0, c0:c0 + 128],
                                     P6b[:, 13:14], start=False, stop=True)
                    nc.vector.tensor_copy(ot[:], ps[:, 0:1])
                for i, c0 in enumerate((0, 128)):
                    ps = pst.tile([128, 8], F32, tag="ps")
                    nc.tensor.matmul(ps[:, 0:1], w6c2_t[0][:, 0, c0:c0 + 128],
                                     X6a[:], start=True, stop=False)
                    nc.tensor.matmul(ps[:, 0:1], w6c2_t[1][:, 0, c0:c0 + 128],
                                     X6b[:], start=False, stop=True)
                    nc.scalar.copy(outt[:, i:i + 1], ps[:, 0:1])
            if DBG:
                nc.sync.dma_start(dbg_d["dX6a"][:], X6a[:])
            nc.sync.dma_start(out_d[0, 0:128], outt[:, 0])
            nc.sync.dma_start(out_d[0, 128:256], outt[:, 1])

    nc.compile()
    return nc



_CACHE = {}


def kernel(features, coors, W0, W1, W2, W3, W4, W5, W6, W7, W8, W9, W10, W11,
           W12, W13):
    features = np.asarray(features, np.float32)
    coors = np.asarray(coors, np.int32)
    Ws = [np.asarray(w, np.float32) for w in
          (W0, W1, W2, W3, W4, W5, W6, W7, W8, W9, W10, W11, W12, W13)]
    in_maps, meta = build_host_inputs(features, coors, Ws)
    key = tuple(sorted(meta["mask_flags"].items()))
    if key not in _CACHE:
        _CACHE[key] = build_kernel(meta)
    nc = _CACHE[key]
    res = run_bass_kernel_spmd(nc, in_maps, core_ids=list(range(NC)))
    out = res.results[0]["out"].reshape(256)
    return out.reshape(1, 1, 1, 1, 256).astype(np.float32)


if __name__ == "__main__":
    pass



# revision 3
# speedup vs baseline: 87.7390x; 87.7390x over previous
"""Trainium2 Bass kernel for the sparse submanifold 3D CNN (nn_Net_38963943309313).

Network: 7 blocks of 2 submanifold 3x3x3 convs on a 64^3 grid, 2x2x2 sparse
max-pools between blocks, channels 3->64->...->256, output [1,1,1,1,256].

Strategy (8 NeuronCores):
 - Shard z-slabs across cores for levels 0-2 (grids 64/32/16), AllGather the
   pooled activations between levels (z-padded gather buffers so per-core
   reads are a single dynamic-offset DMA). Levels 3-6 (grids 8/4/2/1) are
   replicated on every core.
 - Convs are fp32r matmuls: activations channel-major [C, z, y, x] in SBUF
   (y/x zero-padded), 27 shifted-window matmuls accumulated in PSUM.
 - conv1 of block 0 uses a host-side im2col (81 contract rows, masked
   columns so the submanifold mask is free).
 - 64-channel contractions (L0 conv2, L1 conv1) pack z-pairs into K=128 via
   duplicated storage; L0 conv2 additionally pairs two output slices into
   the two 64-column halves of the PE array.
 - Submanifold masking: conv1 evictions multiply by a broadcast mask (also
   zeroes the out-of-grid halo slices); conv2 evictions add (mask-1)*BIG so
   the following max-pool ignores inactive voxels; pool result is multiplied
   by the pooled mask.
"""

import sys

sys.path.insert(0, "/opt/trn_rl_repo")

import numpy as np
import concourse.bass as bass
import concourse.tile as tile
from concourse.tile import add_dep_helper
from concourse import bacc, mybir
from concourse.bass_utils import run_bass_kernel_spmd


class _Runner:
    """Compile-once, inputs-resident executor.

    Replicates concourse.bass2jax.run_bass_via_pjrt's lowering, but keeps the
    jitted shard_map executable and the device-placed input buffers alive
    across calls, so repeat executions cost only the PJRT dispatch + the NEFF
    execution itself (run_bass_kernel_spmd rebuilds the jit closure and
    re-uploads every input on each call).
    """

    def __init__(self, nc, n_cores):
        import jax
        from jax.experimental.shard_map import shard_map
        from jax.sharding import Mesh, PartitionSpec
        from concourse import bass2jax

        bass2jax.install_neuronx_cc_hook()
        self.jax = jax
        self.nc, self.n = nc, n_cores
        partition_name = (nc.partition_id_tensor.name
                          if nc.partition_id_tensor else None)
        in_names, out_names, out_avals = [], [], []
        for alloc in nc.m.functions[0].allocations:
            if not isinstance(alloc, mybir.MemoryLocationSet):
                continue
            name = alloc.memorylocations[0].name
            if alloc.kind == "ExternalInput":
                if name != partition_name:
                    in_names.append(name)
            elif alloc.kind == "ExternalOutput":
                out_names.append(name)
                out_avals.append(jax.core.ShapedArray(
                    tuple(alloc.tensor_shape), mybir.dt.np(alloc.dtype)))
        self.param_names = list(in_names)
        self.out_names, self.out_avals = out_names, out_avals
        self.dbg_name = nc.dbg_addr.name if nc.dbg_addr is not None else None
        full_names = (in_names + out_names
                      + ([partition_name] if partition_name else []))
        n_params, n_outs = len(in_names), len(out_names)

        def _body(*args):
            operands = list(args)
            if partition_name is not None:
                operands.append(bass2jax.partition_id_tensor())
            outs = bass2jax._bass_exec_p.bind(
                *operands,
                out_avals=tuple(out_avals),
                in_names=tuple(full_names),
                out_names=tuple(out_names),
                lowering_input_output_aliases=(),
                sim_require_finite=True,
                sim_require_nnan=True,
                nc=nc,
            )
            return tuple(outs)

        devices = jax.devices()[:n_cores]
        self.mesh = Mesh(np.asarray(devices), ("core",))
        in_specs = (PartitionSpec("core"),) * (n_params + n_outs)
        out_specs = (PartitionSpec("core"),) * n_outs
        self._fn = jax.jit(
            shard_map(_body, mesh=self.mesh, in_specs=in_specs,
                      out_specs=out_specs, check_rep=False),
            donate_argnums=tuple(range(n_params, n_params + n_outs)),
            keep_unused=True,
        )
        self._dev_in = None

    def place(self, in_maps):
        from jax.sharding import NamedSharding, PartitionSpec
        if self.dbg_name is not None:
            in_maps = [{**m, self.dbg_name: np.zeros((1, 2), np.uint32)}
                       for m in in_maps]
        sh = NamedSharding(self.mesh, PartitionSpec("core"))
        concat = [np.concatenate([np.asarray(m[name]) for m in in_maps], 0)
                  for name in self.param_names]
        self._dev_in = [self.jax.device_put(a, sh) for a in concat]
        self.jax.block_until_ready(self._dev_in)

    def run(self):
        zeros = [np.zeros((self.n * av.shape[0], *av.shape[1:]), av.dtype)
                 for av in self.out_avals]
        outs = self._fn(*self._dev_in, *zeros)
        return self.jax.block_until_ready(outs)

    def result(self, outs, name, core=0):
        i = self.out_names.index(name)
        av = self.out_avals[i]
        return np.asarray(outs[i]).reshape(self.n, *av.shape)[core]

NC = 8
GRID = 64
BIG = 1.0e30
CHANNELS = [(3, 64), (64, 64), (64, 96), (96, 96), (96, 128), (128, 128),
            (128, 160), (160, 160), (160, 192), (192, 192), (192, 224),
            (224, 224), (224, 256), (256, 256)]
F32 = mybir.dt.float32
F32R = mybir.dt.float32r

OFFSETS = [(dz, dy, dx) for dz in (-1, 0, 1) for dy in (-1, 0, 1) for dx in (-1, 0, 1)]
# 9 (dy,dx) pairs for z-pair-packed layers
DYDX = [(dy, dx) for dy in (-1, 0, 1) for dx in (-1, 0, 1)]


def _pool_np(x, m):
    # x: [D,D,D,C] or [D,D,D]; max over active voxels of 2x2x2 windows
    D = x.shape[0]
    if x.ndim == 3:
        xr = x.reshape(D // 2, 2, D // 2, 2, D // 2, 2)
        return xr.max(axis=(1, 3, 5))
    neg = np.where(m[..., None] > 0, x, -np.inf)
    xr = neg.reshape(D // 2, 2, D // 2, 2, D // 2, 2, -1)
    p = xr.max(axis=(1, 3, 5))
    mp = m.reshape(D // 2, 2, D // 2, 2, D // 2, 2).max(axis=(1, 3, 5))
    return np.where(mp[..., None] > 0, p, 0.0), mp


def _ceil_div(a, b):
    return (a + b - 1) // b


def build_host_inputs(features, coors, Ws):
    """All host-side data marshalling. Returns (in_maps, meta)."""
    z, y, x = coors[:, 0], coors[:, 1], coors[:, 2]
    dense = np.zeros((GRID, GRID, GRID, 3), np.float32)
    mask0 = np.zeros((GRID, GRID, GRID), np.float32)
    dense[z, y, x] = features  # last write wins (matches XLA CPU scatter)
    mask0[z, y, x] = 1.0

    # mask pyramid
    masks = [mask0]
    m = mask0
    for _ in range(6):
        mr = m.reshape(m.shape[0] // 2, 2, m.shape[1] // 2, 2, m.shape[2] // 2, 2)
        m = mr.max(axis=(1, 3, 5))
        masks.append(m)

    # ---- X1col: host im2col for conv1 of block 0, column-masked ----
    # padded dense [3, 66, 66, 66]
    dpad = np.zeros((3, GRID + 2, GRID + 2, GRID + 2), np.float32)
    dpad[:, 1:-1, 1:-1, 1:-1] = dense.transpose(3, 0, 1, 2)
    # X1col_full[(off*3+ci), zglob, y, x] ; z in [-1, 65) handled per-core
    # build per-core slabs directly: core k conv1-out slices global [8k-1, 8k+9)
    x1cols = []
    for k in range(NC):
        xc = np.zeros((10, 81, GRID * GRID), np.float32)
        for sl in range(10):
            zg = 8 * k - 1 + sl
            if zg < 0 or zg >= GRID:
                continue
            cols = np.zeros((81, GRID, GRID), np.float32)
            for o, (dz, dy, dx) in enumerate(OFFSETS):
                # padded coords: (zg+dz+1, y+dy+1, x+dx+1) over y,x in [0,64)
                cols[o * 3:(o + 1) * 3] = dpad[:, zg + dz + 1,
                                               1 + dy:GRID + 1 + dy,
                                               1 + dx:GRID + 1 + dx]
            cols *= mask0[zg][None, :, :]
            xc[sl] = cols.reshape(81, -1)
        x1cols.append(xc)

    # ---- weight packs ----
    # W0 for im2col conv1: [81, 128] (co=64 duplicated for col-pairing)
    W0 = Ws[0]  # [3,3,3,3,64]
    w1col = np.zeros((81, 128), np.float32)
    for o, (dz, dy, dx) in enumerate(OFFSETS):
        w1col[o * 3:(o + 1) * 3, 0:64] = W0[dz + 1, dy + 1, dx + 1]
        w1col[o * 3:(o + 1) * 3, 64:128] = W0[dz + 1, dy + 1, dx + 1]

    def pack_pair(W):  # [3,3,3,cin,co] -> pair [2*cin, 9, co] + left [cin, 9, co]
        cin, co = W.shape[3], W.shape[4]
        wp = np.zeros((2 * cin, 9, co), np.float32)
        wl = np.zeros((cin, 9, co), np.float32)
        for j, (dy, dx) in enumerate(DYDX):
            wp[0:cin, j] = W[0, dy + 1, dx + 1]      # dz=-1
            wp[cin:2 * cin, j] = W[1, dy + 1, dx + 1]  # dz=0
            wl[:, j] = W[2, dy + 1, dx + 1]          # dz=+1
        return wp, wl

    w0p, w0l = pack_pair(Ws[1])   # L0 conv2 64->64
    w1p, w1l = pack_pair(Ws[2])   # L1 conv1 64->96
    w0l = np.concatenate([w0l, w0l], axis=0)  # [128, 9, 64] both halves
    w1l = np.concatenate([w1l, w1l], axis=0)  # [128, 9, 96]

    def pack_generic(W):  # -> list of [kchunk, 27, co] arrays
        cin, co = W.shape[3], W.shape[4]
        wf = W.reshape(27, cin, co)
        out = []
        for k0 in range(0, cin, 128):
            kc = min(128, cin - k0)
            out.append(np.ascontiguousarray(
                wf[:, k0:k0 + kc, :].transpose(1, 0, 2)))  # [kc, 27, co]
        return out

    gen_w = {}
    for li, wi in [("w1c2", 3), ("w2c1", 4), ("w2c2", 5), ("w3c1", 6),
                   ("w3c2", 7), ("w4c1", 8), ("w4c2", 9), ("w5c1", 10),
                   ("w5c2", 11)]:
        gen_w[li] = pack_generic(Ws[wi])
    # L6: center tap only (1^3 grid)
    for li, wi in [("w6c1", 12), ("w6c2", 13)]:
        W = Ws[wi]
        cin, co = W.shape[3], W.shape[4]
        wc = W[1, 1, 1]  # [cin, co]
        gen_w[li] = [np.ascontiguousarray(wc[k0:k0 + min(128, cin - k0)][:, None, :])
                     for k0 in range(0, cin, 128)]

    # ---- per-core mask arrays ----
    # L0 maskneg for conv2-evict: [8, 4096]
    mn0 = [((masks[0][8 * k:8 * k + 8] - 1.0) * BIG).reshape(8, -1).astype(np.float32)
           for k in range(NC)]
    # L0 pool-out multiply: m1 on core's L1 slices [4, 1024]
    m1p = [masks[1][4 * k:4 * k + 4].reshape(4, -1).astype(np.float32)
           for k in range(NC)]

    def slab_mask(mask, z0, nsl):
        D2 = mask.shape[1] * mask.shape[2]
        out = np.zeros((nsl, D2), np.float32)
        for i in range(nsl):
            zg = z0 + i
            if 0 <= zg < mask.shape[0]:
                out[i] = mask[zg].reshape(-1)
        return out

    # L1 conv1-evict multiply mask (m1 x ingrid): slices [4k-1, 4k+5)
    m1mul = [slab_mask(masks[1], 4 * k - 1, 6) for k in range(NC)]
    # L1 conv2-evict maskneg: slices [4k, 4k+4)
    mn1 = [((slab_mask(masks[1], 4 * k, 4) - 1.0) * BIG).astype(np.float32)
           for k in range(NC)]
    # L1 pool-out multiply: m2 on core's L2 slices [2, 256]
    m2p = [slab_mask(masks[2], 2 * k, 2) for k in range(NC)]
    # L2 conv1-evict multiply (m2 x ingrid): slices [2k-1, 2k+3)
    m2mul = [slab_mask(masks[2], 2 * k - 1, 4) for k in range(NC)]
    # L2 conv2-evict maskneg: slices [2k, 2k+2)
    mn2 = [((slab_mask(masks[2], 2 * k, 2) - 1.0) * BIG).astype(np.float32)
           for k in range(NC)]
    # L2 pool-out multiply: m3 on core's L3 slice [1, 64]
    m3p = [slab_mask(masks[3], k, 1) for k in range(NC)]
    # L3 (replicated): conv1-evict mul (m3 x ingrid) slices [-1, 9)
    m3mul_r = slab_mask(masks[3], -1, 10)
    mn3_r = ((slab_mask(masks[3], 0, 8) - 1.0) * BIG).astype(np.float32)
    m4p_r = slab_mask(masks[4], 0, 4)       # [4, 16]
    m4mul_r = slab_mask(masks[4], 0, 4)     # L4 out all valid (full grid)
    mn4_r = ((slab_mask(masks[4], 0, 4) - 1.0) * BIG).astype(np.float32)
    m5p_r = slab_mask(masks[5], 0, 2)
    m5mul_r = slab_mask(masks[5], 0, 2)
    mn5_r = ((slab_mask(masks[5], 0, 2) - 1.0) * BIG).astype(np.float32)
    m6p_r = slab_mask(masks[6], 0, 1)

    meta = {
        "mask_flags": {
            # whether the real mask (not just ingrid) has zeros at each level
            1: not np.all(masks[1] == 1.0),
            2: not np.all(masks[2] == 1.0),
            3: not np.all(masks[3] == 1.0),
            4: not np.all(masks[4] == 1.0),
            5: not np.all(masks[5] == 1.0),
            6: not np.all(masks[6] == 1.0),
        },
    }

    in_maps = []
    for k in range(NC):
        im = {
            "x1col": x1cols[k],
            "w1col": w1col,
            "w0p": w0p, "w0l": w0l, "w1p": w1p, "w1l": w1l,
            "mn0": mn0[k], "m1p": m1p[k],
            "m1mul": m1mul[k], "mn1": mn1[k], "m2p": m2p[k],
            "m2mul": m2mul[k], "mn2": mn2[k], "m3p": m3p[k],
            "m3mul": m3mul_r, "mn3": mn3_r, "m4p": m4p_r,
            "m4mul": m4mul_r, "mn4": mn4_r, "m5p": m5p_r,
            "m5mul": m5mul_r, "mn5": mn5_r, "m6p": m6p_r,
        }
        for name, chunks in gen_w.items():
            for ci, arr in enumerate(chunks):
                im[f"{name}_{ci}"] = arr
        in_maps.append(im)
    return in_maps, meta


def build_kernel(meta):
    import contextlib
    nc = bacc.Bacc("TRN2", target_bir_lowering=False, debug=False, num_devices=NC)
    mf = meta["mask_flags"]

    # ---------- DRAM I/O declarations ----------
    def din(name, shape):
        return nc.dram_tensor(name, list(shape), F32, kind="ExternalInput")

    x1col = din("x1col", (10, 81, 4096))
    w1col_d = din("w1col", (81, 128))
    w0p_d = din("w0p", (128, 9, 64)); w0l_d = din("w0l", (128, 9, 64))
    w1p_d = din("w1p", (128, 9, 96)); w1l_d = din("w1l", (128, 9, 96))
    mn0_d = din("mn0", (8, 4096)); m1p_d = din("m1p", (4, 1024))
    m1mul_d = din("m1mul", (6, 1024)); mn1_d = din("mn1", (4, 1024))
    m2p_d = din("m2p", (2, 256))
    m2mul_d = din("m2mul", (4, 256)); mn2_d = din("mn2", (2, 256))
    m3p_d = din("m3p", (1, 64))
    m3mul_d = din("m3mul", (10, 64)); mn3_d = din("mn3", (8, 64))
    m4p_d = din("m4p", (4, 16)); m4mul_d = din("m4mul", (4, 16))
    mn4_d = din("mn4", (4, 16))
    m5p_d = din("m5p", (2, 4)); m5mul_d = din("m5mul", (2, 4))
    mn5_d = din("mn5", (2, 4)); m6p_d = din("m6p", (1, 1))

    genw_d = {}
    genw_shapes = {
        "w1c2": [(96, 27, 96)], "w2c1": [(96, 27, 128)], "w2c2": [(128, 27, 128)],
        "w3c1": [(128, 27, 160)], "w3c2": [(128, 27, 160), (32, 27, 160)],
        "w4c1": [(128, 27, 192), (32, 27, 192)],
        "w4c2": [(128, 27, 192), (64, 27, 192)],
        "w5c1": [(128, 27, 224), (64, 27, 224)],
        "w5c2": [(128, 27, 224), (96, 27, 224)],
        "w6c1": [(128, 1, 256), (96, 1, 256)],
        "w6c2": [(128, 1, 256), (128, 1, 256)],
    }
    for name, shl in genw_shapes.items():
        genw_d[name] = [din(f"{name}_{i}", s) for i, s in enumerate(shl)]

    out_d = nc.dram_tensor("out", [1, 256], F32, kind="ExternalOutput")
    import os as _os
    DBG = bool(_os.environ.get("K_DEBUG"))
    dbg_d = {}
    if DBG:
        for nm, sh in [("dP0", (64, 4, 1156)), ("dA1", (128, 8, 1156)),
                       ("dB1", (96, 6, 1156)), ("dC1", (96, 4, 1024)),
                       ("dP1", (96, 2, 324)), ("dA2", (96, 6, 324)),
                       ("dB2", (128, 4, 324)), ("dC2", (128, 2, 256)),
                       ("dP2", (128, 1, 100)), ("dA3", (128, 12, 100)),
                       ("dB3a", (128, 10, 100)), ("dC3a", (128, 512)),
                       ("dP4a", (128, 216)), ("dB4a", (128, 216)),
                       ("dC4a", (128, 64)), ("dP5a", (128, 64)),
                       ("dB5a", (128, 64)), ("dP6a", (128, 27)),
                       ("dX6a", (128, 1)), ("dC0", (64, 2, 4096))]:
            dbg_d[nm] = nc.dram_tensor(nm, list(sh), F32, kind="ExternalOutput")

    with tile.TileContext(nc) as tc:
        ctx = contextlib.ExitStack()
        with ctx:
            pst = ctx.enter_context(tc.tile_pool(name="ps", bufs=4, space="PSUM"))
            drm = ctx.enter_context(tc.tile_pool(name="dram", bufs=1, space="DRAM"))
            glob = ctx.enter_context(tc.tile_pool(name="glob", bufs=1))

            pid = nc.sync.partition_id()

            def wload(pool, d, shape=None, name=None, dt=F32R):
                sh = shape or d.shape
                t = pool.tile(list(sh), dt, name=name or f"sb_{d.name}")
                nc.sync.dma_start(t[:], d[:].bitcast(dt) if dt is F32R else d[:])
                return t

            # zero tile for G-pad zeroing
            zt = glob.tile([128, 1156], F32)
            nc.vector.memset(zt[:], 0.0)

            # DRAM gather buffers
            c1_d = drm.tile([4, 64, 1156], F32)
            G1 = drm.tile([36, 64, 1156], F32)
            c2_d = drm.tile([2, 96, 324], F32)
            G2 = drm.tile([20, 96, 324], F32)
            c3_d = drm.tile([1, 128, 100], F32)
            G3 = drm.tile([12, 128, 100], F32)
            gpad_insts = []
            for G, csz, npad in ((G1, (64, 1156), 2), (G2, (96, 324), 2),
                                 (G3, (128, 100), 2)):
                n = G.shape[0]
                for s in list(range(npad)) + list(range(n - npad, n)):
                    gpad_insts.append(
                        nc.sync.dma_start(G[s], zt[0:csz[0], 0:csz[1]]))

            # persistent tail tensors (small; cross level boundaries)
            P4a = glob.tile([128, 216], F32); P4b = glob.tile([32, 216], F32)
            P5a = glob.tile([128, 64], F32); P5b = glob.tile([64, 64], F32)
            P6a = glob.tile([128, 27], F32); P6b = glob.tile([96, 27], F32)
            X6a = glob.tile([128, 1], F32); X6b = glob.tile([128, 1], F32)
            outt = glob.tile([128, 2], F32)
            for t in (P4a, P4b, P5a, P5b, P6a, P6b):
                nc.vector.memset(t[:].bitcast(F32), 0.0)

            # ================ LEVEL 0 ================
            with tc.tile_pool(name="l0w", bufs=1) as wp, \
                 tc.tile_pool(name="l0p", bufs=1) as pp, \
                 tc.tile_pool(name="l0s", bufs=2) as ss, \
                 tc.tile_pool(name="l0m", bufs=4) as sm:
                w1col_t = wload(wp, w1col_d)
                w0p_t = wload(wp, w0p_d)
                w0l_t = wload(wp, w0l_d)

                A0 = pp.tile([128, 4, 4356], F32R)
                C0 = pp.tile([64, 2, 4096], F32R)
                P0 = pp.tile([64, 4, 1156], F32R)
                for _s in range(4):
                    nc.vector.memset(A0[:, _s, :].bitcast(F32), 0.0)
                nc.vector.memset(P0[:].bitcast(F32), 0.0)

                def l0_conv1(sl):
                    xs = ss.tile([81, 4096], F32R, tag="x1s")
                    nc.sync.dma_start(xs[:], x1col[sl].bitcast(F32R))
                    for chunk in range(8):
                        ps = pst.tile([64, 512], F32, tag="ps")
                        nc.tensor.matmul(ps[:], w1col_t[:, 0:64],
                                         xs[:, chunk * 512:chunk * 512 + 512],
                                         start=True, stop=True)
                        r0, r1 = sl % 4, (sl - 1) % 4
                        yb = chunk * 8
                        src = ps[:].rearrange("p (a b) -> p a b", b=64)
                        d0 = A0[0:64, r0, :].rearrange("p (a b) -> p a b", b=66)
                        d1 = A0[64:128, r1, :].rearrange("p (a b) -> p a b", b=66)
                        nc.scalar.copy(d0[:, yb + 1:yb + 9, 1:65], src)
                        nc.gpsimd.tensor_copy(d1[:, yb + 1:yb + 9, 1:65],
                                              d0[:, yb + 1:yb + 9, 1:65])

                def l0_conv2(z):
                    # ring r: rows0 = h1[local r mod 4 writer], i.e.
                    # conv1(sl) wrote rows0@sl%4 and rows64@(sl-1)%4.
                    # out z needs h1 locals (z, z+1, z+2); out z+1 one more.
                    rA = z % 4         # rows0=h1[z], rows64=h1[z+1]
                    rB = (z + 1) % 4   # rows0=h1[z+1], rows64=h1[z+2]
                    rD = (z + 3) % 4   # rows0=h1[z+3]
                    for chunk in range(8):
                        yb = chunk * 8
                        psA = pst.tile([64, 512], F32, tag="ps")
                        psB = pst.tile([64, 512], F32, tag="ps")
                        wA = A0[:, rA, :].rearrange("p (a b) -> p a b", b=66)
                        wB = A0[:, rB, :].rearrange("p (a b) -> p a b", b=66)
                        wD = A0[:, rD, :].rearrange("p (a b) -> p a b", b=66)
                        for j, (dy, dx) in enumerate(DYDX):
                            first, last = (j == 0), (j == 8)
                            ys = slice(yb + 1 + dy, yb + 9 + dy)
                            xsl = slice(1 + dx, 65 + dx)
                            vA = psA[:].rearrange("p (a b) -> p a b", b=64)
                            vB = psB[:].rearrange("p (a b) -> p a b", b=64)
                            # K=128 z-pair mms (full rows)
                            nc.tensor.matmul(vA, w0p_t[:, j, :],
                                             wA[:, ys, xsl],
                                             start=first, stop=False)
                            nc.tensor.matmul(vB, w0p_t[:, j, :],
                                             wB[:, ys, xsl],
                                             start=first, stop=False)
                            # K=64 leftovers, row-group paired:
                            # out z reads h1[z+1] at rows64 of rB;
                            # out z+1 reads h1[z+2] at rows0 of rD.
                            nc.tensor.matmul(vA, w0l_t[64:128, j, :],
                                             wB[64:128, ys, xsl],
                                             start=False, stop=last)
                            nc.tensor.matmul(vB, w0l_t[0:64, j, :],
                                             wD[0:64, ys, xsl],
                                             start=False, stop=last)
                        for ps_, zz, h in ((psA, z, 0), (psB, z + 1, 1)):
                            mt = sm.tile([64, 512], F32, tag="mn0")
                            nc.sync.dma_start(
                                mt[:], mn0_d[zz, yb * 64:yb * 64 + 512]
                                .unsqueeze(0).to_broadcast((64, 512)))
                            nc.vector.tensor_add(
                                C0[:, h, yb * 64:yb * 64 + 512], ps_[:], mt[:])

                def l0_pool(z):
                    zp = z // 2
                    nc.vector.tensor_max(C0[:, 0, :], C0[:, 0, :], C0[:, 1, :])
                    v = C0[:, 0, :].rearrange("p (a b) -> p a b", b=64)
                    t2 = ss.tile([64, 32, 64], F32R, tag="pool0b", bufs=1)
                    nc.vector.tensor_max(t2[:], v[:, 0::2, :], v[:, 1::2, :])
                    t3 = ss.tile([64, 32, 32], F32R, tag="pool0c", bufs=1)
                    nc.vector.tensor_max(t3[:], t2[:, :, 0::2], t2[:, :, 1::2])
                    mt = sm.tile([64, 1024], F32, tag="m1p", bufs=2)
                    nc.sync.dma_start(mt[:], m1p_d[zp].unsqueeze(0)
                                      .to_broadcast((64, 1024)))
                    dst = P0[:, zp, :].rearrange("p (a b) -> p a b", b=34)
                    nc.vector.tensor_mul(
                        dst[:, 1:33, 1:33], t3[:],
                        mt[:].rearrange("p (a b) -> p a b", b=32))

                for sl in range(10):
                    l0_conv1(sl)
                    if sl >= 3 and (sl - 3) % 2 == 0:
                        zz = sl - 3
                        l0_conv2(zz)
                        l0_pool(zz)

                if DBG:
                    nc.sync.dma_start(dbg_d["dP0"][:], P0[:].bitcast(F32))
                    nc.sync.dma_start(dbg_d["dC0"][:], C0[:].bitcast(F32))
                nc.sync.dma_start(c1_d[:].rearrange("z c v -> c z v"),
                                  P0[:].bitcast(F32))

            # ---- AllGather L0 -> L1 ----
            ag1 = nc.gpsimd.collective_compute(
                "AllGather", mybir.AluOpType.bypass,
                replica_groups=[list(range(NC))],
                ins=[c1_d[:].opt()], outs=[G1[2:34].opt()])
            for gi in gpad_insts:
                add_dep_helper(ag1.ins, gi.ins, reason="G pads zeroed before gathers")

            # ================ LEVEL 1 ================
            with tc.tile_pool(name="l1w", bufs=1) as wp, \
                 tc.tile_pool(name="l1p", bufs=1) as pp, \
                 tc.tile_pool(name="l1s", bufs=2) as ss, \
                 tc.tile_pool(name="l1m", bufs=4) as sm:
                w1p_t = wload(wp, w1p_d)
                w1l_t = wload(wp, w1l_d)
                w1c2_t = wload(wp, genw_d["w1c2"][0])

                A1 = pp.tile([128, 8, 1156], F32R)
                B1 = pp.tile([96, 6, 1156], F32R)
                C1 = pp.tile([96, 4, 1024], F32R)
                P1 = pp.tile([96, 2, 324], F32R)
                nc.vector.memset(B1[:].bitcast(F32), 0.0)
                nc.vector.memset(P1[:].bitcast(F32), 0.0)
                _r1 = nc.sync.dma_start(A1[0:64, :, :],
                                  G1[bass.ds(pid * 4, 8)].rearrange("z c v -> c z v").bitcast(F32R))
                _r2 = nc.sync.dma_start(A1[64:128, 0:7, :],
                                  G1[bass.ds(pid * 4 + 1, 7)].rearrange("z c v -> c z v").bitcast(F32R))
                add_dep_helper(_r1.ins, ag1.ins, reason="gather before dynamic read")
                add_dep_helper(_r2.ins, ag1.ins, reason="gather before dynamic read")

                def l1_conv1(sl):
                    # A1 rows0 idx i = x1[4k-2+i]; rows64 idx i = x1[4k-1+i].
                    # out sl (global 4k-1+sl): pair = A1[:, sl] (dz=-1,0);
                    # leftover dz=+1 = rows64 idx sl+1 == rows0 idx sl+2.
                    mt = sm.tile([96, 1024], F32, tag="m1mul")
                    nc.sync.dma_start(mt[:], m1mul_d[sl].unsqueeze(0)
                                      .to_broadcast((96, 1024)))
                    pss = [pst.tile([96, 512], F32, tag="ps", name=f"ps_l1_{sl}_{_c}") for _c in range(2)]
                    wA = A1[:, sl, :].rearrange("p (a b) -> p a b", b=34)
                    wB = A1[64:128, sl + 1, :].rearrange("p (a b) -> p a b", b=34)
                    wC = A1[0:64, sl + 2, :].rearrange("p (a b) -> p a b", b=34)
                    for j, (dy, dx) in enumerate(DYDX):
                        xsl = slice(1 + dx, 33 + dx)
                        for chunk in range(2):
                            yb = chunk * 16
                            ys = slice(yb + 1 + dy, yb + 17 + dy)
                            nc.tensor.matmul(
                                pss[chunk][:].rearrange("p (a b) -> p a b", b=32),
                                w1p_t[:, j, :], wA[:, ys, xsl],
                                start=(j == 0), stop=False)
                        # row-paired leftovers: chunk0 on rows 64:128,
                        # chunk1 on rows 0:64 (concurrent row groups)
                        ys0 = slice(1 + dy, 17 + dy)
                        ys1 = slice(17 + dy, 33 + dy)
                        nc.tensor.matmul(
                            pss[0][:].rearrange("p (a b) -> p a b", b=32),
                            w1l_t[64:128, j, :], wB[:, ys0, xsl],
                            start=False, stop=(j == 8))
                        nc.tensor.matmul(
                            pss[1][:].rearrange("p (a b) -> p a b", b=32),
                            w1l_t[0:64, j, :], wC[:, ys1, xsl],
                            start=False, stop=(j == 8))
                    for chunk in range(2):
                        yb = chunk * 16
                        dst = B1[:, sl, :].rearrange("p (a b) -> p a b", b=34)
                        nc.vector.tensor_mul(
                            dst[:, yb + 1:yb + 17, 1:33],
                            pss[chunk][:].rearrange("p (a b) -> p a b", b=32),
                            mt[:, yb * 32:yb * 32 + 512].rearrange(
                                "p (a b) -> p a b", b=32))

                def l1_conv2(sl):
                    mt = sm.tile([96, 1024], F32, tag="mn1")
                    nc.sync.dma_start(mt[:], mn1_d[sl].unsqueeze(0)
                                      .to_broadcast((96, 1024)))
                    for chunk in range(2):
                        yb = chunk * 16
                        ps = pst.tile([96, 512], F32, tag="ps")
                        for o, (dz, dy, dx) in enumerate(OFFSETS):
                            w = B1[:, sl + 1 + dz, :].rearrange(
                                "p (a b) -> p a b", b=34)
                            nc.tensor.matmul(
                                ps[:].rearrange("p (a b) -> p a b", b=32),
                                w1c2_t[:, o, :],
                                w[:, yb + 1 + dy:yb + 17 + dy, 1 + dx:33 + dx],
                                start=(o == 0), stop=(o == 26))
                        nc.vector.tensor_add(C1[:, sl, yb * 32:yb * 32 + 512],
                                             ps[:],
                                             mt[:, yb * 32:yb * 32 + 512])

                def l1_pool(zz):
                    zp = zz // 2
                    nc.vector.tensor_max(C1[:, zz, :], C1[:, zz, :], C1[:, zz + 1, :])
                    v = C1[:, zz, :].rearrange("p (a b) -> p a b", b=32)
                    t2 = ss.tile([96, 16, 32], F32R, tag="pool1b")
                    nc.vector.tensor_max(t2[:], v[:, 0::2, :], v[:, 1::2, :])
                    t3 = ss.tile([96, 16, 16], F32R, tag="pool1c")
                    nc.vector.tensor_max(t3[:], t2[:, :, 0::2], t2[:, :, 1::2])
                    mt = sm.tile([96, 256], F32, tag="m2p")
                    nc.sync.dma_start(mt[:], m2p_d[zp].unsqueeze(0)
                                      .to_broadcast((96, 256)))
                    dst = P1[:, zp, :].rearrange("p (a b) -> p a b", b=18)
                    nc.vector.tensor_mul(
                        dst[:, 1:17, 1:17], t3[:],
                        mt[:].rearrange("p (a b) -> p a b", b=16))

                for sl in range(6):
                    l1_conv1(sl)
                    if sl >= 2:
                        l1_conv2(sl - 2)
                        if sl >= 3 and (sl - 3) % 2 == 0:
                            l1_pool(sl - 3)

                if DBG:
                    nc.sync.dma_start(dbg_d["dA1"][:], A1[:].bitcast(F32))
                    nc.sync.dma_start(dbg_d["dB1"][:], B1[:].bitcast(F32))
                    nc.sync.dma_start(dbg_d["dC1"][:], C1[:].bitcast(F32))
                    nc.sync.dma_start(dbg_d["dP1"][:], P1[:].bitcast(F32))
                nc.sync.dma_start(c2_d[:].rearrange("z c v -> c z v"),
                                  P1[:].bitcast(F32))

            # ---- AllGather L1 -> L2 ----
            ag2 = nc.gpsimd.collective_compute(
                "AllGather", mybir.AluOpType.bypass,
                replica_groups=[list(range(NC))],
                ins=[c2_d[:].opt()], outs=[G2[2:18].opt()])
            for gi in gpad_insts:
                add_dep_helper(ag2.ins, gi.ins, reason="G pads zeroed before gathers")

            # ================ LEVEL 2 ================
            with tc.tile_pool(name="l2w", bufs=1) as wp, \
                 tc.tile_pool(name="l2p", bufs=1) as pp, \
                 tc.tile_pool(name="l2s", bufs=2) as ss, \
                 tc.tile_pool(name="l2m", bufs=4) as sm:
                w2c1_t = wload(wp, genw_d["w2c1"][0])
                w2c2_t = wload(wp, genw_d["w2c2"][0])
                A2 = pp.tile([96, 6, 324], F32R)
                B2 = pp.tile([128, 4, 324], F32R)
                C2 = pp.tile([128, 2, 256], F32R)
                P2 = pp.tile([128, 1, 100], F32R)
                nc.vector.memset(B2[:].bitcast(F32), 0.0)
                nc.vector.memset(P2[:].bitcast(F32), 0.0)
                _r3 = nc.sync.dma_start(A2[:], G2[bass.ds(pid * 2, 6)].rearrange("z c v -> c z v").bitcast(F32R))
                add_dep_helper(_r3.ins, ag2.ins, reason="gather before dynamic read")

                for s0 in (0, 2):
                    ps = pst.tile([128, 512], F32, tag="ps")
                    for o, (dz, dy, dx) in enumerate(OFFSETS):
                        w = A2[:].rearrange("p z (a b) -> p z a b", b=18)
                        nc.tensor.matmul(
                            ps[:].rearrange("p (z a b) -> p z a b", z=2, a=16),
                            w2c1_t[:, o, :],
                            w[:, s0 + dz + 1:s0 + dz + 3,
                              1 + dy:17 + dy, 1 + dx:17 + dx],
                            start=(o == 0), stop=(o == 26))
                    mt = sm.tile([128, 512], F32, tag="m2mul")
                    nc.sync.dma_start(
                        mt[:], m2mul_d[s0:s0 + 2].flatten().unsqueeze(0)
                        .to_broadcast((128, 512)))
                    dst = B2[:].rearrange("p z (a b) -> p z a b", b=18)
                    nc.vector.tensor_mul(
                        dst[:, s0:s0 + 2, 1:17, 1:17],
                        ps[:].rearrange("p (z a b) -> p z a b", z=2, a=16),
                        mt[:].rearrange("p (z a b) -> p z a b", z=2, a=16))

                ps = pst.tile([128, 512], F32, tag="ps")
                for o, (dz, dy, dx) in enumerate(OFFSETS):
                    w = B2[:].rearrange("p z (a b) -> p z a b", b=18)
                    nc.tensor.matmul(
                        ps[:].rearrange("p (z a b) -> p z a b", z=2, a=16),
                        w2c2_t[:, o, :],
                        w[:, dz + 1:dz + 3, 1 + dy:17 + dy, 1 + dx:17 + dx],
                        start=(o == 0), stop=(o == 26))
                if mf[2]:
                    mt = sm.tile([128, 512], F32, tag="mn2")
                    nc.sync.dma_start(mt[:], mn2_d[:].flatten().unsqueeze(0)
                                      .to_broadcast((128, 512)))
                    nc.vector.tensor_add(C2[:].rearrange("p a b -> p (a b)"),
                                         ps[:], mt[:])
                else:
                    nc.scalar.copy(C2[:].rearrange("p a b -> p (a b)"), ps[:])

                # L2 pool
                nc.vector.tensor_max(C2[:, 0, :], C2[:, 0, :], C2[:, 1, :])
                v = C2[:, 0, :].rearrange("p (a b) -> p a b", b=16)
                t2 = ss.tile([128, 8, 16], F32R, tag="pool2b")
                nc.vector.tensor_max(t2[:], v[:, 0::2, :], v[:, 1::2, :])
                dst = P2[:, 0, :].rearrange("p (a b) -> p a b", b=10)
                if mf[3]:
                    t3 = ss.tile([128, 8, 8], F32R, tag="pool2c")
                    nc.vector.tensor_max(t3[:], t2[:, :, 0::2], t2[:, :, 1::2])
                    mt = sm.tile([128, 64], F32, tag="m3p")
                    nc.sync.dma_start(mt[:], m3p_d[0].unsqueeze(0)
                                      .to_broadcast((128, 64)))
                    nc.vector.tensor_mul(
                        dst[:, 1:9, 1:9], t3[:],
                        mt[:].rearrange("p (a b) -> p a b", b=8))
                else:
                    nc.vector.tensor_max(dst[:, 1:9, 1:9],
                                         t2[:, :, 0::2], t2[:, :, 1::2])

                if DBG:
                    nc.sync.dma_start(dbg_d["dA2"][:], A2[:].bitcast(F32))
                    nc.sync.dma_start(dbg_d["dB2"][:], B2[:].bitcast(F32))
                    nc.sync.dma_start(dbg_d["dC2"][:], C2[:].bitcast(F32))
                    nc.sync.dma_start(dbg_d["dP2"][:], P2[:].bitcast(F32))
                nc.sync.dma_start(c3_d[:].rearrange("z c v -> c z v"),
                                  P2[:].bitcast(F32))

            # ---- AllGather L2 -> L3 ----
            ag3 = nc.gpsimd.collective_compute(
                "AllGather", mybir.AluOpType.bypass,
                replica_groups=[list(range(NC))],
                ins=[c3_d[:].opt()], outs=[G3[2:10].opt()])
            for gi in gpad_insts:
                add_dep_helper(ag3.ins, gi.ins, reason="G pads zeroed before gathers")

            # ================ LEVEL 3 (replicated) ================
            with tc.tile_pool(name="l3w", bufs=1) as wp, \
                 tc.tile_pool(name="l3p", bufs=1) as pp, \
                 tc.tile_pool(name="l3s", bufs=2) as ss, \
                 tc.tile_pool(name="l3m", bufs=4) as sm:
                w3c1_t = wload(wp, genw_d["w3c1"][0])
                w3c2_t = [wload(wp, d) for d in genw_d["w3c2"]]
                A3 = pp.tile([128, 12, 100], F32R)
                B3a = pp.tile([128, 10, 100], F32R)
                B3b = pp.tile([32, 10, 100], F32R)
                C3a = pp.tile([128, 512], F32R)
                C3b = pp.tile([32, 512], F32R)
                nc.vector.memset(B3a[:].bitcast(F32), 0.0)
                nc.vector.memset(B3b[:].bitcast(F32), 0.0)
                _r4 = nc.sync.dma_start(A3[:], G3[:].rearrange("z c v -> c z v").bitcast(F32R))
                add_dep_helper(_r4.ins, ag3.ins, reason="gather before read")

                # conv1
                for (z0, nz) in ((0, 8), (2, 8)):
                    N = nz * 64
                    for (c0, co_n) in ((0, 128), (128, 32)):
                        ps = pst.tile([co_n, 512], F32, tag="ps")
                        for o, (dz, dy, dx) in enumerate(OFFSETS):
                            w = A3[:].rearrange("p z (a b) -> p z a b", b=10)
                            nc.tensor.matmul(
                                ps[:, 0:N].rearrange(
                                    "p (z a b) -> p z a b", z=nz, a=8),
                                w3c1_t[:, o, c0:c0 + co_n],
                                w[:, z0 + dz + 1:z0 + dz + 1 + nz,
                                  1 + dy:9 + dy, 1 + dx:9 + dx],
                                start=(o == 0), stop=(o == 26))
                        mt = sm.tile([co_n, 512], F32, tag="m3mul")
                        nc.sync.dma_start(
                            mt[:, 0:N],
                            m3mul_d[z0:z0 + nz].flatten().unsqueeze(0)
                            .to_broadcast((co_n, N)))
                        B3 = B3a if c0 == 0 else B3b
                        dst = B3[:].rearrange("p z (a b) -> p z a b", b=10)
                        nc.vector.tensor_mul(
                            dst[:, z0:z0 + nz, 1:9, 1:9],
                            ps[:, 0:N].rearrange(
                                "p (z a b) -> p z a b", z=nz, a=8),
                            mt[:, 0:N].rearrange(
                                "p (z a b) -> p z a b", z=nz, a=8))

                # conv2
                for (c0, co_n) in ((0, 128), (128, 32)):
                    ps = pst.tile([co_n, 512], F32, tag="ps")
                    for o, (dz, dy, dx) in enumerate(OFFSETS):
                        for ki, B3 in enumerate((B3a, B3b)):
                            w = B3[:].rearrange("p z (a b) -> p z a b", b=10)
                            nc.tensor.matmul(
                                ps[:].rearrange("p (z a b) -> p z a b",
                                                z=8, a=8),
                                w3c2_t[ki][:, o, c0:c0 + co_n],
                                w[:, dz + 1:dz + 9, 1 + dy:9 + dy,
                                  1 + dx:9 + dx],
                                start=(o == 0 and ki == 0),
                                stop=(o == 26 and ki == 1))
                    C3 = C3a if c0 == 0 else C3b
                    if mf[3]:
                        mt = sm.tile([co_n, 512], F32, tag="mn3")
                        nc.sync.dma_start(mt[:], mn3_d[:].flatten().unsqueeze(0)
                                          .to_broadcast((co_n, 512)))
                        nc.vector.tensor_add(C3[:], ps[:], mt[:])
                    else:
                        nc.scalar.copy(C3[:], ps[:])

                # pool -> P4
                for C3, P4, cn in ((C3a, P4a, 128), (C3b, P4b, 32)):
                    v = C3[:].rearrange("p (z v) -> p z v", v=64)
                    t1 = ss.tile([cn, 4, 64], F32R, tag="pool3a")
                    nc.vector.tensor_max(t1[:], v[:, 0::2, :], v[:, 1::2, :])
                    u = t1[:].rearrange("p z (a b) -> p z a b", b=8)
                    t2 = ss.tile([cn, 4, 4, 8], F32R, tag="pool3b")
                    nc.vector.tensor_max(t2[:], u[:, :, 0::2, :],
                                         u[:, :, 1::2, :])
                    dst = P4[:].rearrange("p (z a b) -> p z a b", z=6, a=6)
                    if mf[4]:
                        t3 = ss.tile([cn, 4, 4, 4], F32R, tag="pool3c")
                        nc.vector.tensor_max(t3[:], t2[:, :, :, 0::2],
                                             t2[:, :, :, 1::2])
                        mt = sm.tile([cn, 64], F32, tag="m4p")
                        nc.sync.dma_start(mt[:], m4p_d[:].flatten().unsqueeze(0)
                                          .to_broadcast((cn, 64)))
                        nc.vector.tensor_mul(
                            dst[:, 1:5, 1:5, 1:5], t3[:],
                            mt[:].rearrange("p (z a b) -> p z a b", z=4, a=4))
                    else:
                        nc.vector.tensor_max(dst[:, 1:5, 1:5, 1:5],
                                             t2[:, :, :, 0::2],
                                             t2[:, :, :, 1::2])

                if DBG:
                    nc.sync.dma_start(dbg_d["dA3"][:], A3[:].bitcast(F32))
                    nc.sync.dma_start(dbg_d["dB3a"][:], B3a[:].bitcast(F32))
                    nc.sync.dma_start(dbg_d["dC3a"][:], C3a[:].bitcast(F32))

            # ================ TAIL (levels 4-6, replicated) ================
            def tail_conv(sm, wts, ins, outs, pg, og, mode, mdram, mname):
                N = og * og * og
                noff = wts[0].shape[1]
                offs = OFFSETS if noff == 27 else [(0, 0, 0)]
                for (ot, c0, co_n, padded) in outs:
                    ps = pst.tile([co_n, max(N, 8)], F32, tag="ps")
                    nmm = len(offs) * len(ins)
                    i = 0
                    for o, (dz, dy, dx) in enumerate(offs):
                        for ki, it in enumerate(ins):
                            w = it[:].rearrange("p (z a b) -> p z a b",
                                                z=pg, a=pg)
                            nc.tensor.matmul(
                                ps[:, 0:N].rearrange(
                                    "p (z a b) -> p z a b", z=og, a=og),
                                wts[ki][:, o, c0:c0 + co_n],
                                w[:, 1 + dz:1 + dz + og, 1 + dy:1 + dy + og,
                                  1 + dx:1 + dx + og],
                                start=(i == 0), stop=(i == nmm - 1))
                            i += 1
                    if padded:
                        opg = og + 2
                        dst = ot[:].rearrange("p (z a b) -> p z a b",
                                              z=opg, a=opg)[:, 1:1 + og,
                                                            1:1 + og, 1:1 + og]
                    else:
                        dst = ot[:, 0:N].rearrange("p (z a b) -> p z a b",
                                                   z=og, a=og)
                    src = ps[:, 0:N].rearrange("p (z a b) -> p z a b",
                                               z=og, a=og)
                    if mode == "copy":
                        nc.scalar.copy(dst, src)
                    else:
                        mt = sm.tile([co_n, N], F32, tag=mname)
                        nc.sync.dma_start(
                            mt[:], mdram[:].flatten().unsqueeze(0)
                            .to_broadcast((co_n, N)))
                        mm = mt[:].rearrange("p (z a b) -> p z a b", z=og, a=og)
                        if mode == "mul":
                            nc.vector.tensor_mul(dst, src, mm)
                        else:
                            nc.vector.tensor_add(dst, src, mm)

            def tail_pool(sm, ss, cs, ps_out, g, has_mask, mdram):
                go = g // 2
                for (ct, cn), (pt, _) in zip(cs, ps_out):
                    v = ct[:, 0:g * g * g].rearrange("p (z v) -> p z v",
                                                     v=g * g)
                    t1 = ss.tile([cn, go, g * g], F32, tag=f"tp{g}a")
                    nc.vector.tensor_max(t1[:], v[:, 0::2, :], v[:, 1::2, :])
                    u = t1[:].rearrange("p z (a b) -> p z a b", b=g)
                    t2 = ss.tile([cn, go, go, g], F32, tag=f"tp{g}b")
                    nc.vector.tensor_max(t2[:], u[:, :, 0::2, :],
                                         u[:, :, 1::2, :])
                    gp = go + 2
                    dst = pt[:].rearrange("p (z a b) -> p z a b", z=gp, a=gp)
                    if has_mask:
                        t3 = ss.tile([cn, go, go, go], F32, tag=f"tp{g}c")
                        nc.vector.tensor_max(t3[:], t2[:, :, :, 0::2],
                                             t2[:, :, :, 1::2])
                        mt = sm.tile([cn, go * go * go], F32, tag=f"tp{g}m")
                        nc.sync.dma_start(
                            mt[:], mdram[:].flatten().unsqueeze(0)
                            .to_broadcast((cn, go * go * go)))
                        nc.vector.tensor_mul(
                            dst[:, 1:1 + go, 1:1 + go, 1:1 + go], t3[:],
                            mt[:].rearrange("p (z a b) -> p z a b",
                                            z=go, a=go))
                    else:
                        nc.vector.tensor_max(
                            dst[:, 1:1 + go, 1:1 + go, 1:1 + go],
                            t2[:, :, :, 0::2], t2[:, :, :, 1::2])

            # ---- L4 ----
            with tc.tile_pool(name="l4w", bufs=1) as wp, \
                 tc.tile_pool(name="l4p", bufs=1) as pp, \
                 tc.tile_pool(name="l4s", bufs=2) as ss, \
                 tc.tile_pool(name="l4m", bufs=2) as sm:
                w4c1_t = [wload(wp, d, dt=F32) for d in genw_d["w4c1"]]
                w4c2_t = [wload(wp, d, dt=F32) for d in genw_d["w4c2"]]
                B4a = pp.tile([128, 216], F32); B4b = pp.tile([64, 216], F32)
                C4a = pp.tile([128, 64], F32); C4b = pp.tile([64, 64], F32)
                nc.vector.memset(B4a[:].bitcast(F32), 0.0)
                nc.vector.memset(B4b[:].bitcast(F32), 0.0)
                tail_conv(sm, w4c1_t, [P4a, P4b],
                          [(B4a, 0, 128, True), (B4b, 128, 64, True)], 6, 4,
                          "mul" if mf[4] else "copy", m4mul_d, "m4mul")
                tail_conv(sm, w4c2_t, [B4a, B4b],
                          [(C4a, 0, 128, False), (C4b, 128, 64, False)], 6, 4,
                          "add" if mf[4] else "copy", mn4_d, "mn4")
                tail_pool(sm, ss, [(C4a, 128), (C4b, 64)],
                          [(P5a, 128), (P5b, 64)], 4, mf[5], m5p_d)

                if DBG:
                    nc.sync.dma_start(dbg_d["dP4a"][:], P4a[:])
                    nc.sync.dma_start(dbg_d["dB4a"][:], B4a[:])
                    nc.sync.dma_start(dbg_d["dC4a"][:], C4a[:])

            # ---- L5 ----
            with tc.tile_pool(name="l5w", bufs=1) as wp, \
                 tc.tile_pool(name="l5p", bufs=1) as pp, \
                 tc.tile_pool(name="l5s", bufs=2) as ss, \
                 tc.tile_pool(name="l5m", bufs=2) as sm:
                w5c1_t = [wload(wp, d, dt=F32) for d in genw_d["w5c1"]]
                w5c2_t = [wload(wp, d, dt=F32) for d in genw_d["w5c2"]]
                B5a = pp.tile([128, 64], F32); B5b = pp.tile([96, 64], F32)
                C5a = pp.tile([128, 8], F32); C5b = pp.tile([96, 8], F32)
                nc.vector.memset(B5a[:].bitcast(F32), 0.0)
                nc.vector.memset(B5b[:].bitcast(F32), 0.0)
                tail_conv(sm, w5c1_t, [P5a, P5b],
                          [(B5a, 0, 128, True), (B5b, 128, 96, True)], 4, 2,
                          "mul" if mf[5] else "copy", m5mul_d, "m5mul")
                tail_conv(sm, w5c2_t, [B5a, B5b],
                          [(C5a, 0, 128, False), (C5b, 128, 96, False)], 4, 2,
                          "add" if mf[5] else "copy", mn5_d, "mn5")
                tail_pool(sm, ss, [(C5a, 128), (C5b, 96)],
                          [(P6a, 128), (P6b, 96)], 2, mf[6], m6p_d)

                if DBG:
                    nc.sync.dma_start(dbg_d["dP5a"][:], P5a[:])
                    nc.sync.dma_start(dbg_d["dB5a"][:], B5a[:])
                    nc.sync.dma_start(dbg_d["dP6a"][:], P6a[:])

            # ---- L6 (1^3, center tap only) ----
            with tc.tile_pool(name="l6w", bufs=1) as wp:
                w6c1_t = [wload(wp, d, dt=F32) for d in genw_d["w6c1"]]
                w6c2_t = [wload(wp, d, dt=F32) for d in genw_d["w6c2"]]
                for (ot, c0) in ((X6a, 0), (X6b, 128)):
                    ps = pst.tile([128, 8], F32, tag="ps")
                    nc.tensor.matmul(ps[:, 0:1], w6c1_t[0][:, 0, c0:c0 + 128],
                                     P6a[:, 13:14], start=True, stop=False)
                    nc.tensor.matmul(ps[:, 0:1], w6c1_t[1][:, 0, c0:c0 + 128],
                                     P6b[:, 13:14], start=False, stop=True)
                    nc.vector.tensor_copy(ot[:], ps[:, 0:1])
                for i, c0 in enumerate((0, 128)):
                    ps = pst.tile([128, 8], F32, tag="ps")
                    nc.tensor.matmul(ps[:, 0:1], w6c2_t[0][:, 0, c0:c0 + 128],
                                     X6a[:], start=True, stop=False)
                    nc.tensor.matmul(ps[:, 0:1], w6c2_t[1][:, 0, c0:c0 + 128],
                                     X6b[:], start=False, stop=True)
                    nc.scalar.copy(outt[:, i:i + 1], ps[:, 0:1])
            if DBG:
                nc.sync.dma_start(dbg_d["dX6a"][:], X6a[:])
            nc.sync.dma_start(out_d[0, 0:128], outt[:, 0])
            nc.sync.dma_start(out_d[0, 128:256], outt[:, 1])

    nc.compile()
    return nc



_CACHE = {}
_RUNNERS = {}


def kernel(features, coors, W0, W1, W2, W3, W4, W5, W6, W7, W8, W9, W10, W11,
           W12, W13):
    features = np.asarray(features, np.float32)
    coors = np.asarray(coors, np.int32)
    Ws = [np.asarray(w, np.float32) for w in
          (W0, W1, W2, W3, W4, W5, W6, W7, W8, W9, W10, W11, W12, W13)]
    in_maps, meta = build_host_inputs(features, coors, Ws)
    key = tuple(sorted(meta["mask_flags"].items()))
    if key not in _CACHE:
        _CACHE[key] = build_kernel(meta)
    nc = _CACHE[key]
    try:
        if key not in _RUNNERS:
            _RUNNERS[key] = _Runner(nc, NC)
        r = _RUNNERS[key]
        r.place(in_maps)
        outs = r.run()
        out = r.result(outs, "out").reshape(256)
    except Exception:
        res = run_bass_kernel_spmd(nc, in_maps, core_ids=list(range(NC)))
        out = res.results[0]["out"].reshape(256)
    return out.reshape(1, 1, 1, 1, 256).astype(np.float32)


if __name__ == "__main__":
    pass



# revision 9
# speedup vs baseline: 5043.2831x; 57.4805x over previous
"""Trainium2 Bass kernel for the sparse submanifold 3D CNN (nn_Net_38963943309313).

Network: 7 blocks of 2 submanifold 3x3x3 convs on a 64^3 grid, 2x2x2 sparse
max-pools between blocks, channels 3->64->...->256, output [1,1,1,1,256].

Strategy (8 NeuronCores):
 - Shard z-slabs across cores for levels 0-2 (grids 64/32/16), AllGather the
   pooled activations between levels (z-padded gather buffers so per-core
   reads are a single dynamic-offset DMA). Levels 3-6 (grids 8/4/2/1) are
   replicated on every core.
 - Convs are fp32r matmuls: activations channel-major [C, z, y, x] in SBUF
   (y/x zero-padded), 27 shifted-window matmuls accumulated in PSUM.
 - conv1 of block 0 uses a host-side im2col (81 contract rows, masked
   columns so the submanifold mask is free).
 - 64-channel contractions (L0 conv2, L1 conv1) pack z-pairs into K=128 via
   duplicated storage; L0 conv2 additionally pairs two output slices into
   the two 64-column halves of the PE array.
 - Submanifold masking: conv1 evictions multiply by a broadcast mask (also
   zeroes the out-of-grid halo slices); conv2 evictions add (mask-1)*BIG so
   the following max-pool ignores inactive voxels; pool result is multiplied
   by the pooled mask.
"""

import sys

sys.path.insert(0, "/opt/trn_rl_repo")

import numpy as np
import concourse.bass as bass
import concourse.tile as tile
from concourse.tile import add_dep_helper
from concourse import bacc, mybir
from concourse.bass_utils import run_bass_kernel_spmd


class _Runner:
    """Compile-once, inputs-resident executor.

    Replicates concourse.bass2jax.run_bass_via_pjrt's lowering, but keeps the
    jitted shard_map executable and the device-placed input buffers alive
    across calls, so repeat executions cost only the PJRT dispatch + the NEFF
    execution itself (run_bass_kernel_spmd rebuilds the jit closure and
    re-uploads every input on each call).
    """

    def __init__(self, nc, n_cores, fully_written_outputs=True):
        import jax
        from jax.experimental.shard_map import shard_map
        from jax.sharding import Mesh, PartitionSpec
        from concourse import bass2jax

        bass2jax.install_neuronx_cc_hook()
        self.jax = jax
        self.bass2jax = bass2jax
        self.nc, self.n = nc, n_cores
        partition_name = (nc.partition_id_tensor.name
                          if nc.partition_id_tensor else None)
        in_names, out_names, out_avals = [], [], []
        for alloc in nc.m.functions[0].allocations:
            if not isinstance(alloc, mybir.MemoryLocationSet):
                continue
            name = alloc.memorylocations[0].name
            if alloc.kind == "ExternalInput":
                if name != partition_name:
                    in_names.append(name)
            elif alloc.kind == "ExternalOutput":
                out_names.append(name)
                out_avals.append(jax.core.ShapedArray(
                    tuple(alloc.tensor_shape), mybir.dt.np(alloc.dtype)))
        self.param_names = list(in_names)
        self.out_names, self.out_avals = out_names, out_avals
        self.dbg_name = nc.dbg_addr.name if nc.dbg_addr is not None else None
        n_params, n_outs = len(in_names), len(out_names)
        # When every output tensor is fully written by the kernel, skip the
        # donated zero-initialized output buffers run_bass_via_pjrt uses —
        # they cost a host->device transfer per call.
        self.pass_out_bufs = not fully_written_outputs
        if self.pass_out_bufs:
            full_names = (in_names + out_names
                          + ([partition_name] if partition_name else []))
        else:
            full_names = in_names + ([partition_name] if partition_name else [])

        def _body(*args):
            operands = list(args)
            if partition_name is not None:
                operands.append(bass2jax.partition_id_tensor())
            outs = bass2jax._bass_exec_p.bind(
                *operands,
                out_avals=tuple(out_avals),
                in_names=tuple(full_names),
                out_names=tuple(out_names),
                lowering_input_output_aliases=(),
                sim_require_finite=True,
                sim_require_nnan=True,
                nc=nc,
            )
            return tuple(outs)

        devices = jax.devices()[:n_cores]
        self.mesh = Mesh(np.asarray(devices), ("core",))
        n_args = n_params + (n_outs if self.pass_out_bufs else 0)
        self._shmapped = shard_map(
            _body, mesh=self.mesh,
            in_specs=(PartitionSpec("core"),) * n_args,
            out_specs=(PartitionSpec("core"),) * n_outs,
            check_rep=False)
        self._donate = (tuple(range(n_params, n_params + n_outs))
                        if self.pass_out_bufs else ())
        self._dev_in = None
        self._compiled = None

    def place(self, in_maps):
        from jax.sharding import NamedSharding, PartitionSpec
        if self.dbg_name is not None:
            in_maps = [{**m, self.dbg_name: np.zeros((1, 2), np.uint32)}
                       for m in in_maps]
        sh = NamedSharding(self.mesh, PartitionSpec("core"))
        concat = [np.concatenate([np.asarray(m[name]) for m in in_maps], 0)
                  for name in self.param_names]
        self._dev_in = [self.jax.device_put(a, sh) for a in concat]
        self.jax.block_until_ready(self._dev_in)
        if self._compiled is None:
            jax = self.jax
            example = list(self._dev_in) + self._fresh_out_bufs()
            try:
                self._compiled = self.bass2jax.fast_dispatch_compile(
                    lambda: jax.jit(self._shmapped, donate_argnums=self._donate,
                                    keep_unused=True).lower(*example).compile())
            except Exception:
                self._compiled = jax.jit(
                    self._shmapped, donate_argnums=self._donate,
                    keep_unused=True)

    def _fresh_out_bufs(self):
        if not self.pass_out_bufs:
            return []
        return [np.zeros((self.n * av.shape[0], *av.shape[1:]), av.dtype)
                for av in self.out_avals]

    def run(self):
        outs = self._compiled(*self._dev_in, *self._fresh_out_bufs())
        return self.jax.block_until_ready(outs)

    def run_async(self):
        return self._compiled(*self._dev_in, *self._fresh_out_bufs())

    def result(self, outs, name, core=0):
        i = self.out_names.index(name)
        av = self.out_avals[i]
        return np.asarray(outs[i]).reshape(self.n, *av.shape)[core]

NC = 8
GRID = 64
BIG = 1.0e30
CHANNELS = [(3, 64), (64, 64), (64, 96), (96, 96), (96, 128), (128, 128),
            (128, 160), (160, 160), (160, 192), (192, 192), (192, 224),
            (224, 224), (224, 256), (256, 256)]
F32 = mybir.dt.float32
F32R = mybir.dt.float32r

OFFSETS = [(dz, dy, dx) for dz in (-1, 0, 1) for dy in (-1, 0, 1) for dx in (-1, 0, 1)]
# 9 (dy,dx) pairs for z-pair-packed layers
DYDX = [(dy, dx) for dy in (-1, 0, 1) for dx in (-1, 0, 1)]

GENW_SHAPES = {
    "w1c2": [(96, 27, 96)], "w2c1": [(96, 27, 128)], "w2c2": [(128, 27, 128)],
    "w3c1": [(128, 27, 160)], "w3c2": [(128, 27, 160), (32, 27, 160)],
    "w4c1": [(128, 27, 192), (32, 27, 192)],
    "w4c2": [(128, 27, 192), (64, 27, 192)],
    "w5c1": [(128, 27, 224), (64, 27, 224)],
    "w5c2": [(128, 27, 224), (96, 27, 224)],
    "w6c1": [(128, 1, 256), (96, 1, 256)],
    "w6c2": [(128, 1, 256), (128, 1, 256)],
}

# Every constant input is packed into one flat f32 blob per core (single
# NEFF input tensor -> minimal per-dispatch arg overhead).
MANIFEST = [
    ("x1col", (10, 81, 4096)),
    ("w1col", (81, 128)),
    ("w0p", (128, 9, 64)), ("w0l", (128, 9, 64)),
    ("w1p", (128, 9, 96)), ("w1l", (128, 9, 96)),
    ("mn0", (8, 4096)), ("m1p", (4, 1024)),
    ("m1mul", (6, 1024)), ("mn1", (4, 1024)), ("m2p", (2, 256)),
    ("m2mul", (4, 256)), ("mn2", (2, 256)), ("m3p", (1, 64)),
    ("m3mul", (10, 64)), ("mn3", (8, 64)), ("m4p", (4, 16)),
    ("m4mul", (4, 16)), ("mn4", (4, 16)), ("m5p", (2, 4)),
    ("m5mul", (2, 4)), ("mn5", (2, 4)), ("m6p", (1, 1)),
] + [(f"{n}_{i}", s) for n, shl in GENW_SHAPES.items() for i, s in enumerate(shl)]
BLOB_SIZE = sum(int(np.prod(s)) for _, s in MANIFEST)


def _pool_np(x, m):
    # x: [D,D,D,C] or [D,D,D]; max over active voxels of 2x2x2 windows
    D = x.shape[0]
    if x.ndim == 3:
        xr = x.reshape(D // 2, 2, D // 2, 2, D // 2, 2)
        return xr.max(axis=(1, 3, 5))
    neg = np.where(m[..., None] > 0, x, -np.inf)
    xr = neg.reshape(D // 2, 2, D // 2, 2, D // 2, 2, -1)
    p = xr.max(axis=(1, 3, 5))
    mp = m.reshape(D // 2, 2, D // 2, 2, D // 2, 2).max(axis=(1, 3, 5))
    return np.where(mp[..., None] > 0, p, 0.0), mp


def _ceil_div(a, b):
    return (a + b - 1) // b


def build_host_inputs(features, coors, Ws):
    """All host-side data marshalling. Returns (in_maps, meta)."""
    z, y, x = coors[:, 0], coors[:, 1], coors[:, 2]
    dense = np.zeros((GRID, GRID, GRID, 3), np.float32)
    mask0 = np.zeros((GRID, GRID, GRID), np.float32)
    dense[z, y, x] = features  # last write wins (matches XLA CPU scatter)
    mask0[z, y, x] = 1.0

    # mask pyramid
    masks = [mask0]
    m = mask0
    for _ in range(6):
        mr = m.reshape(m.shape[0] // 2, 2, m.shape[1] // 2, 2, m.shape[2] // 2, 2)
        m = mr.max(axis=(1, 3, 5))
        masks.append(m)

    # ---- X1col: host im2col for conv1 of block 0, column-masked ----
    # padded dense [3, 66, 66, 66]
    dpad = np.zeros((3, GRID + 2, GRID + 2, GRID + 2), np.float32)
    dpad[:, 1:-1, 1:-1, 1:-1] = dense.transpose(3, 0, 1, 2)
    # X1col_full[z, (dz,dy,dx,ci), y*64+x] = dpad[ci, z+dz+1, y+dy+1, x+dx+1]
    sw = np.lib.stride_tricks.sliding_window_view(dpad, (3, 3, 3),
                                                  axis=(1, 2, 3))
    # sw[ci, z, y, x, a, b, c] = dpad[ci, z+a, y+b, x+c]
    xfull = np.ascontiguousarray(sw.transpose(1, 4, 5, 6, 0, 2, 3)).reshape(
        GRID, 81, GRID * GRID)
    xfull *= mask0.reshape(GRID, 1, GRID * GRID)
    xpad = np.zeros((GRID + 10, 81, GRID * GRID), np.float32)
    xpad[1:GRID + 1] = xfull
    # core k conv1-out slices global [8k-1, 8k+9)
    x1cols = [xpad[8 * k:8 * k + 10] for k in range(NC)]

    # ---- weight packs ----
    # W0 for im2col conv1: [81, 128] (co=64 duplicated for col-pairing)
    W0 = Ws[0]  # [3,3,3,3,64]
    w1col = np.zeros((81, 128), np.float32)
    for o, (dz, dy, dx) in enumerate(OFFSETS):
        w1col[o * 3:(o + 1) * 3, 0:64] = W0[dz + 1, dy + 1, dx + 1]
        w1col[o * 3:(o + 1) * 3, 64:128] = W0[dz + 1, dy + 1, dx + 1]

    def pack_pair(W):  # [3,3,3,cin,co] -> pair [2*cin, 9, co] + left [cin, 9, co]
        cin, co = W.shape[3], W.shape[4]
        wp = np.zeros((2 * cin, 9, co), np.float32)
        wl = np.zeros((cin, 9, co), np.float32)
        for j, (dy, dx) in enumerate(DYDX):
            wp[0:cin, j] = W[0, dy + 1, dx + 1]      # dz=-1
            wp[cin:2 * cin, j] = W[1, dy + 1, dx + 1]  # dz=0
            wl[:, j] = W[2, dy + 1, dx + 1]          # dz=+1
        return wp, wl

    w0p, w0l = pack_pair(Ws[1])   # L0 conv2 64->64
    w1p, w1l = pack_pair(Ws[2])   # L1 conv1 64->96
    w0l = np.concatenate([w0l, w0l], axis=0)  # [128, 9, 64] both halves
    w1l = np.concatenate([w1l, w1l], axis=0)  # [128, 9, 96]

    def pack_generic(W):  # -> list of [kchunk, 27, co] arrays
        cin, co = W.shape[3], W.shape[4]
        wf = W.reshape(27, cin, co)
        out = []
        for k0 in range(0, cin, 128):
            kc = min(128, cin - k0)
            out.append(np.ascontiguousarray(
                wf[:, k0:k0 + kc, :].transpose(1, 0, 2)))  # [kc, 27, co]
        return out

    gen_w = {}
    for li, wi in [("w1c2", 3), ("w2c1", 4), ("w2c2", 5), ("w3c1", 6),
                   ("w3c2", 7), ("w4c1", 8), ("w4c2", 9), ("w5c1", 10),
                   ("w5c2", 11)]:
        gen_w[li] = pack_generic(Ws[wi])
    # L6: center tap only (1^3 grid)
    for li, wi in [("w6c1", 12), ("w6c2", 13)]:
        W = Ws[wi]
        cin, co = W.shape[3], W.shape[4]
        wc = W[1, 1, 1]  # [cin, co]
        gen_w[li] = [np.ascontiguousarray(wc[k0:k0 + min(128, cin - k0)][:, None, :])
                     for k0 in range(0, cin, 128)]

    # ---- per-core mask arrays ----
    # L0 maskneg for conv2-evict: [8, 4096]
    mn0 = [((masks[0][8 * k:8 * k + 8] - 1.0) * BIG).reshape(8, -1).astype(np.float32)
           for k in range(NC)]
    # L0 pool-out multiply: m1 on core's L1 slices [4, 1024]
    m1p = [masks[1][4 * k:4 * k + 4].reshape(4, -1).astype(np.float32)
           for k in range(NC)]

    def slab_mask(mask, z0, nsl):
        D2 = mask.shape[1] * mask.shape[2]
        out = np.zeros((nsl, D2), np.float32)
        for i in range(nsl):
            zg = z0 + i
            if 0 <= zg < mask.shape[0]:
                out[i] = mask[zg].reshape(-1)
        return out

    # L1 conv1-evict multiply mask (m1 x ingrid): slices [4k-1, 4k+5)
    m1mul = [slab_mask(masks[1], 4 * k - 1, 6) for k in range(NC)]
    # L1 conv2-evict maskneg: slices [4k, 4k+4)
    mn1 = [((slab_mask(masks[1], 4 * k, 4) - 1.0) * BIG).astype(np.float32)
           for k in range(NC)]
    # L1 pool-out multiply: m2 on core's L2 slices [2, 256]
    m2p = [slab_mask(masks[2], 2 * k, 2) for k in range(NC)]
    # L2 conv1-evict multiply (m2 x ingrid): slices [2k-1, 2k+3)
    m2mul = [slab_mask(masks[2], 2 * k - 1, 4) for k in range(NC)]
    # L2 conv2-evict maskneg: slices [2k, 2k+2)
    mn2 = [((slab_mask(masks[2], 2 * k, 2) - 1.0) * BIG).astype(np.float32)
           for k in range(NC)]
    # L2 pool-out multiply: m3 on core's L3 slice [1, 64]
    m3p = [slab_mask(masks[3], k, 1) for k in range(NC)]
    # L3 (replicated): conv1-evict mul (m3 x ingrid) slices [-1, 9)
    m3mul_r = slab_mask(masks[3], -1, 10)
    mn3_r = ((slab_mask(masks[3], 0, 8) - 1.0) * BIG).astype(np.float32)
    m4p_r = slab_mask(masks[4], 0, 4)       # [4, 16]
    m4mul_r = slab_mask(masks[4], 0, 4)     # L4 out all valid (full grid)
    mn4_r = ((slab_mask(masks[4], 0, 4) - 1.0) * BIG).astype(np.float32)
    m5p_r = slab_mask(masks[5], 0, 2)
    m5mul_r = slab_mask(masks[5], 0, 2)
    mn5_r = ((slab_mask(masks[5], 0, 2) - 1.0) * BIG).astype(np.float32)
    m6p_r = slab_mask(masks[6], 0, 1)

    meta = {
        "mask_flags": {
            # whether the real mask (not just ingrid) has zeros at each level
            1: not np.all(masks[1] == 1.0),
            2: not np.all(masks[2] == 1.0),
            3: not np.all(masks[3] == 1.0),
            4: not np.all(masks[4] == 1.0),
            5: not np.all(masks[5] == 1.0),
            6: not np.all(masks[6] == 1.0),
        },
    }

    in_maps = []
    for k in range(NC):
        im = {
            "x1col": x1cols[k],
            "w1col": w1col,
            "w0p": w0p, "w0l": w0l, "w1p": w1p, "w1l": w1l,
            "mn0": mn0[k], "m1p": m1p[k],
            "m1mul": m1mul[k], "mn1": mn1[k], "m2p": m2p[k],
            "m2mul": m2mul[k], "mn2": mn2[k], "m3p": m3p[k],
            "m3mul": m3mul_r, "mn3": mn3_r, "m4p": m4p_r,
            "m4mul": m4mul_r, "mn4": mn4_r, "m5p": m5p_r,
            "m5mul": m5mul_r, "mn5": mn5_r, "m6p": m6p_r,
        }
        for name, chunks in gen_w.items():
            for ci, arr in enumerate(chunks):
                im[f"{name}_{ci}"] = arr
        parts = []
        for name, sh in MANIFEST:
            a = np.ascontiguousarray(im[name], np.float32)
            assert a.shape == sh, (name, a.shape, sh)
            parts.append(a.reshape(-1))
        in_maps.append({"blob": np.concatenate(parts)})
    return in_maps, meta


def build_kernel(meta):
    import contextlib
    nc = bacc.Bacc("TRN2", target_bir_lowering=False, debug=False, num_devices=NC)
    mf = meta["mask_flags"]

    # ---------- DRAM I/O declarations ----------
    # single flat input blob; every constant is an AP view into it
    blob_d = nc.dram_tensor("blob", [BLOB_SIZE], F32, kind="ExternalInput")
    views = {}
    off = 0
    for name, sh in MANIFEST:
        n = int(np.prod(sh))
        v = blob_d[off:off + n]
        if len(sh) == 2:
            v = v.rearrange("(a b) -> a b", a=sh[0], b=sh[1])
        elif len(sh) == 3:
            v = v.rearrange("(a b c) -> a b c", a=sh[0], b=sh[1], c=sh[2])
        views[name] = v
        off += n

    x1col = views["x1col"]
    w1col_d = views["w1col"]
    w0p_d = views["w0p"]; w0l_d = views["w0l"]
    w1p_d = views["w1p"]; w1l_d = views["w1l"]
    mn0_d = views["mn0"]; m1p_d = views["m1p"]
    m1mul_d = views["m1mul"]; mn1_d = views["mn1"]
    m2p_d = views["m2p"]
    m2mul_d = views["m2mul"]; mn2_d = views["mn2"]
    m3p_d = views["m3p"]
    m3mul_d = views["m3mul"]; mn3_d = views["mn3"]
    m4p_d = views["m4p"]; m4mul_d = views["m4mul"]
    mn4_d = views["mn4"]
    m5p_d = views["m5p"]; m5mul_d = views["m5mul"]
    mn5_d = views["mn5"]; m6p_d = views["m6p"]

    genw_d = {name: [views[f"{name}_{i}"] for i in range(len(shl))]
              for name, shl in GENW_SHAPES.items()}

    out_d = nc.dram_tensor("out", [1, 256], F32, kind="ExternalOutput")
    import os as _os
    DBG = bool(_os.environ.get("K_DEBUG"))
    dbg_d = {}
    if DBG:
        for nm, sh in [("dP0", (64, 4, 1156)), ("dA1", (128, 8, 1156)),
                       ("dB1", (96, 6, 1156)), ("dC1", (96, 4, 1024)),
                       ("dP1", (96, 2, 324)), ("dA2", (96, 6, 324)),
                       ("dB2", (128, 4, 324)), ("dC2", (128, 2, 256)),
                       ("dP2", (128, 1, 100)), ("dA3", (128, 12, 100)),
                       ("dB3a", (128, 10, 100)), ("dC3a", (128, 512)),
                       ("dP4a", (128, 216)), ("dB4a", (128, 216)),
                       ("dC4a", (128, 64)), ("dP5a", (128, 64)),
                       ("dB5a", (128, 64)), ("dP6a", (128, 27)),
                       ("dX6a", (128, 1)), ("dC0", (64, 2, 4096))]:
            dbg_d[nm] = nc.dram_tensor(nm, list(sh), F32, kind="ExternalOutput")

    with tile.TileContext(nc) as tc:
        ctx = contextlib.ExitStack()
        with ctx:
            pst = ctx.enter_context(tc.tile_pool(name="ps", bufs=4, space="PSUM"))
            drm = ctx.enter_context(tc.tile_pool(name="dram", bufs=1, space="DRAM"))
            glob = ctx.enter_context(tc.tile_pool(name="glob", bufs=1))

            pid = nc.sync.partition_id()

            _wl_ctr = [0]

            def wload(pool, d, shape=None, name=None, dt=F32R):
                sh = shape or d.shape
                _wl_ctr[0] += 1
                t = pool.tile(list(sh), dt,
                              name=name or f"sb_w{_wl_ctr[0]}")
                nc.sync.dma_start(t[:], d[:].bitcast(dt) if dt is F32R else d[:])
                return t

            # zero tile for G-pad zeroing
            zt = glob.tile([128, 1156], F32)
            nc.vector.memset(zt[:], 0.0)

            # DRAM gather buffers
            c1_d = drm.tile([4, 64, 1156], F32)
            G1 = drm.tile([36, 64, 1156], F32)
            c2_d = drm.tile([2, 96, 324], F32)
            G2 = drm.tile([20, 96, 324], F32)
            c3_d = drm.tile([1, 128, 100], F32)
            G3 = drm.tile([12, 128, 100], F32)
            gpad_insts = []
            for G, csz, npad in ((G1, (64, 1156), 2), (G2, (96, 324), 2),
                                 (G3, (128, 100), 2)):
                n = G.shape[0]
                for s in list(range(npad)) + list(range(n - npad, n)):
                    gpad_insts.append(
                        nc.sync.dma_start(G[s], zt[0:csz[0], 0:csz[1]]))

            # persistent tail tensors (small; cross level boundaries)
            P4a = glob.tile([128, 216], F32); P4b = glob.tile([32, 216], F32)
            P5a = glob.tile([128, 64], F32); P5b = glob.tile([64, 64], F32)
            P6a = glob.tile([128, 27], F32); P6b = glob.tile([96, 27], F32)
            X6a = glob.tile([128, 1], F32); X6b = glob.tile([128, 1], F32)
            outt = glob.tile([128, 2], F32)
            for t in (P4a, P4b, P5a, P5b, P6a, P6b):
                nc.vector.memset(t[:].bitcast(F32), 0.0)

            # ================ LEVEL 0 ================
            with tc.tile_pool(name="l0w", bufs=1) as wp, \
                 tc.tile_pool(name="l0p", bufs=1) as pp, \
                 tc.tile_pool(name="l0s", bufs=2) as ss, \
                 tc.tile_pool(name="l0m", bufs=4) as sm:
                w1col_t = wload(wp, w1col_d)
                w0p_t = wload(wp, w0p_d)
                w0l_t = wload(wp, w0l_d)

                A0 = pp.tile([128, 4, 4356], F32R)
                C0 = pp.tile([64, 2, 4096], F32R)
                P0 = pp.tile([64, 4, 1156], F32R)
                for _s in range(4):
                    nc.vector.memset(A0[:, _s, :].bitcast(F32), 0.0)
                nc.vector.memset(P0[:].bitcast(F32), 0.0)

                def l0_conv1(sl):
                    xs = ss.tile([81, 4096], F32R, tag="x1s")
                    nc.sync.dma_start(xs[:], x1col[sl].bitcast(F32R))
                    for chunk in range(8):
                        ps = pst.tile([64, 512], F32, tag="ps")
                        nc.tensor.matmul(ps[:], w1col_t[:, 0:64],
                                         xs[:, chunk * 512:chunk * 512 + 512],
                                         start=True, stop=True)
                        r0, r1 = sl % 4, (sl - 1) % 4
                        yb = chunk * 8
                        src = ps[:].rearrange("p (a b) -> p a b", b=64)
                        d0 = A0[0:64, r0, :].rearrange("p (a b) -> p a b", b=66)
                        d1 = A0[64:128, r1, :].rearrange("p (a b) -> p a b", b=66)
                        nc.scalar.copy(d0[:, yb + 1:yb + 9, 1:65], src)
                        nc.gpsimd.tensor_copy(d1[:, yb + 1:yb + 9, 1:65],
                                              d0[:, yb + 1:yb + 9, 1:65])

                def l0_conv2(z):
                    # ring r: rows0 = h1[local r mod 4 writer], i.e.
                    # conv1(sl) wrote rows0@sl%4 and rows64@(sl-1)%4.
                    # out z needs h1 locals (z, z+1, z+2); out z+1 one more.
                    rA = z % 4         # rows0=h1[z], rows64=h1[z+1]
                    rB = (z + 1) % 4   # rows0=h1[z+1], rows64=h1[z+2]
                    rD = (z + 3) % 4   # rows0=h1[z+3]
                    for chunk in range(8):
                        yb = chunk * 8
                        psA = pst.tile([64, 512], F32, tag="ps")
                        psB = pst.tile([64, 512], F32, tag="ps")
                        wA = A0[:, rA, :].rearrange("p (a b) -> p a b", b=66)
                        wB = A0[:, rB, :].rearrange("p (a b) -> p a b", b=66)
                        wD = A0[:, rD, :].rearrange("p (a b) -> p a b", b=66)
                        for j, (dy, dx) in enumerate(DYDX):
                            first, last = (j == 0), (j == 8)
                            ys = slice(yb + 1 + dy, yb + 9 + dy)
                            xsl = slice(1 + dx, 65 + dx)
                            vA = psA[:].rearrange("p (a b) -> p a b", b=64)
                            vB = psB[:].rearrange("p (a b) -> p a b", b=64)
                            # K=128 z-pair mms (full rows)
                            nc.tensor.matmul(vA, w0p_t[:, j, :],
                                             wA[:, ys, xsl],
                                             start=first, stop=False)
                            nc.tensor.matmul(vB, w0p_t[:, j, :],
                                             wB[:, ys, xsl],
                                             start=first, stop=False)
                            # K=64 leftovers, row-group paired:
                            # out z reads h1[z+1] at rows64 of rB;
                            # out z+1 reads h1[z+2] at rows0 of rD.
                            nc.tensor.matmul(vA, w0l_t[64:128, j, :],
                                             wB[64:128, ys, xsl],
                                             start=False, stop=last)
                            nc.tensor.matmul(vB, w0l_t[0:64, j, :],
                                             wD[0:64, ys, xsl],
                                             start=False, stop=last)
                        for ps_, zz, h in ((psA, z, 0), (psB, z + 1, 1)):
                            mt = sm.tile([64, 512], F32, tag="mn0")
                            nc.sync.dma_start(
                                mt[:], mn0_d[zz, yb * 64:yb * 64 + 512]
                                .unsqueeze(0).to_broadcast((64, 512)))
                            nc.vector.tensor_add(
                                C0[:, h, yb * 64:yb * 64 + 512], ps_[:], mt[:])

                def l0_pool(z):
                    zp = z // 2
                    nc.vector.tensor_max(C0[:, 0, :], C0[:, 0, :], C0[:, 1, :])
                    v = C0[:, 0, :].rearrange("p (a b) -> p a b", b=64)
                    t2 = ss.tile([64, 32, 64], F32R, tag="pool0b", bufs=1)
                    nc.vector.tensor_max(t2[:], v[:, 0::2, :], v[:, 1::2, :])
                    t3 = ss.tile([64, 32, 32], F32R, tag="pool0c", bufs=1)
                    nc.vector.tensor_max(t3[:], t2[:, :, 0::2], t2[:, :, 1::2])
                    mt = sm.tile([64, 1024], F32, tag="m1p", bufs=2)
                    nc.sync.dma_start(mt[:], m1p_d[zp].unsqueeze(0)
                                      .to_broadcast((64, 1024)))
                    dst = P0[:, zp, :].rearrange("p (a b) -> p a b", b=34)
                    nc.vector.tensor_mul(
                        dst[:, 1:33, 1:33], t3[:],
                        mt[:].rearrange("p (a b) -> p a b", b=32))

                for sl in range(10):
                    l0_conv1(sl)
                    if sl >= 3 and (sl - 3) % 2 == 0:
                        zz = sl - 3
                        l0_conv2(zz)
                        l0_pool(zz)

                if DBG:
                    nc.sync.dma_start(dbg_d["dP0"][:], P0[:].bitcast(F32))
                    nc.sync.dma_start(dbg_d["dC0"][:], C0[:].bitcast(F32))
                nc.sync.dma_start(c1_d[:].rearrange("z c v -> c z v"),
                                  P0[:].bitcast(F32))

            # ---- AllGather L0 -> L1 ----
            ag1 = nc.gpsimd.collective_compute(
                "AllGather", mybir.AluOpType.bypass,
                replica_groups=[list(range(NC))],
                ins=[c1_d[:].opt()], outs=[G1[2:34].opt()])
            for gi in gpad_insts:
                add_dep_helper(ag1.ins, gi.ins, reason="G pads zeroed before gathers")

            # ================ LEVEL 1 ================
            with tc.tile_pool(name="l1w", bufs=1) as wp, \
                 tc.tile_pool(name="l1p", bufs=1) as pp, \
                 tc.tile_pool(name="l1s", bufs=2) as ss, \
                 tc.tile_pool(name="l1m", bufs=4) as sm:
                w1p_t = wload(wp, w1p_d)
                w1l_t = wload(wp, w1l_d)
                w1c2_t = wload(wp, genw_d["w1c2"][0])

                A1 = pp.tile([128, 8, 1156], F32R)
                B1 = pp.tile([96, 6, 1156], F32R)
                C1 = pp.tile([96, 4, 1024], F32R)
                P1 = pp.tile([96, 2, 324], F32R)
                nc.vector.memset(B1[:].bitcast(F32), 0.0)
                nc.vector.memset(P1[:].bitcast(F32), 0.0)
                _r1 = nc.sync.dma_start(A1[0:64, :, :],
                                  G1[bass.ds(pid * 4, 8)].rearrange("z c v -> c z v").bitcast(F32R))
                _r2 = nc.sync.dma_start(A1[64:128, 0:7, :],
                                  G1[bass.ds(pid * 4 + 1, 7)].rearrange("z c v -> c z v").bitcast(F32R))
                add_dep_helper(_r1.ins, ag1.ins, reason="gather before dynamic read")
                add_dep_helper(_r2.ins, ag1.ins, reason="gather before dynamic read")

                def l1_conv1(sl):
                    # A1 rows0 idx i = x1[4k-2+i]; rows64 idx i = x1[4k-1+i].
                    # out sl (global 4k-1+sl): pair = A1[:, sl] (dz=-1,0);
                    # leftover dz=+1 = rows64 idx sl+1 == rows0 idx sl+2.
                    mt = sm.tile([96, 1024], F32, tag="m1mul")
                    nc.sync.dma_start(mt[:], m1mul_d[sl].unsqueeze(0)
                                      .to_broadcast((96, 1024)))
                    pss = [pst.tile([96, 512], F32, tag="ps", name=f"ps_l1_{sl}_{_c}") for _c in range(2)]
                    wA = A1[:, sl, :].rearrange("p (a b) -> p a b", b=34)
                    wB = A1[64:128, sl + 1, :].rearrange("p (a b) -> p a b", b=34)
                    wC = A1[0:64, sl + 2, :].rearrange("p (a b) -> p a b", b=34)
                    for j, (dy, dx) in enumerate(DYDX):
                        xsl = slice(1 + dx, 33 + dx)
                        for chunk in range(2):
                            yb = chunk * 16
                            ys = slice(yb + 1 + dy, yb + 17 + dy)
                            nc.tensor.matmul(
                                pss[chunk][:].rearrange("p (a b) -> p a b", b=32),
                                w1p_t[:, j, :], wA[:, ys, xsl],
                                start=(j == 0), stop=False)
                        # row-paired leftovers: chunk0 on rows 64:128,
                        # chunk1 on rows 0:64 (concurrent row groups)
                        ys0 = slice(1 + dy, 17 + dy)
                        ys1 = slice(17 + dy, 33 + dy)
                        nc.tensor.matmul(
                            pss[0][:].rearrange("p (a b) -> p a b", b=32),
                            w1l_t[64:128, j, :], wB[:, ys0, xsl],
                            start=False, stop=(j == 8))
                        nc.tensor.matmul(
                            pss[1][:].rearrange("p (a b) -> p a b", b=32),
                            w1l_t[0:64, j, :], wC[:, ys1, xsl],
                            start=False, stop=(j == 8))
                    for chunk in range(2):
                        yb = chunk * 16
                        dst = B1[:, sl, :].rearrange("p (a b) -> p a b", b=34)
                        nc.vector.tensor_mul(
                            dst[:, yb + 1:yb + 17, 1:33],
                            pss[chunk][:].rearrange("p (a b) -> p a b", b=32),
                            mt[:, yb * 32:yb * 32 + 512].rearrange(
                                "p (a b) -> p a b", b=32))

                def l1_conv2(sl):
                    mt = sm.tile([96, 1024], F32, tag="mn1")
                    nc.sync.dma_start(mt[:], mn1_d[sl].unsqueeze(0)
                                      .to_broadcast((96, 1024)))
                    for chunk in range(2):
                        yb = chunk * 16
                        ps = pst.tile([96, 512], F32, tag="ps")
                        for o, (dz, dy, dx) in enumerate(OFFSETS):
                            w = B1[:, sl + 1 + dz, :].rearrange(
                                "p (a b) -> p a b", b=34)
                            nc.tensor.matmul(
                                ps[:].rearrange("p (a b) -> p a b", b=32),
                                w1c2_t[:, o, :],
                                w[:, yb + 1 + dy:yb + 17 + dy, 1 + dx:33 + dx],
                                start=(o == 0), stop=(o == 26))
                        nc.vector.tensor_add(C1[:, sl, yb * 32:yb * 32 + 512],
                                             ps[:],
                                             mt[:, yb * 32:yb * 32 + 512])

                def l1_pool(zz):
                    zp = zz // 2
                    nc.vector.tensor_max(C1[:, zz, :], C1[:, zz, :], C1[:, zz + 1, :])
                    v = C1[:, zz, :].rearrange("p (a b) -> p a b", b=32)
                    t2 = ss.tile([96, 16, 32], F32R, tag="pool1b")
                    nc.vector.tensor_max(t2[:], v[:, 0::2, :], v[:, 1::2, :])
                    t3 = ss.tile([96, 16, 16], F32R, tag="pool1c")
                    nc.vector.tensor_max(t3[:], t2[:, :, 0::2], t2[:, :, 1::2])
                    mt = sm.tile([96, 256], F32, tag="m2p")
                    nc.sync.dma_start(mt[:], m2p_d[zp].unsqueeze(0)
                                      .to_broadcast((96, 256)))
                    dst = P1[:, zp, :].rearrange("p (a b) -> p a b", b=18)
                    nc.vector.tensor_mul(
                        dst[:, 1:17, 1:17], t3[:],
                        mt[:].rearrange("p (a b) -> p a b", b=16))

                for sl in range(6):
                    l1_conv1(sl)
                    if sl >= 2:
                        l1_conv2(sl - 2)
                        if sl >= 3 and (sl - 3) % 2 == 0:
                            l1_pool(sl - 3)

                if DBG:
                    nc.sync.dma_start(dbg_d["dA1"][:], A1[:].bitcast(F32))
                    nc.sync.dma_start(dbg_d["dB1"][:], B1[:].bitcast(F32))
                    nc.sync.dma_start(dbg_d["dC1"][:], C1[:].bitcast(F32))
                    nc.sync.dma_start(dbg_d["dP1"][:], P1[:].bitcast(F32))
                nc.sync.dma_start(c2_d[:].rearrange("z c v -> c z v"),
                                  P1[:].bitcast(F32))

            # ---- AllGather L1 -> L2 ----
            ag2 = nc.gpsimd.collective_compute(
                "AllGather", mybir.AluOpType.bypass,
                replica_groups=[list(range(NC))],
                ins=[c2_d[:].opt()], outs=[G2[2:18].opt()])
            for gi in gpad_insts:
                add_dep_helper(ag2.ins, gi.ins, reason="G pads zeroed before gathers")

            # ================ LEVEL 2 ================
            with tc.tile_pool(name="l2w", bufs=1) as wp, \
                 tc.tile_pool(name="l2p", bufs=1) as pp, \
                 tc.tile_pool(name="l2s", bufs=2) as ss, \
                 tc.tile_pool(name="l2m", bufs=4) as sm:
                w2c1_t = wload(wp, genw_d["w2c1"][0])
                w2c2_t = wload(wp, genw_d["w2c2"][0])
                A2 = pp.tile([96, 6, 324], F32R)
                B2 = pp.tile([128, 4, 324], F32R)
                C2 = pp.tile([128, 2, 256], F32R)
                P2 = pp.tile([128, 1, 100], F32R)
                nc.vector.memset(B2[:].bitcast(F32), 0.0)
                nc.vector.memset(P2[:].bitcast(F32), 0.0)
                _r3 = nc.sync.dma_start(A2[:], G2[bass.ds(pid * 2, 6)].rearrange("z c v -> c z v").bitcast(F32R))
                add_dep_helper(_r3.ins, ag2.ins, reason="gather before dynamic read")

                for s0 in (0, 2):
                    ps = pst.tile([128, 512], F32, tag="ps")
                    for o, (dz, dy, dx) in enumerate(OFFSETS):
                        w = A2[:].rearrange("p z (a b) -> p z a b", b=18)
                        nc.tensor.matmul(
                            ps[:].rearrange("p (z a b) -> p z a b", z=2, a=16),
                            w2c1_t[:, o, :],
                            w[:, s0 + dz + 1:s0 + dz + 3,
                              1 + dy:17 + dy, 1 + dx:17 + dx],
                            start=(o == 0), stop=(o == 26))
                    mt = sm.tile([128, 512], F32, tag="m2mul")
                    nc.sync.dma_start(
                        mt[:], m2mul_d[s0:s0 + 2].flatten().unsqueeze(0)
                        .to_broadcast((128, 512)))
                    dst = B2[:].rearrange("p z (a b) -> p z a b", b=18)
                    nc.vector.tensor_mul(
                        dst[:, s0:s0 + 2, 1:17, 1:17],
                        ps[:].rearrange("p (z a b) -> p z a b", z=2, a=16),
                        mt[:].rearrange("p (z a b) -> p z a b", z=2, a=16))

                ps = pst.tile([128, 512], F32, tag="ps")
                for o, (dz, dy, dx) in enumerate(OFFSETS):
                    w = B2[:].rearrange("p z (a b) -> p z a b", b=18)
                    nc.tensor.matmul(
                        ps[:].rearrange("p (z a b) -> p z a b", z=2, a=16),
                        w2c2_t[:, o, :],
                        w[:, dz + 1:dz + 3, 1 + dy:17 + dy, 1 + dx:17 + dx],
                        start=(o == 0), stop=(o == 26))
                if mf[2]:
                    mt = sm.tile([128, 512], F32, tag="mn2")
                    nc.sync.dma_start(mt[:], mn2_d[:].flatten().unsqueeze(0)
                                      .to_broadcast((128, 512)))
                    nc.vector.tensor_add(C2[:].rearrange("p a b -> p (a b)"),
                                         ps[:], mt[:])
                else:
                    nc.scalar.copy(C2[:].rearrange("p a b -> p (a b)"), ps[:])

                # L2 pool
                nc.vector.tensor_max(C2[:, 0, :], C2[:, 0, :], C2[:, 1, :])
                v = C2[:, 0, :].rearrange("p (a b) -> p a b", b=16)
                t2 = ss.tile([128, 8, 16], F32R, tag="pool2b")
                nc.vector.tensor_max(t2[:], v[:, 0::2, :], v[:, 1::2, :])
                dst = P2[:, 0, :].rearrange("p (a b) -> p a b", b=10)
                if mf[3]:
                    t3 = ss.tile([128, 8, 8], F32R, tag="pool2c")
                    nc.vector.tensor_max(t3[:], t2[:, :, 0::2], t2[:, :, 1::2])
                    mt = sm.tile([128, 64], F32, tag="m3p")
                    nc.sync.dma_start(mt[:], m3p_d[0].unsqueeze(0)
                                      .to_broadcast((128, 64)))
                    nc.vector.tensor_mul(
                        dst[:, 1:9, 1:9], t3[:],
                        mt[:].rearrange("p (a b) -> p a b", b=8))
                else:
                    nc.vector.tensor_max(dst[:, 1:9, 1:9],
                                         t2[:, :, 0::2], t2[:, :, 1::2])

                if DBG:
                    nc.sync.dma_start(dbg_d["dA2"][:], A2[:].bitcast(F32))
                    nc.sync.dma_start(dbg_d["dB2"][:], B2[:].bitcast(F32))
                    nc.sync.dma_start(dbg_d["dC2"][:], C2[:].bitcast(F32))
                    nc.sync.dma_start(dbg_d["dP2"][:], P2[:].bitcast(F32))
                nc.sync.dma_start(c3_d[:].rearrange("z c v -> c z v"),
                                  P2[:].bitcast(F32))

            # ---- AllGather L2 -> L3 ----
            ag3 = nc.gpsimd.collective_compute(
                "AllGather", mybir.AluOpType.bypass,
                replica_groups=[list(range(NC))],
                ins=[c3_d[:].opt()], outs=[G3[2:10].opt()])
            for gi in gpad_insts:
                add_dep_helper(ag3.ins, gi.ins, reason="G pads zeroed before gathers")

            # ================ LEVEL 3 (replicated) ================
            with tc.tile_pool(name="l3w", bufs=1) as wp, \
                 tc.tile_pool(name="l3p", bufs=1) as pp, \
                 tc.tile_pool(name="l3s", bufs=2) as ss, \
                 tc.tile_pool(name="l3m", bufs=4) as sm:
                w3c1_t = wload(wp, genw_d["w3c1"][0])
                w3c2_t = [wload(wp, d) for d in genw_d["w3c2"]]
                A3 = pp.tile([128, 12, 100], F32R)
                B3a = pp.tile([128, 10, 100], F32R)
                B3b = pp.tile([32, 10, 100], F32R)
                C3a = pp.tile([128, 512], F32R)
                C3b = pp.tile([32, 512], F32R)
                nc.vector.memset(B3a[:].bitcast(F32), 0.0)
                nc.vector.memset(B3b[:].bitcast(F32), 0.0)
                _r4 = nc.sync.dma_start(A3[:], G3[:].rearrange("z c v -> c z v").bitcast(F32R))
                add_dep_helper(_r4.ins, ag3.ins, reason="gather before read")

                # conv1
                for (z0, nz) in ((0, 8), (2, 8)):
                    N = nz * 64
                    for (c0, co_n) in ((0, 128), (128, 32)):
                        ps = pst.tile([co_n, 512], F32, tag="ps")
                        for o, (dz, dy, dx) in enumerate(OFFSETS):
                            w = A3[:].rearrange("p z (a b) -> p z a b", b=10)
                            nc.tensor.matmul(
                                ps[:, 0:N].rearrange(
                                    "p (z a b) -> p z a b", z=nz, a=8),
                                w3c1_t[:, o, c0:c0 + co_n],
                                w[:, z0 + dz + 1:z0 + dz + 1 + nz,
                                  1 + dy:9 + dy, 1 + dx:9 + dx],
                                start=(o == 0), stop=(o == 26))
                        mt = sm.tile([co_n, 512], F32, tag="m3mul")
                        nc.sync.dma_start(
                            mt[:, 0:N],
                            m3mul_d[z0:z0 + nz].flatten().unsqueeze(0)
                            .to_broadcast((co_n, N)))
                        B3 = B3a if c0 == 0 else B3b
                        dst = B3[:].rearrange("p z (a b) -> p z a b", b=10)
                        nc.vector.tensor_mul(
                            dst[:, z0:z0 + nz, 1:9, 1:9],
                            ps[:, 0:N].rearrange(
                                "p (z a b) -> p z a b", z=nz, a=8),
                            mt[:, 0:N].rearrange(
                                "p (z a b) -> p z a b", z=nz, a=8))

                # conv2
                for (c0, co_n) in ((0, 128), (128, 32)):
                    ps = pst.tile([co_n, 512], F32, tag="ps")
                    for o, (dz, dy, dx) in enumerate(OFFSETS):
                        for ki, B3 in enumerate((B3a, B3b)):
                            w = B3[:].rearrange("p z (a b) -> p z a b", b=10)
                            nc.tensor.matmul(
                                ps[:].rearrange("p (z a b) -> p z a b",
                                                z=8, a=8),
                                w3c2_t[ki][:, o, c0:c0 + co_n],
                                w[:, dz + 1:dz + 9, 1 + dy:9 + dy,
                                  1 + dx:9 + dx],
                                start=(o == 0 and ki == 0),
                                stop=(o == 26 and ki == 1))
                    C3 = C3a if c0 == 0 else C3b
                    if mf[3]:
                        mt = sm.tile([co_n, 512], F32, tag="mn3")
                        nc.sync.dma_start(mt[:], mn3_d[:].flatten().unsqueeze(0)
                                          .to_broadcast((co_n, 512)))
                        nc.vector.tensor_add(C3[:], ps[:], mt[:])
                    else:
                        nc.scalar.copy(C3[:], ps[:])

                # pool -> P4
                for C3, P4, cn in ((C3a, P4a, 128), (C3b, P4b, 32)):
                    v = C3[:].rearrange("p (z v) -> p z v", v=64)
                    t1 = ss.tile([cn, 4, 64], F32R, tag="pool3a")
                    nc.vector.tensor_max(t1[:], v[:, 0::2, :], v[:, 1::2, :])
                    u = t1[:].rearrange("p z (a b) -> p z a b", b=8)
                    t2 = ss.tile([cn, 4, 4, 8], F32R, tag="pool3b")
                    nc.vector.tensor_max(t2[:], u[:, :, 0::2, :],
                                         u[:, :, 1::2, :])
                    dst = P4[:].rearrange("p (z a b) -> p z a b", z=6, a=6)
                    if mf[4]:
                        t3 = ss.tile([cn, 4, 4, 4], F32R, tag="pool3c")
                        nc.vector.tensor_max(t3[:], t2[:, :, :, 0::2],
                                             t2[:, :, :, 1::2])
                        mt = sm.tile([cn, 64], F32, tag="m4p")
                        nc.sync.dma_start(mt[:], m4p_d[:].flatten().unsqueeze(0)
                                          .to_broadcast((cn, 64)))
                        nc.vector.tensor_mul(
                            dst[:, 1:5, 1:5, 1:5], t3[:],
                            mt[:].rearrange("p (z a b) -> p z a b", z=4, a=4))
                    else:
                        nc.vector.tensor_max(dst[:, 1:5, 1:5, 1:5],
                                             t2[:, :, :, 0::2],
                                             t2[:, :, :, 1::2])

                if DBG:
                    nc.sync.dma_start(dbg_d["dA3"][:], A3[:].bitcast(F32))
                    nc.sync.dma_start(dbg_d["dB3a"][:], B3a[:].bitcast(F32))
                    nc.sync.dma_start(dbg_d["dC3a"][:], C3a[:].bitcast(F32))

            # ================ TAIL (levels 4-6, replicated) ================
            def tail_conv(sm, wts, ins, outs, pg, og, mode, mdram, mname):
                N = og * og * og
                noff = wts[0].shape[1]
                offs = OFFSETS if noff == 27 else [(0, 0, 0)]
                for (ot, c0, co_n, padded) in outs:
                    ps = pst.tile([co_n, max(N, 8)], F32, tag="ps")
                    nmm = len(offs) * len(ins)
                    i = 0
                    for o, (dz, dy, dx) in enumerate(offs):
                        for ki, it in enumerate(ins):
                            w = it[:].rearrange("p (z a b) -> p z a b",
                                                z=pg, a=pg)
                            nc.tensor.matmul(
                                ps[:, 0:N].rearrange(
                                    "p (z a b) -> p z a b", z=og, a=og),
                                wts[ki][:, o, c0:c0 + co_n],
                                w[:, 1 + dz:1 + dz + og, 1 + dy:1 + dy + og,
                                  1 + dx:1 + dx + og],
                                start=(i == 0), stop=(i == nmm - 1))
                            i += 1
                    if padded:
                        opg = og + 2
                        dst = ot[:].rearrange("p (z a b) -> p z a b",
                                              z=opg, a=opg)[:, 1:1 + og,
                                                            1:1 + og, 1:1 + og]
                    else:
                        dst = ot[:, 0:N].rearrange("p (z a b) -> p z a b",
                                                   z=og, a=og)
                    src = ps[:, 0:N].rearrange("p (z a b) -> p z a b",
                                               z=og, a=og)
                    if mode == "copy":
                        nc.scalar.copy(dst, src)
                    else:
                        mt = sm.tile([co_n, N], F32, tag=mname)
                        nc.sync.dma_start(
                            mt[:], mdram[:].flatten().unsqueeze(0)
                            .to_broadcast((co_n, N)))
                        mm = mt[:].rearrange("p (z a b) -> p z a b", z=og, a=og)
                        if mode == "mul":
                            nc.vector.tensor_mul(dst, src, mm)
                        else:
                            nc.vector.tensor_add(dst, src, mm)

            def tail_pool(sm, ss, cs, ps_out, g, has_mask, mdram):
                go = g // 2
                for (ct, cn), (pt, _) in zip(cs, ps_out):
                    v = ct[:, 0:g * g * g].rearrange("p (z v) -> p z v",
                                                     v=g * g)
                    t1 = ss.tile([cn, go, g * g], F32, tag=f"tp{g}a")
                    nc.vector.tensor_max(t1[:], v[:, 0::2, :], v[:, 1::2, :])
                    u = t1[:].rearrange("p z (a b) -> p z a b", b=g)
                    t2 = ss.tile([cn, go, go, g], F32, tag=f"tp{g}b")
                    nc.vector.tensor_max(t2[:], u[:, :, 0::2, :],
                                         u[:, :, 1::2, :])
                    gp = go + 2
                    dst = pt[:].rearrange("p (z a b) -> p z a b", z=gp, a=gp)
                    if has_mask:
                        t3 = ss.tile([cn, go, go, go], F32, tag=f"tp{g}c")
                        nc.vector.tensor_max(t3[:], t2[:, :, :, 0::2],
                                             t2[:, :, :, 1::2])
                        mt = sm.tile([cn, go * go * go], F32, tag=f"tp{g}m")
                        nc.sync.dma_start(
                            mt[:], mdram[:].flatten().unsqueeze(0)
                            .to_broadcast((cn, go * go * go)))
                        nc.vector.tensor_mul(
                            dst[:, 1:1 + go, 1:1 + go, 1:1 + go], t3[:],
                            mt[:].rearrange("p (z a b) -> p z a b",
                                            z=go, a=go))
                    else:
                        nc.vector.tensor_max(
                            dst[:, 1:1 + go, 1:1 + go, 1:1 + go],
                            t2[:, :, :, 0::2], t2[:, :, :, 1::2])

            # ---- L4 ----
            with tc.tile_pool(name="l4w", bufs=1) as wp, \
                 tc.tile_pool(name="l4p", bufs=1) as pp, \
                 tc.tile_pool(name="l4s", bufs=2) as ss, \
                 tc.tile_pool(name="l4m", bufs=2) as sm:
                w4c1_t = [wload(wp, d, dt=F32) for d in genw_d["w4c1"]]
                w4c2_t = [wload(wp, d, dt=F32) for d in genw_d["w4c2"]]
                B4a = pp.tile([128, 216], F32); B4b = pp.tile([64, 216], F32)
                C4a = pp.tile([128, 64], F32); C4b = pp.tile([64, 64], F32)
                nc.vector.memset(B4a[:].bitcast(F32), 0.0)
                nc.vector.memset(B4b[:].bitcast(F32), 0.0)
                tail_conv(sm, w4c1_t, [P4a, P4b],
                          [(B4a, 0, 128, True), (B4b, 128, 64, True)], 6, 4,
                          "mul" if mf[4] else "copy", m4mul_d, "m4mul")
                tail_conv(sm, w4c2_t, [B4a, B4b],
                          [(C4a, 0, 128, False), (C4b, 128, 64, False)], 6, 4,
                          "add" if mf[4] else "copy", mn4_d, "mn4")
                tail_pool(sm, ss, [(C4a, 128), (C4b, 64)],
                          [(P5a, 128), (P5b, 64)], 4, mf[5], m5p_d)

                if DBG:
                    nc.sync.dma_start(dbg_d["dP4a"][:], P4a[:])
                    nc.sync.dma_start(dbg_d["dB4a"][:], B4a[:])
                    nc.sync.dma_start(dbg_d["dC4a"][:], C4a[:])

            # ---- L5 ----
            with tc.tile_pool(name="l5w", bufs=1) as wp, \
                 tc.tile_pool(name="l5p", bufs=1) as pp, \
                 tc.tile_pool(name="l5s", bufs=2) as ss, \
                 tc.tile_pool(name="l5m", bufs=2) as sm:
                w5c1_t = [wload(wp, d, dt=F32) for d in genw_d["w5c1"]]
                w5c2_t = [wload(wp, d, dt=F32) for d in genw_d["w5c2"]]
                B5a = pp.tile([128, 64], F32); B5b = pp.tile([96, 64], F32)
                C5a = pp.tile([128, 8], F32); C5b = pp.tile([96, 8], F32)
                nc.vector.memset(B5a[:].bitcast(F32), 0.0)
                nc.vector.memset(B5b[:].bitcast(F32), 0.0)
                tail_conv(sm, w5c1_t, [P5a, P5b],
                          [(B5a, 0, 128, True), (B5b, 128, 96, True)], 4, 2,
                          "mul" if mf[5] else "copy", m5mul_d, "m5mul")
                tail_conv(sm, w5c2_t, [B5a, B5b],
                          [(C5a, 0, 128, False), (C5b, 128, 96, False)], 4, 2,
                          "add" if mf[5] else "copy", mn5_d, "mn5")
                tail_pool(sm, ss, [(C5a, 128), (C5b, 96)],
                          [(P6a, 128), (P6b, 96)], 2, mf[6], m6p_d)

                if DBG:
                    nc.sync.dma_start(dbg_d["dP5a"][:], P5a[:])
                    nc.sync.dma_start(dbg_d["dB5a"][:], B5a[:])
                    nc.sync.dma_start(dbg_d["dP6a"][:], P6a[:])

            # ---- L6 (1^3, center tap only) ----
            with tc.tile_pool(name="l6w", bufs=1) as wp:
                w6c1_t = [wload(wp, d, dt=F32) for d in genw_d["w6c1"]]
                w6c2_t = [wload(wp, d, dt=F32) for d in genw_d["w6c2"]]
                for (ot, c0) in ((X6a, 0), (X6b, 128)):
                    ps = pst.tile([128, 8], F32, tag="ps")
                    nc.tensor.matmul(ps[:, 0:1], w6c1_t[0][:, 0, c0:c0 + 128],
                                     P6a[:, 13:14], start=True, stop=False)
                    nc.tensor.matmul(ps[:, 0:1], w6c1_t[1][:, 0, c0:c0 + 128],
                                     P6b[:, 13:14], start=False, stop=True)
                    nc.vector.tensor_copy(ot[:], ps[:, 0:1])
                for i, c0 in enumerate((0, 128)):
                    ps = pst.tile([128, 8], F32, tag="ps")
                    nc.tensor.matmul(ps[:, 0:1], w6c2_t[0][:, 0, c0:c0 + 128],
                                     X6a[:], start=True, stop=False)
                    nc.tensor.matmul(ps[:, 0:1], w6c2_t[1][:, 0, c0:c0 + 128],
                                     X6b[:], start=False, stop=True)
                    nc.scalar.copy(outt[:, i:i + 1], ps[:, 0:1])
            if DBG:
                nc.sync.dma_start(dbg_d["dX6a"][:], X6a[:])
            nc.sync.dma_start(out_d[0, 0:128], outt[:, 0])
            nc.sync.dma_start(out_d[0, 128:256], outt[:, 1])

    nc.compile()
    return nc



_CACHE = {}
_RUNNERS = {}


def kernel(features, coors, W0, W1, W2, W3, W4, W5, W6, W7, W8, W9, W10, W11,
           W12, W13):
    features = np.asarray(features, np.float32)
    coors = np.asarray(coors, np.int32)
    Ws = [np.asarray(w, np.float32) for w in
          (W0, W1, W2, W3, W4, W5, W6, W7, W8, W9, W10, W11, W12, W13)]
    in_maps, meta = build_host_inputs(features, coors, Ws)
    key = tuple(sorted(meta["mask_flags"].items()))
    if key not in _CACHE:
        _CACHE[key] = build_kernel(meta)
    nc = _CACHE[key]
    try:
        if key not in _RUNNERS:
            _RUNNERS[key] = _Runner(nc, NC)
        r = _RUNNERS[key]
        r.place(in_maps)
        outs = r.run()
        out = r.result(outs, "out").reshape(256)
    except Exception:
        res = run_bass_kernel_spmd(nc, in_maps, core_ids=list(range(NC)))
        out = res.results[0]["out"].reshape(256)
    return out.reshape(1, 1, 1, 1, 256).astype(np.float32)


if __name__ == "__main__":
    pass



# revision 19
# speedup vs baseline: 31928.6245x; 6.3309x over previous
"""Trainium2 Bass kernel for the sparse submanifold 3D CNN (nn_Net_38963943309313).

Network: 7 blocks of 2 submanifold 3x3x3 convs on a 64^3 grid, 2x2x2 sparse
max-pools between blocks, channels 3->64->...->256, output [1,1,1,1,256].

Strategy (8 NeuronCores):
 - Shard z-slabs across cores for levels 0-2 (grids 64/32/16), AllGather the
   pooled activations between levels (z-padded gather buffers so per-core
   reads are a single dynamic-offset DMA). Levels 3-6 (grids 8/4/2/1) are
   replicated on every core.
 - Convs are fp32r matmuls: activations channel-major [C, z, y, x] in SBUF
   (y/x zero-padded), 27 shifted-window matmuls accumulated in PSUM.
 - conv1 of block 0 uses a host-side im2col (81 contract rows, masked
   columns so the submanifold mask is free).
 - 64-channel contractions (L0 conv2, L1 conv1) pack z-pairs into K=128 via
   duplicated storage; L0 conv2 additionally pairs two output slices into
   the two 64-column halves of the PE array.
 - Submanifold masking: conv1 evictions multiply by a broadcast mask (also
   zeroes the out-of-grid halo slices); conv2 evictions add (mask-1)*BIG so
   the following max-pool ignores inactive voxels; pool result is multiplied
   by the pooled mask.
"""

import sys

sys.path.insert(0, "/opt/trn_rl_repo")

import numpy as np
import concourse.bass as bass
import concourse.tile as tile
from concourse.tile import add_dep_helper
from concourse import bacc, mybir
from concourse.bass_utils import run_bass_kernel_spmd


class _Runner:
    """Compile-once, inputs-resident executor.

    Replicates concourse.bass2jax.run_bass_via_pjrt's lowering, but keeps the
    jitted shard_map executable and the device-placed input buffers alive
    across calls, so repeat executions cost only the PJRT dispatch + the NEFF
    execution itself (run_bass_kernel_spmd rebuilds the jit closure and
    re-uploads every input on each call).
    """

    def __init__(self, nc, n_cores, fully_written_outputs=True):
        import jax
        from jax.experimental.shard_map import shard_map
        from jax.sharding import Mesh, PartitionSpec
        from concourse import bass2jax

        bass2jax.install_neuronx_cc_hook()
        self.jax = jax
        self.bass2jax = bass2jax
        self.nc, self.n = nc, n_cores
        partition_name = (nc.partition_id_tensor.name
                          if nc.partition_id_tensor else None)
        in_names, out_names, out_avals = [], [], []
        for alloc in nc.m.functions[0].allocations:
            if not isinstance(alloc, mybir.MemoryLocationSet):
                continue
            name = alloc.memorylocations[0].name
            if alloc.kind == "ExternalInput":
                if name != partition_name:
                    in_names.append(name)
            elif alloc.kind == "ExternalOutput":
                out_names.append(name)
                out_avals.append(jax.core.ShapedArray(
                    tuple(alloc.tensor_shape), mybir.dt.np(alloc.dtype)))
        self.param_names = list(in_names)
        self.out_names, self.out_avals = out_names, out_avals
        self.dbg_name = nc.dbg_addr.name if nc.dbg_addr is not None else None
        n_params, n_outs = len(in_names), len(out_names)
        # When every output tensor is fully written by the kernel, skip the
        # donated zero-initialized output buffers run_bass_via_pjrt uses —
        # they cost a host->device transfer per call.
        self.pass_out_bufs = not fully_written_outputs
        if self.pass_out_bufs:
            full_names = (in_names + out_names
                          + ([partition_name] if partition_name else []))
        else:
            full_names = in_names + ([partition_name] if partition_name else [])

        def _body(*args):
            operands = list(args)
            if partition_name is not None:
                operands.append(bass2jax.partition_id_tensor())
            outs = bass2jax._bass_exec_p.bind(
                *operands,
                out_avals=tuple(out_avals),
                in_names=tuple(full_names),
                out_names=tuple(out_names),
                lowering_input_output_aliases=(),
                sim_require_finite=True,
                sim_require_nnan=True,
                nc=nc,
            )
            return tuple(outs)

        devices = jax.devices()[:n_cores]
        self.mesh = Mesh(np.asarray(devices), ("core",))
        n_args = n_params + (n_outs if self.pass_out_bufs else 0)
        self._shmapped = shard_map(
            _body, mesh=self.mesh,
            in_specs=(PartitionSpec("core"),) * n_args,
            out_specs=(PartitionSpec("core"),) * n_outs,
            check_rep=False)
        self._donate = (tuple(range(n_params, n_params + n_outs))
                        if self.pass_out_bufs else ())
        self._dev_in = None
        self._compiled = None

    def place(self, in_maps):
        from jax.sharding import NamedSharding, PartitionSpec
        if self.dbg_name is not None:
            in_maps = [{**m, self.dbg_name: np.zeros((1, 2), np.uint32)}
                       for m in in_maps]
        sh = NamedSharding(self.mesh, PartitionSpec("core"))
        concat = [np.concatenate([np.asarray(m[name]) for m in in_maps], 0)
                  for name in self.param_names]
        self._dev_in = [self.jax.device_put(a, sh) for a in concat]
        self.jax.block_until_ready(self._dev_in)
        if self._compiled is None:
            jax = self.jax
            example = list(self._dev_in) + self._fresh_out_bufs()
            try:
                self._compiled = self.bass2jax.fast_dispatch_compile(
                    lambda: jax.jit(self._shmapped, donate_argnums=self._donate,
                                    keep_unused=True).lower(*example).compile())
            except Exception:
                self._compiled = jax.jit(
                    self._shmapped, donate_argnums=self._donate,
                    keep_unused=True)

    def _fresh_out_bufs(self):
        if not self.pass_out_bufs:
            return []
        return [np.zeros((self.n * av.shape[0], *av.shape[1:]), av.dtype)
                for av in self.out_avals]

    def run(self):
        outs = self._compiled(*self._dev_in, *self._fresh_out_bufs())
        return self.jax.block_until_ready(outs)

    def run_async(self):
        return self._compiled(*self._dev_in, *self._fresh_out_bufs())

    def build_batched(self, k):
        """Compile a program that executes the NEFF k times sequentially
        (ordered bass_effect tokens keep the k custom calls distinct and
        serialized). One dispatch then covers k real network evaluations."""
        import jax
        from jax.experimental.shard_map import shard_map
        from jax.sharding import PartitionSpec

        body = self._shmapped.__wrapped__ if hasattr(self._shmapped, "__wrapped__") else None
        # rebuild from scratch: same _body called k times
        nc, n = self.nc, self.n
        bass2jax = self.bass2jax
        partition_name = (nc.partition_id_tensor.name
                          if nc.partition_id_tensor else None)
        in_names = list(self.param_names)
        out_names, out_avals = self.out_names, self.out_avals
        full_names = in_names + ([partition_name] if partition_name else [])

        def _body_k(*args):
            outs = None
            for _ in range(k):
                operands = list(args)
                if partition_name is not None:
                    operands.append(bass2jax.partition_id_tensor())
                outs = bass2jax._bass_exec_p.bind(
                    *operands,
                    out_avals=tuple(out_avals),
                    in_names=tuple(full_names),
                    out_names=tuple(out_names),
                    lowering_input_output_aliases=(),
                    sim_require_finite=True,
                    sim_require_nnan=True,
                    nc=nc,
                )
            return tuple(outs)

        shm = shard_map(_body_k, mesh=self.mesh,
                        in_specs=(PartitionSpec("core"),) * len(in_names),
                        out_specs=(PartitionSpec("core"),) * len(out_names),
                        check_rep=False)
        self._batched_k = k
        self._batched = jax.jit(shm, keep_unused=True).lower(
            *self._dev_in).compile()

    def run_batched(self):
        outs = self._batched(*self._dev_in)
        return self.jax.block_until_ready(outs)

    def result(self, outs, name, core=0):
        i = self.out_names.index(name)
        av = self.out_avals[i]
        return np.asarray(outs[i]).reshape(self.n, *av.shape)[core]

NC = 8
GRID = 64
BIG = 1.0e30
CHANNELS = [(3, 64), (64, 64), (64, 96), (96, 96), (96, 128), (128, 128),
            (128, 160), (160, 160), (160, 192), (192, 192), (192, 224),
            (224, 224), (224, 256), (256, 256)]
F32 = mybir.dt.float32
F32R = mybir.dt.float32r

OFFSETS = [(dz, dy, dx) for dz in (-1, 0, 1) for dy in (-1, 0, 1) for dx in (-1, 0, 1)]
# 9 (dy,dx) pairs for z-pair-packed layers
DYDX = [(dy, dx) for dy in (-1, 0, 1) for dx in (-1, 0, 1)]

GENW_SHAPES = {
    "w1c2": [(96, 27, 96)], "w2c1": [(96, 27, 128)], "w2c2": [(128, 27, 128)],
    "w3c1": [(128, 27, 160)], "w3c2": [(128, 27, 160), (32, 27, 160)],
    "w4c1": [(128, 27, 192), (32, 27, 192)],
    "w4c2": [(128, 27, 192), (64, 27, 192)],
    "w5c1": [(128, 27, 224), (64, 27, 224)],
    "w5c2": [(128, 27, 224), (96, 27, 224)],
    "w6c1": [(128, 1, 256), (96, 1, 256)],
    "w6c2": [(128, 1, 256), (128, 1, 256)],
}

# Every constant input is packed into one flat f32 blob per core (single
# NEFF input tensor -> minimal per-dispatch arg overhead).
MANIFEST = [
    ("x1col", (10, 81, 4096)),
    ("w1col", (81, 128)),
    ("wq1", (128, 9, 128)), ("wq2", (128, 9, 128)),
    ("w1p", (128, 9, 96)), ("w1l", (128, 9, 96)),
    ("mn0", (8, 4096)), ("m1p", (4, 1024)),
    ("m1mul", (6, 1024)), ("mn1", (4, 1024)), ("m2p", (2, 256)),
    ("m2mul", (4, 256)), ("mn2", (2, 256)), ("m3p", (1, 64)),
    ("m3mul", (10, 64)), ("mn3", (8, 64)), ("m4p", (4, 16)),
    ("m4mul", (4, 16)), ("mn4", (4, 16)), ("m5p", (2, 4)),
    ("m5mul", (2, 4)), ("mn5", (2, 4)), ("m6p", (1, 1)),
] + [(f"{n}_{i}", s) for n, shl in GENW_SHAPES.items() for i, s in enumerate(shl)]
BLOB_SIZE = sum(int(np.prod(s)) for _, s in MANIFEST)


def _pool_np(x, m):
    # x: [D,D,D,C] or [D,D,D]; max over active voxels of 2x2x2 windows
    D = x.shape[0]
    if x.ndim == 3:
        xr = x.reshape(D // 2, 2, D // 2, 2, D // 2, 2)
        return xr.max(axis=(1, 3, 5))
    neg = np.where(m[..., None] > 0, x, -np.inf)
    xr = neg.reshape(D // 2, 2, D // 2, 2, D // 2, 2, -1)
    p = xr.max(axis=(1, 3, 5))
    mp = m.reshape(D // 2, 2, D // 2, 2, D // 2, 2).max(axis=(1, 3, 5))
    return np.where(mp[..., None] > 0, p, 0.0), mp


def _ceil_div(a, b):
    return (a + b - 1) // b


def build_host_inputs(features, coors, Ws):
    """All host-side data marshalling. Returns (in_maps, meta)."""
    z, y, x = coors[:, 0], coors[:, 1], coors[:, 2]
    dense = np.zeros((GRID, GRID, GRID, 3), np.float32)
    mask0 = np.zeros((GRID, GRID, GRID), np.float32)
    dense[z, y, x] = features  # last write wins (matches XLA CPU scatter)
    mask0[z, y, x] = 1.0

    # mask pyramid
    masks = [mask0]
    m = mask0
    for _ in range(6):
        mr = m.reshape(m.shape[0] // 2, 2, m.shape[1] // 2, 2, m.shape[2] // 2, 2)
        m = mr.max(axis=(1, 3, 5))
        masks.append(m)

    # ---- X1col: host im2col for conv1 of block 0, column-masked ----
    # padded dense [3, 66, 66, 66]
    dpad = np.zeros((3, GRID + 2, GRID + 2, GRID + 2), np.float32)
    dpad[:, 1:-1, 1:-1, 1:-1] = dense.transpose(3, 0, 1, 2)
    # X1col_full[z, (dz,dy,dx,ci), y*64+x] = dpad[ci, z+dz+1, y+dy+1, x+dx+1]
    sw = np.lib.stride_tricks.sliding_window_view(dpad, (3, 3, 3),
                                                  axis=(1, 2, 3))
    # sw[ci, z, y, x, a, b, c] = dpad[ci, z+a, y+b, x+c]
    xfull = np.ascontiguousarray(sw.transpose(1, 4, 5, 6, 0, 2, 3)).reshape(
        GRID, 81, GRID * GRID)
    xfull *= mask0.reshape(GRID, 1, GRID * GRID)
    xpad = np.zeros((GRID + 10, 81, GRID * GRID), np.float32)
    xpad[1:GRID + 1] = xfull
    # core k conv1-out slices global [8k-1, 8k+9)
    x1cols = [xpad[8 * k:8 * k + 10] for k in range(NC)]

    # ---- weight packs ----
    # W0 for im2col conv1: [81, 128] (co=64 duplicated for col-pairing)
    W0 = Ws[0]  # [3,3,3,3,64]
    w1col = np.zeros((81, 128), np.float32)
    for o, (dz, dy, dx) in enumerate(OFFSETS):
        w1col[o * 3:(o + 1) * 3, 0:64] = W0[dz + 1, dy + 1, dx + 1]
        w1col[o * 3:(o + 1) * 3, 64:128] = W0[dz + 1, dy + 1, dx + 1]

    def pack_pair(W):  # [3,3,3,cin,co] -> pair [2*cin, 9, co] + left [cin, 9, co]
        cin, co = W.shape[3], W.shape[4]
        wp = np.zeros((2 * cin, 9, co), np.float32)
        wl = np.zeros((cin, 9, co), np.float32)
        for j, (dy, dx) in enumerate(DYDX):
            wp[0:cin, j] = W[0, dy + 1, dx + 1]      # dz=-1
            wp[cin:2 * cin, j] = W[1, dy + 1, dx + 1]  # dz=0
            wl[:, j] = W[2, dy + 1, dx + 1]          # dz=+1
        return wp, wl

    # L0 conv2 64->64: output-pair block packs. Rows = z-pair of inputs,
    # cols = (out z | out z+1). Two K=128,M=128 matmuls per tap cover all 6
    # z-tap contributions of an output pair:
    #   mm1 rows (h[z],h[z+1]):   [[W0, 0 ], [W1, W0]]
    #   mm2 rows (h[z+2],h[z+3]): [[W2, W1], [0,  W2]]
    WL0 = Ws[1]
    wq1 = np.zeros((128, 9, 128), np.float32)
    wq2 = np.zeros((128, 9, 128), np.float32)
    for j, (dy, dx) in enumerate(DYDX):
        W0t, W1t, W2t = (WL0[0, dy + 1, dx + 1], WL0[1, dy + 1, dx + 1],
                         WL0[2, dy + 1, dx + 1])
        wq1[0:64, j, 0:64] = W0t
        wq1[64:128, j, 0:64] = W1t
        wq1[64:128, j, 64:128] = W0t
        wq2[0:64, j, 0:64] = W2t
        wq2[0:64, j, 64:128] = W1t
        wq2[64:128, j, 64:128] = W2t

    w1p, w1l = pack_pair(Ws[2])   # L1 conv1 64->96
    w1l = np.concatenate([w1l, w1l], axis=0)  # [128, 9, 96]

    def pack_generic(W):  # -> list of [kchunk, 27, co] arrays
        cin, co = W.shape[3], W.shape[4]
        wf = W.reshape(27, cin, co)
        out = []
        for k0 in range(0, cin, 128):
            kc = min(128, cin - k0)
            out.append(np.ascontiguousarray(
                wf[:, k0:k0 + kc, :].transpose(1, 0, 2)))  # [kc, 27, co]
        return out

    gen_w = {}
    for li, wi in [("w1c2", 3), ("w2c1", 4), ("w2c2", 5), ("w3c1", 6),
                   ("w3c2", 7), ("w4c1", 8), ("w4c2", 9), ("w5c1", 10),
                   ("w5c2", 11)]:
        gen_w[li] = pack_generic(Ws[wi])
    # L6: center tap only (1^3 grid)
    for li, wi in [("w6c1", 12), ("w6c2", 13)]:
        W = Ws[wi]
        cin, co = W.shape[3], W.shape[4]
        wc = W[1, 1, 1]  # [cin, co]
        gen_w[li] = [np.ascontiguousarray(wc[k0:k0 + min(128, cin - k0)][:, None, :])
                     for k0 in range(0, cin, 128)]

    # ---- per-core mask arrays ----
    # L0 maskneg for conv2-evict: [8, 4096]
    mn0 = [((masks[0][8 * k:8 * k + 8] - 1.0) * BIG).reshape(8, -1).astype(np.float32)
           for k in range(NC)]
    # L0 pool-out multiply: m1 on core's L1 slices [4, 1024]
    m1p = [masks[1][4 * k:4 * k + 4].reshape(4, -1).astype(np.float32)
           for k in range(NC)]

    def slab_mask(mask, z0, nsl):
        D2 = mask.shape[1] * mask.shape[2]
        out = np.zeros((nsl, D2), np.float32)
        for i in range(nsl):
            zg = z0 + i
            if 0 <= zg < mask.shape[0]:
                out[i] = mask[zg].reshape(-1)
        return out

    # L1 conv1-evict multiply mask (m1 x ingrid): slices [4k-1, 4k+5)
    m1mul = [slab_mask(masks[1], 4 * k - 1, 6) for k in range(NC)]
    # L1 conv2-evict maskneg: slices [4k, 4k+4)
    mn1 = [((slab_mask(masks[1], 4 * k, 4) - 1.0) * BIG).astype(np.float32)
           for k in range(NC)]
    # L1 pool-out multiply: m2 on core's L2 slices [2, 256]
    m2p = [slab_mask(masks[2], 2 * k, 2) for k in range(NC)]
    # L2 conv1-evict multiply (m2 x ingrid): slices [2k-1, 2k+3)
    m2mul = [slab_mask(masks[2], 2 * k - 1, 4) for k in range(NC)]
    # L2 conv2-evict maskneg: slices [2k, 2k+2)
    mn2 = [((slab_mask(masks[2], 2 * k, 2) - 1.0) * BIG).astype(np.float32)
           for k in range(NC)]
    # L2 pool-out multiply: m3 on core's L3 slice [1, 64]
    m3p = [slab_mask(masks[3], k, 1) for k in range(NC)]
    # L3 (replicated): conv1-evict mul (m3 x ingrid) slices [-1, 9)
    m3mul_r = slab_mask(masks[3], -1, 10)
    mn3_r = ((slab_mask(masks[3], 0, 8) - 1.0) * BIG).astype(np.float32)
    m4p_r = slab_mask(masks[4], 0, 4)       # [4, 16]
    m4mul_r = slab_mask(masks[4], 0, 4)     # L4 out all valid (full grid)
    mn4_r = ((slab_mask(masks[4], 0, 4) - 1.0) * BIG).astype(np.float32)
    m5p_r = slab_mask(masks[5], 0, 2)
    m5mul_r = slab_mask(masks[5], 0, 2)
    mn5_r = ((slab_mask(masks[5], 0, 2) - 1.0) * BIG).astype(np.float32)
    m6p_r = slab_mask(masks[6], 0, 1)

    meta = {
        "mask_flags": {
            # whether the real mask (not just ingrid) has zeros at each level
            1: not np.all(masks[1] == 1.0),
            2: not np.all(masks[2] == 1.0),
            3: not np.all(masks[3] == 1.0),
            4: not np.all(masks[4] == 1.0),
            5: not np.all(masks[5] == 1.0),
            6: not np.all(masks[6] == 1.0),
        },
    }

    in_maps = []
    for k in range(NC):
        im = {
            "x1col": x1cols[k],
            "w1col": w1col,
            "wq1": wq1, "wq2": wq2, "w1p": w1p, "w1l": w1l,
            "mn0": mn0[k], "m1p": m1p[k],
            "m1mul": m1mul[k], "mn1": mn1[k], "m2p": m2p[k],
            "m2mul": m2mul[k], "mn2": mn2[k], "m3p": m3p[k],
            "m3mul": m3mul_r, "mn3": mn3_r, "m4p": m4p_r,
            "m4mul": m4mul_r, "mn4": mn4_r, "m5p": m5p_r,
            "m5mul": m5mul_r, "mn5": mn5_r, "m6p": m6p_r,
        }
        for name, chunks in gen_w.items():
            for ci, arr in enumerate(chunks):
                im[f"{name}_{ci}"] = arr
        parts = []
        for name, sh in MANIFEST:
            a = np.ascontiguousarray(im[name], np.float32)
            assert a.shape == sh, (name, a.shape, sh)
            parts.append(a.reshape(-1))
        in_maps.append({"blob": np.concatenate(parts)})
    return in_maps, meta


def build_kernel(meta):
    import contextlib
    nc = bacc.Bacc("TRN2", target_bir_lowering=False, debug=False, num_devices=NC)
    mf = meta["mask_flags"]

    # ---------- DRAM I/O declarations ----------
    # single flat input blob; every constant is an AP view into it
    blob_d = nc.dram_tensor("blob", [BLOB_SIZE], F32, kind="ExternalInput")
    views = {}
    off = 0
    for name, sh in MANIFEST:
        n = int(np.prod(sh))
        v = blob_d[off:off + n]
        if len(sh) == 2:
            v = v.rearrange("(a b) -> a b", a=sh[0], b=sh[1])
        elif len(sh) == 3:
            v = v.rearrange("(a b c) -> a b c", a=sh[0], b=sh[1], c=sh[2])
        views[name] = v
        off += n

    x1col = views["x1col"]
    w1col_d = views["w1col"]
    wq1_d = views["wq1"]; wq2_d = views["wq2"]
    w1p_d = views["w1p"]; w1l_d = views["w1l"]
    mn0_d = views["mn0"]; m1p_d = views["m1p"]
    m1mul_d = views["m1mul"]; mn1_d = views["mn1"]
    m2p_d = views["m2p"]
    m2mul_d = views["m2mul"]; mn2_d = views["mn2"]
    m3p_d = views["m3p"]
    m3mul_d = views["m3mul"]; mn3_d = views["mn3"]
    m4p_d = views["m4p"]; m4mul_d = views["m4mul"]
    mn4_d = views["mn4"]
    m5p_d = views["m5p"]; m5mul_d = views["m5mul"]
    mn5_d = views["mn5"]; m6p_d = views["m6p"]

    genw_d = {name: [views[f"{name}_{i}"] for i in range(len(shl))]
              for name, shl in GENW_SHAPES.items()}

    out_d = nc.dram_tensor("out", [1, 256], F32, kind="ExternalOutput")
    import os as _os
    DBG = bool(_os.environ.get("K_DEBUG"))
    dbg_d = {}
    if DBG:
        for nm, sh in [("dP0", (64, 4, 1156)), ("dA1", (128, 8, 1156)),
                       ("dB1", (96, 6, 1156)), ("dC1", (96, 4, 1024)),
                       ("dP1", (96, 2, 324)), ("dA2", (96, 6, 324)),
                       ("dB2", (128, 4, 324)), ("dC2", (128, 2, 256)),
                       ("dP2", (128, 1, 100)), ("dA3", (128, 12, 100)),
                       ("dB3a", (128, 10, 100)), ("dC3a", (128, 512)),
                       ("dP4a", (128, 216)), ("dB4a", (128, 216)),
                       ("dC4a", (128, 64)), ("dP5a", (128, 64)),
                       ("dB5a", (128, 64)), ("dP6a", (128, 27)),
                       ("dX6a", (128, 1)), ("dC0", (128, 4096))]:
            dbg_d[nm] = nc.dram_tensor(nm, list(sh), F32, kind="ExternalOutput")

    with tile.TileContext(nc) as tc:
        ctx = contextlib.ExitStack()
        with ctx:
            pst = ctx.enter_context(tc.tile_pool(name="ps", bufs=4, space="PSUM"))
            drm = ctx.enter_context(tc.tile_pool(name="dram", bufs=1, space="DRAM"))
            glob = ctx.enter_context(tc.tile_pool(name="glob", bufs=1))

            pid = nc.sync.partition_id()

            _wl_ctr = [0]

            def wload(pool, d, shape=None, name=None, dt=F32R):
                sh = shape or d.shape
                _wl_ctr[0] += 1
                t = pool.tile(list(sh), dt,
                              name=name or f"sb_w{_wl_ctr[0]}")
                nc.sync.dma_start(t[:], d[:].bitcast(dt) if dt is F32R else d[:])
                return t

            # zero tile for G-pad zeroing
            zt = glob.tile([128, 1156], F32)
            nc.vector.memset(zt[:], 0.0)

            # DRAM gather buffers (Shared would need single-writer; the halo
            # pad zeroing writes preclude it)
            c1_d = drm.tile([4, 64, 1156], F32)
            G1 = drm.tile([36, 64, 1156], F32)
            c2_d = drm.tile([2, 96, 324], F32)
            G2 = drm.tile([20, 96, 324], F32)
            c3_d = drm.tile([1, 128, 100], F32)
            G3 = drm.tile([12, 128, 100], F32)
            gpad_insts = []
            for G, csz, npad in ((G1, (64, 1156), 2), (G2, (96, 324), 2),
                                 (G3, (128, 100), 2)):
                n = G.shape[0]
                for s in list(range(npad)) + list(range(n - npad, n)):
                    gpad_insts.append(
                        nc.sync.dma_start(G[s], zt[0:csz[0], 0:csz[1]]))

            # persistent tail tensors (small; cross level boundaries)
            P4a = glob.tile([128, 216], F32); P4b = glob.tile([32, 216], F32)
            P5a = glob.tile([128, 64], F32); P5b = glob.tile([64, 64], F32)
            P6a = glob.tile([128, 27], F32); P6b = glob.tile([96, 27], F32)
            X6a = glob.tile([128, 1], F32); X6b = glob.tile([128, 1], F32)
            outt = glob.tile([128, 2], F32)
            for t in (P4a, P4b, P5a, P5b, P6a, P6b):
                nc.vector.memset(t[:].bitcast(F32), 0.0)

            # ================ LEVEL 0 ================
            with tc.tile_pool(name="l0w", bufs=1) as wp, \
                 tc.tile_pool(name="l0p", bufs=1) as pp, \
                 tc.tile_pool(name="l0s", bufs=2) as ss, \
                 tc.tile_pool(name="l0m", bufs=4) as sm:
                w1col_t = wload(wp, w1col_d, name="w1col_t")
                wq1_t = wload(wp, wq1_d, name="wq1_t")
                wq2_t = wload(wp, wq2_d, name="wq2_t")

                A0 = pp.tile([128, 4, 4356], F32R)
                C0 = pp.tile([128, 4096], F32R)   # rows0=out z, rows64=out z+1
                P0 = pp.tile([64, 4, 1156], F32R)
                for _s in range(4):
                    nc.vector.memset(A0[:, _s, :].bitcast(F32), 0.0)
                nc.vector.memset(P0[:].bitcast(F32), 0.0)

                def l0_conv1(sl):
                    # psum M=128 (w1col cols are co duplicated); evict the two
                    # h1 copies from the matching psum partition halves so no
                    # cross-partition copy is needed.
                    xs = ss.tile([81, 4096], F32R, tag="x1s")
                    nc.sync.dma_start(xs[:], x1col[sl].bitcast(F32R))
                    for chunk in range(8):
                        ps = pst.tile([128, 512], F32, tag="ps")
                        nc.tensor.matmul(ps[:], w1col_t[:, :],
                                         xs[:, chunk * 512:chunk * 512 + 512],
                                         start=True, stop=True)
                        r0, r1 = sl % 4, (sl - 1) % 4
                        yb = chunk * 8
                        src = ps[:].rearrange("p (a b) -> p a b", b=64)
                        d0 = A0[0:64, r0, :].rearrange("p (a b) -> p a b", b=66)
                        d1 = A0[64:128, r1, :].rearrange("p (a b) -> p a b", b=66)
                        nc.scalar.copy(d0[:, yb + 1:yb + 9, 1:65], src[0:64])
                        nc.vector.tensor_copy(d1[:, yb + 1:yb + 9, 1:65],
                                              src[64:128])

                def l0_conv2(z):
                    # output pair (z, z+1) in one M=128 psum: two K=128 block
                    # matmuls per tap. Ring slot r: rows0=h1[r], rows64=h1[r+1].
                    s1 = z % 4         # pair (h1[z],   h1[z+1])
                    s2 = (z + 2) % 4   # pair (h1[z+2], h1[z+3])
                    w1v = A0[:, s1, :].rearrange("p (a b) -> p a b", b=66)
                    w2v = A0[:, s2, :].rearrange("p (a b) -> p a b", b=66)
                    for chunk in range(8):
                        yb = chunk * 8
                        ps = pst.tile([128, 512], F32, tag="ps")
                        v = ps[:].rearrange("p (a b) -> p a b", b=64)
                        for j, (dy, dx) in enumerate(DYDX):
                            ys = slice(yb + 1 + dy, yb + 9 + dy)
                            xsl = slice(1 + dx, 65 + dx)
                            nc.tensor.matmul(v, wq1_t[:, j, :],
                                             w1v[:, ys, xsl],
                                             start=(j == 0), stop=False)
                            nc.tensor.matmul(v, wq2_t[:, j, :],
                                             w2v[:, ys, xsl],
                                             start=False, stop=(j == 8))
                        mt = sm.tile([128, 512], F32, tag="mn0")
                        nc.sync.dma_start(
                            mt[0:64, :], mn0_d[z, yb * 64:yb * 64 + 512]
                            .unsqueeze(0).to_broadcast((64, 512)))
                        nc.sync.dma_start(
                            mt[64:128, :], mn0_d[z + 1, yb * 64:yb * 64 + 512]
                            .unsqueeze(0).to_broadcast((64, 512)))
                        nc.vector.tensor_add(
                            C0[:, yb * 64:yb * 64 + 512], ps[:], mt[:])

                def l0_pool(z):
                    zp = z // 2
                    # bring the odd output slice down to partitions 0:64
                    E0 = ss.tile([64, 4096], F32R, tag="pool0e", bufs=1)
                    nc.sync.dma_start(E0[:], C0[64:128, :])
                    nc.vector.tensor_max(E0[:], C0[0:64, :], E0[:])
                    v = E0[:].rearrange("p (a b) -> p a b", b=64)
                    t2 = ss.tile([64, 32, 64], F32R, tag="pool0b", bufs=1)
                    nc.vector.tensor_max(t2[:], v[:, 0::2, :], v[:, 1::2, :])
                    t3 = ss.tile([64, 32, 32], F32R, tag="pool0c", bufs=1)
                    nc.vector.tensor_max(t3[:], t2[:, :, 0::2], t2[:, :, 1::2])
                    mt = sm.tile([64, 1024], F32, tag="m1p", bufs=2)
                    nc.sync.dma_start(mt[:], m1p_d[zp].unsqueeze(0)
                                      .to_broadcast((64, 1024)))
                    dst = P0[:, zp, :].rearrange("p (a b) -> p a b", b=34)
                    nc.vector.tensor_mul(
                        dst[:, 1:33, 1:33], t3[:],
                        mt[:].rearrange("p (a b) -> p a b", b=32))

                for sl in range(10):
                    l0_conv1(sl)
                    if sl >= 3 and (sl - 3) % 2 == 0:
                        zz = sl - 3
                        l0_conv2(zz)
                        l0_pool(zz)

                if DBG:
                    nc.sync.dma_start(dbg_d["dP0"][:], P0[:].bitcast(F32))
                    nc.sync.dma_start(dbg_d["dC0"][:], C0[:].bitcast(F32))
                nc.sync.dma_start(c1_d[:].rearrange("z c v -> c z v"),
                                  P0[:].bitcast(F32))

            # ---- AllGather L0 -> L1 ----
            ag1 = nc.gpsimd.collective_compute(
                "AllGather", mybir.AluOpType.bypass,
                replica_groups=[list(range(NC))],
                ins=[c1_d[:].opt()], outs=[G1[2:34].opt()])
            for gi in gpad_insts:
                add_dep_helper(ag1.ins, gi.ins, reason="G pads zeroed before gathers")

            # ================ LEVEL 1 ================
            with tc.tile_pool(name="l1w", bufs=1) as wp, \
                 tc.tile_pool(name="l1p", bufs=1) as pp, \
                 tc.tile_pool(name="l1s", bufs=2) as ss, \
                 tc.tile_pool(name="l1m", bufs=4) as sm:
                w1p_t = wload(wp, w1p_d)
                w1l_t = wload(wp, w1l_d)
                w1c2_t = wload(wp, genw_d["w1c2"][0])

                A1 = pp.tile([128, 8, 1156], F32R)
                B1 = pp.tile([96, 6, 1156], F32R)
                C1 = pp.tile([96, 4, 1024], F32R)
                P1 = pp.tile([96, 2, 324], F32R)
                nc.vector.memset(B1[:].bitcast(F32), 0.0)
                nc.vector.memset(P1[:].bitcast(F32), 0.0)
                _r1 = nc.sync.dma_start(A1[0:64, :, :],
                                  G1[bass.ds(pid * 4, 8)].rearrange("z c v -> c z v").bitcast(F32R))
                _r2 = nc.sync.dma_start(A1[64:128, 0:7, :],
                                  G1[bass.ds(pid * 4 + 1, 7)].rearrange("z c v -> c z v").bitcast(F32R))
                add_dep_helper(_r1.ins, ag1.ins, reason="gather before dynamic read")
                add_dep_helper(_r2.ins, ag1.ins, reason="gather before dynamic read")

                def l1_conv1(sl):
                    # A1 rows0 idx i = x1[4k-2+i]; rows64 idx i = x1[4k-1+i].
                    # out sl (global 4k-1+sl): pair = A1[:, sl] (dz=-1,0);
                    # leftover dz=+1 = rows64 idx sl+1 == rows0 idx sl+2.
                    mt = sm.tile([96, 1024], F32, tag="m1mul")
                    nc.sync.dma_start(mt[:], m1mul_d[sl].unsqueeze(0)
                                      .to_broadcast((96, 1024)))
                    pss = [pst.tile([96, 512], F32, tag="ps", name=f"ps_l1_{sl}_{_c}") for _c in range(2)]
                    wA = A1[:, sl, :].rearrange("p (a b) -> p a b", b=34)
                    wB = A1[64:128, sl + 1, :].rearrange("p (a b) -> p a b", b=34)
                    wC = A1[0:64, sl + 2, :].rearrange("p (a b) -> p a b", b=34)
                    for j, (dy, dx) in enumerate(DYDX):
                        xsl = slice(1 + dx, 33 + dx)
                        for chunk in range(2):
                            yb = chunk * 16
                            ys = slice(yb + 1 + dy, yb + 17 + dy)
                            nc.tensor.matmul(
                                pss[chunk][:].rearrange("p (a b) -> p a b", b=32),
                                w1p_t[:, j, :], wA[:, ys, xsl],
                                start=(j == 0), stop=False)
                        # row-paired leftovers: chunk0 on rows 64:128,
                        # chunk1 on rows 0:64 (concurrent row groups)
                        ys0 = slice(1 + dy, 17 + dy)
                        ys1 = slice(17 + dy, 33 + dy)
                        nc.tensor.matmul(
                            pss[0][:].rearrange("p (a b) -> p a b", b=32),
                            w1l_t[64:128, j, :], wB[:, ys0, xsl],
                            start=False, stop=(j == 8))
                        nc.tensor.matmul(
                            pss[1][:].rearrange("p (a b) -> p a b", b=32),
                            w1l_t[0:64, j, :], wC[:, ys1, xsl],
                            start=False, stop=(j == 8))
                    for chunk in range(2):
                        yb = chunk * 16
                        dst = B1[:, sl, :].rearrange("p (a b) -> p a b", b=34)
                        nc.vector.tensor_mul(
                            dst[:, yb + 1:yb + 17, 1:33],
                            pss[chunk][:].rearrange("p (a b) -> p a b", b=32),
                            mt[:, yb * 32:yb * 32 + 512].rearrange(
                                "p (a b) -> p a b", b=32))

                def l1_conv2(sl):
                    mt = sm.tile([96, 1024], F32, tag="mn1")
                    nc.sync.dma_start(mt[:], mn1_d[sl].unsqueeze(0)
                                      .to_broadcast((96, 1024)))
                    for chunk in range(2):
                        yb = chunk * 16
                        ps = pst.tile([96, 512], F32, tag="ps")
                        for o, (dz, dy, dx) in enumerate(OFFSETS):
                            w = B1[:, sl + 1 + dz, :].rearrange(
                                "p (a b) -> p a b", b=34)
                            nc.tensor.matmul(
                                ps[:].rearrange("p (a b) -> p a b", b=32),
                                w1c2_t[:, o, :],
                                w[:, yb + 1 + dy:yb + 17 + dy, 1 + dx:33 + dx],
                                start=(o == 0), stop=(o == 26))
                        nc.vector.tensor_add(C1[:, sl, yb * 32:yb * 32 + 512],
                                             ps[:],
                                             mt[:, yb * 32:yb * 32 + 512])

                def l1_pool(zz):
                    zp = zz // 2
                    nc.vector.tensor_max(C1[:, zz, :], C1[:, zz, :], C1[:, zz + 1, :])
                    v = C1[:, zz, :].rearrange("p (a b) -> p a b", b=32)
                    t2 = ss.tile([96, 16, 32], F32R, tag="pool1b")
                    nc.vector.tensor_max(t2[:], v[:, 0::2, :], v[:, 1::2, :])
                    t3 = ss.tile([96, 16, 16], F32R, tag="pool1c")
                    nc.vector.tensor_max(t3[:], t2[:, :, 0::2], t2[:, :, 1::2])
                    mt = sm.tile([96, 256], F32, tag="m2p")
                    nc.sync.dma_start(mt[:], m2p_d[zp].unsqueeze(0)
                                      .to_broadcast((96, 256)))
                    dst = P1[:, zp, :].rearrange("p (a b) -> p a b", b=18)
                    nc.vector.tensor_mul(
                        dst[:, 1:17, 1:17], t3[:],
                        mt[:].rearrange("p (a b) -> p a b", b=16))

                for sl in range(6):
                    l1_conv1(sl)
                    if sl >= 2:
                        l1_conv2(sl - 2)
                        if sl >= 3 and (sl - 3) % 2 == 0:
                            l1_pool(sl - 3)

                if DBG:
                    nc.sync.dma_start(dbg_d["dA1"][:], A1[:].bitcast(F32))
                    nc.sync.dma_start(dbg_d["dB1"][:], B1[:].bitcast(F32))
                    nc.sync.dma_start(dbg_d["dC1"][:], C1[:].bitcast(F32))
                    nc.sync.dma_start(dbg_d["dP1"][:], P1[:].bitcast(F32))
                nc.sync.dma_start(c2_d[:].rearrange("z c v -> c z v"),
                                  P1[:].bitcast(F32))

            # ---- AllGather L1 -> L2 ----
            ag2 = nc.gpsimd.collective_compute(
                "AllGather", mybir.AluOpType.bypass,
                replica_groups=[list(range(NC))],
                ins=[c2_d[:].opt()], outs=[G2[2:18].opt()])
            for gi in gpad_insts:
                add_dep_helper(ag2.ins, gi.ins, reason="G pads zeroed before gathers")

            # ================ LEVEL 2 ================
            with tc.tile_pool(name="l2w", bufs=1) as wp, \
                 tc.tile_pool(name="l2p", bufs=1) as pp, \
                 tc.tile_pool(name="l2s", bufs=2) as ss, \
                 tc.tile_pool(name="l2m", bufs=4) as sm:
                w2c1_t = wload(wp, genw_d["w2c1"][0])
                w2c2_t = wload(wp, genw_d["w2c2"][0])
                A2 = pp.tile([96, 6, 324], F32R)
                B2 = pp.tile([128, 4, 324], F32R)
                C2 = pp.tile([128, 2, 256], F32R)
                P2 = pp.tile([128, 1, 100], F32R)
                nc.vector.memset(B2[:].bitcast(F32), 0.0)
                nc.vector.memset(P2[:].bitcast(F32), 0.0)
                _r3 = nc.sync.dma_start(A2[:], G2[bass.ds(pid * 2, 6)].rearrange("z c v -> c z v").bitcast(F32R))
                add_dep_helper(_r3.ins, ag2.ins, reason="gather before dynamic read")

                for s0 in (0, 2):
                    ps = pst.tile([128, 512], F32, tag="ps")
                    for o, (dz, dy, dx) in enumerate(OFFSETS):
                        w = A2[:].rearrange("p z (a b) -> p z a b", b=18)
                        nc.tensor.matmul(
                            ps[:].rearrange("p (z a b) -> p z a b", z=2, a=16),
                            w2c1_t[:, o, :],
                            w[:, s0 + dz + 1:s0 + dz + 3,
                              1 + dy:17 + dy, 1 + dx:17 + dx],
                            start=(o == 0), stop=(o == 26))
                    mt = sm.tile([128, 512], F32, tag="m2mul")
                    nc.sync.dma_start(
                        mt[:], m2mul_d[s0:s0 + 2].flatten().unsqueeze(0)
                        .to_broadcast((128, 512)))
                    dst = B2[:].rearrange("p z (a b) -> p z a b", b=18)
                    nc.vector.tensor_mul(
                        dst[:, s0:s0 + 2, 1:17, 1:17],
                        ps[:].rearrange("p (z a b) -> p z a b", z=2, a=16),
                        mt[:].rearrange("p (z a b) -> p z a b", z=2, a=16))

                ps = pst.tile([128, 512], F32, tag="ps")
                for o, (dz, dy, dx) in enumerate(OFFSETS):
                    w = B2[:].rearrange("p z (a b) -> p z a b", b=18)
                    nc.tensor.matmul(
                        ps[:].rearrange("p (z a b) -> p z a b", z=2, a=16),
                        w2c2_t[:, o, :],
                        w[:, dz + 1:dz + 3, 1 + dy:17 + dy, 1 + dx:17 + dx],
                        start=(o == 0), stop=(o == 26))
                if mf[2]:
                    mt = sm.tile([128, 512], F32, tag="mn2")
                    nc.sync.dma_start(mt[:], mn2_d[:].flatten().unsqueeze(0)
                                      .to_broadcast((128, 512)))
                    nc.vector.tensor_add(C2[:].rearrange("p a b -> p (a b)"),
                                         ps[:], mt[:])
                else:
                    nc.scalar.copy(C2[:].rearrange("p a b -> p (a b)"), ps[:])

                # L2 pool
                nc.vector.tensor_max(C2[:, 0, :], C2[:, 0, :], C2[:, 1, :])
                v = C2[:, 0, :].rearrange("p (a b) -> p a b", b=16)
                t2 = ss.tile([128, 8, 16], F32R, tag="pool2b")
                nc.vector.tensor_max(t2[:], v[:, 0::2, :], v[:, 1::2, :])
                dst = P2[:, 0, :].rearrange("p (a b) -> p a b", b=10)
                if mf[3]:
                    t3 = ss.tile([128, 8, 8], F32R, tag="pool2c")
                    nc.vector.tensor_max(t3[:], t2[:, :, 0::2], t2[:, :, 1::2])
                    mt = sm.tile([128, 64], F32, tag="m3p")
                    nc.sync.dma_start(mt[:], m3p_d[0].unsqueeze(0)
                                      .to_broadcast((128, 64)))
                    nc.vector.tensor_mul(
                        dst[:, 1:9, 1:9], t3[:],
                        mt[:].rearrange("p (a b) -> p a b", b=8))
                else:
                    nc.vector.tensor_max(dst[:, 1:9, 1:9],
                                         t2[:, :, 0::2], t2[:, :, 1::2])

                if DBG:
                    nc.sync.dma_start(dbg_d["dA2"][:], A2[:].bitcast(F32))
                    nc.sync.dma_start(dbg_d["dB2"][:], B2[:].bitcast(F32))
                    nc.sync.dma_start(dbg_d["dC2"][:], C2[:].bitcast(F32))
                    nc.sync.dma_start(dbg_d["dP2"][:], P2[:].bitcast(F32))
                nc.sync.dma_start(c3_d[:].rearrange("z c v -> c z v"),
                                  P2[:].bitcast(F32))

            # ---- AllGather L2 -> L3 ----
            ag3 = nc.gpsimd.collective_compute(
                "AllGather", mybir.AluOpType.bypass,
                replica_groups=[list(range(NC))],
                ins=[c3_d[:].opt()], outs=[G3[2:10].opt()])
            for gi in gpad_insts:
                add_dep_helper(ag3.ins, gi.ins, reason="G pads zeroed before gathers")

            # ================ LEVEL 3 (replicated) ================
            with tc.tile_pool(name="l3w", bufs=1) as wp, \
                 tc.tile_pool(name="l3p", bufs=1) as pp, \
                 tc.tile_pool(name="l3s", bufs=2) as ss, \
                 tc.tile_pool(name="l3m", bufs=4) as sm:
                w3c1_t = wload(wp, genw_d["w3c1"][0])
                w3c2_t = [wload(wp, d) for d in genw_d["w3c2"]]
                A3 = pp.tile([128, 12, 100], F32R)
                B3a = pp.tile([128, 10, 100], F32R)
                B3b = pp.tile([32, 10, 100], F32R)
                C3a = pp.tile([128, 512], F32R)
                C3b = pp.tile([32, 512], F32R)
                nc.vector.memset(B3a[:].bitcast(F32), 0.0)
                nc.vector.memset(B3b[:].bitcast(F32), 0.0)
                _r4 = nc.sync.dma_start(A3[:], G3[:].rearrange("z c v -> c z v").bitcast(F32R))
                add_dep_helper(_r4.ins, ag3.ins, reason="gather before read")

                # conv1
                for (z0, nz) in ((0, 8), (2, 8)):
                    N = nz * 64
                    for (c0, co_n) in ((0, 128), (128, 32)):
                        ps = pst.tile([co_n, 512], F32, tag="ps")
                        for o, (dz, dy, dx) in enumerate(OFFSETS):
                            w = A3[:].rearrange("p z (a b) -> p z a b", b=10)
                            nc.tensor.matmul(
                                ps[:, 0:N].rearrange(
                                    "p (z a b) -> p z a b", z=nz, a=8),
                                w3c1_t[:, o, c0:c0 + co_n],
                                w[:, z0 + dz + 1:z0 + dz + 1 + nz,
                                  1 + dy:9 + dy, 1 + dx:9 + dx],
                                start=(o == 0), stop=(o == 26))
                        mt = sm.tile([co_n, 512], F32, tag="m3mul")
                        nc.sync.dma_start(
                            mt[:, 0:N],
                            m3mul_d[z0:z0 + nz].flatten().unsqueeze(0)
                            .to_broadcast((co_n, N)))
                        B3 = B3a if c0 == 0 else B3b
                        dst = B3[:].rearrange("p z (a b) -> p z a b", b=10)
                        nc.vector.tensor_mul(
                            dst[:, z0:z0 + nz, 1:9, 1:9],
                            ps[:, 0:N].rearrange(
                                "p (z a b) -> p z a b", z=nz, a=8),
                            mt[:, 0:N].rearrange(
                                "p (z a b) -> p z a b", z=nz, a=8))

                # conv2
                for (c0, co_n) in ((0, 128), (128, 32)):
                    ps = pst.tile([co_n, 512], F32, tag="ps")
                    for o, (dz, dy, dx) in enumerate(OFFSETS):
                        for ki, B3 in enumerate((B3a, B3b)):
                            w = B3[:].rearrange("p z (a b) -> p z a b", b=10)
                            nc.tensor.matmul(
                                ps[:].rearrange("p (z a b) -> p z a b",
                                                z=8, a=8),
                                w3c2_t[ki][:, o, c0:c0 + co_n],
                                w[:, dz + 1:dz + 9, 1 + dy:9 + dy,
                                  1 + dx:9 + dx],
                                start=(o == 0 and ki == 0),
                                stop=(o == 26 and ki == 1))
                    C3 = C3a if c0 == 0 else C3b
                    if mf[3]:
                        mt = sm.tile([co_n, 512], F32, tag="mn3")
                        nc.sync.dma_start(mt[:], mn3_d[:].flatten().unsqueeze(0)
                                          .to_broadcast((co_n, 512)))
                        nc.vector.tensor_add(C3[:], ps[:], mt[:])
                    else:
                        nc.scalar.copy(C3[:], ps[:])

                # pool -> P4
                for C3, P4, cn in ((C3a, P4a, 128), (C3b, P4b, 32)):
                    v = C3[:].rearrange("p (z v) -> p z v", v=64)
                    t1 = ss.tile([cn, 4, 64], F32R, tag="pool3a")
                    nc.vector.tensor_max(t1[:], v[:, 0::2, :], v[:, 1::2, :])
                    u = t1[:].rearrange("p z (a b) -> p z a b", b=8)
                    t2 = ss.tile([cn, 4, 4, 8], F32R, tag="pool3b")
                    nc.vector.tensor_max(t2[:], u[:, :, 0::2, :],
                                         u[:, :, 1::2, :])
                    dst = P4[:].rearrange("p (z a b) -> p z a b", z=6, a=6)
                    if mf[4]:
                        t3 = ss.tile([cn, 4, 4, 4], F32R, tag="pool3c")
                        nc.vector.tensor_max(t3[:], t2[:, :, :, 0::2],
                                             t2[:, :, :, 1::2])
                        mt = sm.tile([cn, 64], F32, tag="m4p")
                        nc.sync.dma_start(mt[:], m4p_d[:].flatten().unsqueeze(0)
                                          .to_broadcast((cn, 64)))
                        nc.vector.tensor_mul(
                            dst[:, 1:5, 1:5, 1:5], t3[:],
                            mt[:].rearrange("p (z a b) -> p z a b", z=4, a=4))
                    else:
                        nc.vector.tensor_max(dst[:, 1:5, 1:5, 1:5],
                                             t2[:, :, :, 0::2],
                                             t2[:, :, :, 1::2])

                if DBG:
                    nc.sync.dma_start(dbg_d["dA3"][:], A3[:].bitcast(F32))
                    nc.sync.dma_start(dbg_d["dB3a"][:], B3a[:].bitcast(F32))
                    nc.sync.dma_start(dbg_d["dC3a"][:], C3a[:].bitcast(F32))

            # ================ TAIL (levels 4-6, replicated) ================
            def tail_conv(sm, wts, ins, outs, pg, og, mode, mdram, mname):
                N = og * og * og
                noff = wts[0].shape[1]
                offs = OFFSETS if noff == 27 else [(0, 0, 0)]
                for (ot, c0, co_n, padded) in outs:
                    ps = pst.tile([co_n, max(N, 8)], F32, tag="ps")
                    nmm = len(offs) * len(ins)
                    i = 0
                    for o, (dz, dy, dx) in enumerate(offs):
                        for ki, it in enumerate(ins):
                            w = it[:].rearrange("p (z a b) -> p z a b",
                                                z=pg, a=pg)
                            nc.tensor.matmul(
                                ps[:, 0:N].rearrange(
                                    "p (z a b) -> p z a b", z=og, a=og),
                                wts[ki][:, o, c0:c0 + co_n],
                                w[:, 1 + dz:1 + dz + og, 1 + dy:1 + dy + og,
                                  1 + dx:1 + dx + og],
                                start=(i == 0), stop=(i == nmm - 1))
                            i += 1
                    if padded:
                        opg = og + 2
                        dst = ot[:].rearrange("p (z a b) -> p z a b",
                                              z=opg, a=opg)[:, 1:1 + og,
                                                            1:1 + og, 1:1 + og]
                    else:
                        dst = ot[:, 0:N].rearrange("p (z a b) -> p z a b",
                                                   z=og, a=og)
                    src = ps[:, 0:N].rearrange("p (z a b) -> p z a b",
                                               z=og, a=og)
                    if mode == "copy":
                        nc.scalar.copy(dst, src)
                    else:
                        mt = sm.tile([co_n, N], F32, tag=mname)
                        nc.sync.dma_start(
                            mt[:], mdram[:].flatten().unsqueeze(0)
                            .to_broadcast((co_n, N)))
                        mm = mt[:].rearrange("p (z a b) -> p z a b", z=og, a=og)
                        if mode == "mul":
                            nc.vector.tensor_mul(dst, src, mm)
                        else:
                            nc.vector.tensor_add(dst, src, mm)

            def tail_pool(sm, ss, cs, ps_out, g, has_mask, mdram):
                go = g // 2
                for (ct, cn), (pt, _) in zip(cs, ps_out):
                    v = ct[:, 0:g * g * g].rearrange("p (z v) -> p z v",
                                                     v=g * g)
                    t1 = ss.tile([cn, go, g * g], F32, tag=f"tp{g}a")
                    nc.vector.tensor_max(t1[:], v[:, 0::2, :], v[:, 1::2, :])
                    u = t1[:].rearrange("p z (a b) -> p z a b", b=g)
                    t2 = ss.tile([cn, go, go, g], F32, tag=f"tp{g}b")
                    nc.vector.tensor_max(t2[:], u[:, :, 0::2, :],
                                         u[:, :, 1::2, :])
                    gp = go + 2
                    dst = pt[:].rearrange("p (z a b) -> p z a b", z=gp, a=gp)
                    if has_mask:
                        t3 = ss.tile([cn, go, go, go], F32, tag=f"tp{g}c")
                        nc.vector.tensor_max(t3[:], t2[:, :, :, 0::2],
                                             t2[:, :, :, 1::2])
                        mt = sm.tile([cn, go * go * go], F32, tag=f"tp{g}m")
                        nc.sync.dma_start(
                            mt[:], mdram[:].flatten().unsqueeze(0)
                            .to_broadcast((cn, go * go * go)))
                        nc.vector.tensor_mul(
                            dst[:, 1:1 + go, 1:1 + go, 1:1 + go], t3[:],
                            mt[:].rearrange("p (z a b) -> p z a b",
                                            z=go, a=go))
                    else:
                        nc.vector.tensor_max(
                            dst[:, 1:1 + go, 1:1 + go, 1:1 + go],
                            t2[:, :, :, 0::2], t2[:, :, :, 1::2])

            # ---- L4 ----
            with tc.tile_pool(name="l4w", bufs=1) as wp, \
                 tc.tile_pool(name="l4p", bufs=1) as pp, \
                 tc.tile_pool(name="l4s", bufs=2) as ss, \
                 tc.tile_pool(name="l4m", bufs=2) as sm:
                w4c1_t = [wload(wp, d, dt=F32) for d in genw_d["w4c1"]]
                w4c2_t = [wload(wp, d, dt=F32) for d in genw_d["w4c2"]]
                B4a = pp.tile([128, 216], F32); B4b = pp.tile([64, 216], F32)
                C4a = pp.tile([128, 64], F32); C4b = pp.tile([64, 64], F32)
                nc.vector.memset(B4a[:].bitcast(F32), 0.0)
                nc.vector.memset(B4b[:].bitcast(F32), 0.0)
                tail_conv(sm, w4c1_t, [P4a, P4b],
                          [(B4a, 0, 128, True), (B4b, 128, 64, True)], 6, 4,
                          "mul" if mf[4] else "copy", m4mul_d, "m4mul")
                tail_conv(sm, w4c2_t, [B4a, B4b],
                          [(C4a, 0, 128, False), (C4b, 128, 64, False)], 6, 4,
                          "add" if mf[4] else "copy", mn4_d, "mn4")
                tail_pool(sm, ss, [(C4a, 128), (C4b, 64)],
                          [(P5a, 128), (P5b, 64)], 4, mf[5], m5p_d)

                if DBG:
                    nc.sync.dma_start(dbg_d["dP4a"][:], P4a[:])
                    nc.sync.dma_start(dbg_d["dB4a"][:], B4a[:])
                    nc.sync.dma_start(dbg_d["dC4a"][:], C4a[:])

            # ---- L5 ----
            with tc.tile_pool(name="l5w", bufs=1) as wp, \
                 tc.tile_pool(name="l5p", bufs=1) as pp, \
                 tc.tile_pool(name="l5s", bufs=2) as ss, \
                 tc.tile_pool(name="l5m", bufs=2) as sm:
                w5c1_t = [wload(wp, d, dt=F32) for d in genw_d["w5c1"]]
                w5c2_t = [wload(wp, d, dt=F32) for d in genw_d["w5c2"]]
                B5a = pp.tile([128, 64], F32); B5b = pp.tile([96, 64], F32)
                C5a = pp.tile([128, 8], F32); C5b = pp.tile([96, 8], F32)
                nc.vector.memset(B5a[:].bitcast(F32), 0.0)
                nc.vector.memset(B5b[:].bitcast(F32), 0.0)
                tail_conv(sm, w5c1_t, [P5a, P5b],
                          [(B5a, 0, 128, True), (B5b, 128, 96, True)], 4, 2,
                          "mul" if mf[5] else "copy", m5mul_d, "m5mul")
                tail_conv(sm, w5c2_t, [B5a, B5b],
                          [(C5a, 0, 128, False), (C5b, 128, 96, False)], 4, 2,
                          "add" if mf[5] else "copy", mn5_d, "mn5")
                tail_pool(sm, ss, [(C5a, 128), (C5b, 96)],
                          [(P6a, 128), (P6b, 96)], 2, mf[6], m6p_d)

                if DBG:
                    nc.sync.dma_start(dbg_d["dP5a"][:], P5a[:])
                    nc.sync.dma_start(dbg_d["dB5a"][:], B5a[:])
                    nc.sync.dma_start(dbg_d["dP6a"][:], P6a[:])

            # ---- L6 (1^3, center tap only) ----
            with tc.tile_pool(name="l6w", bufs=1) as wp:
                w6c1_t = [wload(wp, d, dt=F32) for d in genw_d["w6c1"]]
                w6c2_t = [wload(wp, d, dt=F32) for d in genw_d["w6c2"]]
                for (ot, c0) in ((X6a, 0), (X6b, 128)):
                    ps = pst.tile([128, 8], F32, tag="ps")
                    nc.tensor.matmul(ps[:, 0:1], w6c1_t[0][:, 0, c0:c0 + 128],
                                     P6a[:, 13:14], start=True, stop=False)
                    nc.tensor.matmul(ps[:, 0:1], w6c1_t[1][:, 0, c0:c0 + 128],
                                     P6b[:, 13:14], start=False, stop=True)
                    nc.vector.tensor_copy(ot[:], ps[:, 0:1])
                for i, c0 in enumerate((0, 128)):
                    ps = pst.tile([128, 8], F32, tag="ps")
                    nc.tensor.matmul(ps[:, 0:1], w6c2_t[0][:, 0, c0:c0 + 128],
                                     X6a[:], start=True, stop=False)
                    nc.tensor.matmul(ps[:, 0:1], w6c2_t[1][:, 0, c0:c0 + 128],
                                     X6b[:], start=False, stop=True)
                    nc.scalar.copy(outt[:, i:i + 1], ps[:, 0:1])
            if DBG:
                nc.sync.dma_start(dbg_d["dX6a"][:], X6a[:])
            nc.sync.dma_start(out_d[0, 0:128], outt[:, 0])
            nc.sync.dma_start(out_d[0, 128:256], outt[:, 1])

    nc.compile()
    return nc



_CACHE = {}
_RUNNERS = {}


def kernel(features, coors, W0, W1, W2, W3, W4, W5, W6, W7, W8, W9, W10, W11,
           W12, W13):
    features = np.asarray(features, np.float32)
    coors = np.asarray(coors, np.int32)
    Ws = [np.asarray(w, np.float32) for w in
          (W0, W1, W2, W3, W4, W5, W6, W7, W8, W9, W10, W11, W12, W13)]
    in_maps, meta = build_host_inputs(features, coors, Ws)
    key = tuple(sorted(meta["mask_flags"].items()))
    if key not in _CACHE:
        _CACHE[key] = build_kernel(meta)
    nc = _CACHE[key]
    try:
        if key not in _RUNNERS:
            _RUNNERS[key] = _Runner(nc, NC)
        r = _RUNNERS[key]
        r.place(in_maps)
        outs = r.run()
        out = r.result(outs, "out").reshape(256)
    except Exception:
        res = run_bass_kernel_spmd(nc, in_maps, core_ids=list(range(NC)))
        out = res.results[0]["out"].reshape(256)
    return out.reshape(1, 1, 1, 1, 256).astype(np.float32)


if __name__ == "__main__":
    pass



# revision 29
# speedup vs baseline: 137230.8162x; 4.2980x over previous
"""Trainium2 Bass kernel for the sparse submanifold 3D CNN (nn_Net_38963943309313).

Network: 7 blocks of 2 submanifold 3x3x3 convs on a 64^3 grid, 2x2x2 sparse
max-pools between blocks, channels 3->64->...->256, output [1,1,1,1,256].

Strategy (8 NeuronCores):
 - Shard z-slabs across cores for levels 0-2 (grids 64/32/16), AllGather the
   pooled activations between levels (z-padded gather buffers so per-core
   reads are a single dynamic-offset DMA). Levels 3-6 (grids 8/4/2/1) are
   replicated on every core.
 - Convs are fp32r matmuls: activations channel-major [C, z, y, x] in SBUF
   (y/x zero-padded), 27 shifted-window matmuls accumulated in PSUM.
 - conv1 of block 0 uses a host-side im2col (81 contract rows, masked
   columns so the submanifold mask is free).
 - 64-channel contractions (L0 conv2, L1 conv1) pack z-pairs into K=128 via
   duplicated storage; L0 conv2 additionally pairs two output slices into
   the two 64-column halves of the PE array.
 - Submanifold masking: conv1 evictions multiply by a broadcast mask (also
   zeroes the out-of-grid halo slices); conv2 evictions add (mask-1)*BIG so
   the following max-pool ignores inactive voxels; pool result is multiplied
   by the pooled mask.
"""

import sys

sys.path.insert(0, "/opt/trn_rl_repo")

import numpy as np
import concourse.bass as bass
import concourse.tile as tile
from concourse.tile import add_dep_helper
from concourse import bacc, mybir
from concourse.bass_utils import run_bass_kernel_spmd


class _Runner:
    """Compile-once, inputs-resident executor.

    Replicates concourse.bass2jax.run_bass_via_pjrt's lowering, but keeps the
    jitted shard_map executable and the device-placed input buffers alive
    across calls, so repeat executions cost only the PJRT dispatch + the NEFF
    execution itself (run_bass_kernel_spmd rebuilds the jit closure and
    re-uploads every input on each call).
    """

    def __init__(self, nc, n_cores, fully_written_outputs=True):
        import jax
        from jax.experimental.shard_map import shard_map
        from jax.sharding import Mesh, PartitionSpec
        from concourse import bass2jax

        bass2jax.install_neuronx_cc_hook()
        self.jax = jax
        self.bass2jax = bass2jax
        self.nc, self.n = nc, n_cores
        partition_name = (nc.partition_id_tensor.name
                          if nc.partition_id_tensor else None)
        in_names, out_names, out_avals = [], [], []
        for alloc in nc.m.functions[0].allocations:
            if not isinstance(alloc, mybir.MemoryLocationSet):
                continue
            name = alloc.memorylocations[0].name
            if alloc.kind == "ExternalInput":
                if name != partition_name:
                    in_names.append(name)
            elif alloc.kind == "ExternalOutput":
                out_names.append(name)
                out_avals.append(jax.core.ShapedArray(
                    tuple(alloc.tensor_shape), mybir.dt.np(alloc.dtype)))
        self.param_names = list(in_names)
        self.out_names, self.out_avals = out_names, out_avals
        self.dbg_name = nc.dbg_addr.name if nc.dbg_addr is not None else None
        n_params, n_outs = len(in_names), len(out_names)
        # When every output tensor is fully written by the kernel, skip the
        # donated zero-initialized output buffers run_bass_via_pjrt uses —
        # they cost a host->device transfer per call.
        self.pass_out_bufs = not fully_written_outputs
        if self.pass_out_bufs:
            full_names = (in_names + out_names
                          + ([partition_name] if partition_name else []))
        else:
            full_names = in_names + ([partition_name] if partition_name else [])

        def _body(*args):
            operands = list(args)
            if partition_name is not None:
                operands.append(bass2jax.partition_id_tensor())
            outs = bass2jax._bass_exec_p.bind(
                *operands,
                out_avals=tuple(out_avals),
                in_names=tuple(full_names),
                out_names=tuple(out_names),
                lowering_input_output_aliases=(),
                sim_require_finite=True,
                sim_require_nnan=True,
                nc=nc,
            )
            return tuple(outs)

        devices = jax.devices()[:n_cores]
        self.mesh = Mesh(np.asarray(devices), ("core",))
        n_args = n_params + (n_outs if self.pass_out_bufs else 0)
        self._shmapped = shard_map(
            _body, mesh=self.mesh,
            in_specs=(PartitionSpec("core"),) * n_args,
            out_specs=(PartitionSpec("core"),) * n_outs,
            check_rep=False)
        self._donate = (tuple(range(n_params, n_params + n_outs))
                        if self.pass_out_bufs else ())
        self._dev_in = None
        self._compiled = None

    def place(self, in_maps):
        from jax.sharding import NamedSharding, PartitionSpec
        if self.dbg_name is not None:
            in_maps = [{**m, self.dbg_name: np.zeros((1, 2), np.uint32)}
                       for m in in_maps]
        sh = NamedSharding(self.mesh, PartitionSpec("core"))
        concat = [np.concatenate([np.asarray(m[name]) for m in in_maps], 0)
                  for name in self.param_names]
        self._dev_in = [self.jax.device_put(a, sh) for a in concat]
        self.jax.block_until_ready(self._dev_in)
        if self._compiled is None:
            jax = self.jax
            example = list(self._dev_in) + self._fresh_out_bufs()
            try:
                self._compiled = self.bass2jax.fast_dispatch_compile(
                    lambda: jax.jit(self._shmapped, donate_argnums=self._donate,
                                    keep_unused=True).lower(*example).compile())
            except Exception:
                self._compiled = jax.jit(
                    self._shmapped, donate_argnums=self._donate,
                    keep_unused=True)

    def _fresh_out_bufs(self):
        if not self.pass_out_bufs:
            return []
        return [np.zeros((self.n * av.shape[0], *av.shape[1:]), av.dtype)
                for av in self.out_avals]

    def run(self):
        outs = self._compiled(*self._dev_in, *self._fresh_out_bufs())
        return self.jax.block_until_ready(outs)

    def run_async(self):
        return self._compiled(*self._dev_in, *self._fresh_out_bufs())

    def build_batched(self, k):
        """Compile a program that executes the NEFF k times sequentially
        (ordered bass_effect tokens keep the k custom calls distinct and
        serialized). One dispatch then covers k real network evaluations."""
        import jax
        from jax.experimental.shard_map import shard_map
        from jax.sharding import PartitionSpec

        body = self._shmapped.__wrapped__ if hasattr(self._shmapped, "__wrapped__") else None
        # rebuild from scratch: same _body called k times
        nc, n = self.nc, self.n
        bass2jax = self.bass2jax
        partition_name = (nc.partition_id_tensor.name
                          if nc.partition_id_tensor else None)
        in_names = list(self.param_names)
        out_names, out_avals = self.out_names, self.out_avals
        full_names = in_names + ([partition_name] if partition_name else [])

        def _body_k(*args):
            outs = None
            for _ in range(k):
                operands = list(args)
                if partition_name is not None:
                    operands.append(bass2jax.partition_id_tensor())
                outs = bass2jax._bass_exec_p.bind(
                    *operands,
                    out_avals=tuple(out_avals),
                    in_names=tuple(full_names),
                    out_names=tuple(out_names),
                    lowering_input_output_aliases=(),
                    sim_require_finite=True,
                    sim_require_nnan=True,
                    nc=nc,
                )
            return tuple(outs)

        shm = shard_map(_body_k, mesh=self.mesh,
                        in_specs=(PartitionSpec("core"),) * len(in_names),
                        out_specs=(PartitionSpec("core"),) * len(out_names),
                        check_rep=False)
        self._batched_k = k
        self._batched = jax.jit(shm, keep_unused=True).lower(
            *self._dev_in).compile()

    def run_batched(self):
        outs = self._batched(*self._dev_in)
        return self.jax.block_until_ready(outs)

    def result(self, outs, name, core=0):
        i = self.out_names.index(name)
        av = self.out_avals[i]
        return np.asarray(outs[i]).reshape(self.n, *av.shape)[core]

NC = 8
GRID = 64
BIG = 1.0e30
CHANNELS = [(3, 64), (64, 64), (64, 96), (96, 96), (96, 128), (128, 128),
            (128, 160), (160, 160), (160, 192), (192, 192), (192, 224),
            (224, 224), (224, 256), (256, 256)]
F32 = mybir.dt.float32
F32R = mybir.dt.float32r

OFFSETS = [(dz, dy, dx) for dz in (-1, 0, 1) for dy in (-1, 0, 1) for dx in (-1, 0, 1)]
# 9 (dy,dx) pairs for z-pair-packed layers
DYDX = [(dy, dx) for dy in (-1, 0, 1) for dx in (-1, 0, 1)]

GENW_SHAPES = {
    "w1c2": [(96, 27, 96)], "w2c1": [(96, 27, 128)], "w2c2": [(128, 27, 128)],
    "w3c1": [(128, 27, 160)], "w3c2": [(128, 27, 160), (32, 27, 160)],
    "w4c1": [(128, 27, 192), (32, 27, 192)],
    "w4c2": [(128, 27, 192), (64, 27, 192)],
    "w5c1": [(128, 27, 224), (64, 27, 224)],
    "w5c2": [(128, 27, 224), (96, 27, 224)],
    "w6c1": [(128, 1, 256), (96, 1, 256)],
    "w6c2": [(128, 1, 256), (128, 1, 256)],
}

# Every constant input is packed into one flat f32 blob per core (single
# NEFF input tensor -> minimal per-dispatch arg overhead).
MANIFEST = [
    ("x1col", (10, 81, 4096)),
    ("w1col", (81, 128)),
    ("wq1", (128, 9, 128)), ("wq2", (128, 9, 128)),
    ("w1p", (128, 9, 96)), ("w1l", (128, 9, 96)),
    ("mn0", (8, 4096)), ("m1p", (4, 1024)),
    ("m1mul", (6, 1024)), ("mn1", (4, 1024)), ("m2p", (2, 256)),
    ("m2mul", (4, 256)), ("mn2", (2, 256)), ("m3p", (1, 64)),
    ("m3mul", (10, 64)), ("mn3", (8, 64)), ("m4p", (4, 16)),
    ("m4mul", (4, 16)), ("mn4", (4, 16)), ("m5p", (2, 4)),
    ("m5mul", (2, 4)), ("mn5", (2, 4)), ("m6p", (1, 1)),
] + [(f"{n}_{i}", s) for n, shl in GENW_SHAPES.items() for i, s in enumerate(shl)]
BLOB_SIZE = sum(int(np.prod(s)) for _, s in MANIFEST)


def _pool_np(x, m):
    # x: [D,D,D,C] or [D,D,D]; max over active voxels of 2x2x2 windows
    D = x.shape[0]
    if x.ndim == 3:
        xr = x.reshape(D // 2, 2, D // 2, 2, D // 2, 2)
        return xr.max(axis=(1, 3, 5))
    neg = np.where(m[..., None] > 0, x, -np.inf)
    xr = neg.reshape(D // 2, 2, D // 2, 2, D // 2, 2, -1)
    p = xr.max(axis=(1, 3, 5))
    mp = m.reshape(D // 2, 2, D // 2, 2, D // 2, 2).max(axis=(1, 3, 5))
    return np.where(mp[..., None] > 0, p, 0.0), mp


def _ceil_div(a, b):
    return (a + b - 1) // b


def build_host_inputs(features, coors, Ws):
    """All host-side data marshalling. Returns (in_maps, meta)."""
    z, y, x = coors[:, 0], coors[:, 1], coors[:, 2]
    dense = np.zeros((GRID, GRID, GRID, 3), np.float32)
    mask0 = np.zeros((GRID, GRID, GRID), np.float32)
    dense[z, y, x] = features  # last write wins (matches XLA CPU scatter)
    mask0[z, y, x] = 1.0

    # mask pyramid
    masks = [mask0]
    m = mask0
    for _ in range(6):
        mr = m.reshape(m.shape[0] // 2, 2, m.shape[1] // 2, 2, m.shape[2] // 2, 2)
        m = mr.max(axis=(1, 3, 5))
        masks.append(m)

    # ---- X1col: host im2col for conv1 of block 0, column-masked ----
    # padded dense [3, 66, 66, 66]
    dpad = np.zeros((3, GRID + 2, GRID + 2, GRID + 2), np.float32)
    dpad[:, 1:-1, 1:-1, 1:-1] = dense.transpose(3, 0, 1, 2)
    # X1col_full[z, (dz,dy,dx,ci), y*64+x] = dpad[ci, z+dz+1, y+dy+1, x+dx+1]
    sw = np.lib.stride_tricks.sliding_window_view(dpad, (3, 3, 3),
                                                  axis=(1, 2, 3))
    # sw[ci, z, y, x, a, b, c] = dpad[ci, z+a, y+b, x+c]
    xfull = np.ascontiguousarray(sw.transpose(1, 4, 5, 6, 0, 2, 3)).reshape(
        GRID, 81, GRID * GRID)
    xfull *= mask0.reshape(GRID, 1, GRID * GRID)
    xpad = np.zeros((GRID + 10, 81, GRID * GRID), np.float32)
    xpad[1:GRID + 1] = xfull
    # core k conv1-out slices global [8k-1, 8k+9)
    x1cols = [xpad[8 * k:8 * k + 10] for k in range(NC)]

    # ---- weight packs ----
    # W0 for im2col conv1: [81, 128] (co=64 duplicated for col-pairing)
    W0 = Ws[0]  # [3,3,3,3,64]
    w1col = np.zeros((81, 128), np.float32)
    for o, (dz, dy, dx) in enumerate(OFFSETS):
        w1col[o * 3:(o + 1) * 3, 0:64] = W0[dz + 1, dy + 1, dx + 1]
        w1col[o * 3:(o + 1) * 3, 64:128] = W0[dz + 1, dy + 1, dx + 1]

    def pack_pair(W):  # [3,3,3,cin,co] -> pair [2*cin, 9, co] + left [cin, 9, co]
        cin, co = W.shape[3], W.shape[4]
        wp = np.zeros((2 * cin, 9, co), np.float32)
        wl = np.zeros((cin, 9, co), np.float32)
        for j, (dy, dx) in enumerate(DYDX):
            wp[0:cin, j] = W[0, dy + 1, dx + 1]      # dz=-1
            wp[cin:2 * cin, j] = W[1, dy + 1, dx + 1]  # dz=0
            wl[:, j] = W[2, dy + 1, dx + 1]          # dz=+1
        return wp, wl

    # L0 conv2 64->64: output-pair block packs. Rows = z-pair of inputs,
    # cols = (out z | out z+1). Two K=128,M=128 matmuls per tap cover all 6
    # z-tap contributions of an output pair:
    #   mm1 rows (h[z],h[z+1]):   [[W0, 0 ], [W1, W0]]
    #   mm2 rows (h[z+2],h[z+3]): [[W2, W1], [0,  W2]]
    WL0 = Ws[1]
    wq1 = np.zeros((128, 9, 128), np.float32)
    wq2 = np.zeros((128, 9, 128), np.float32)
    for j, (dy, dx) in enumerate(DYDX):
        W0t, W1t, W2t = (WL0[0, dy + 1, dx + 1], WL0[1, dy + 1, dx + 1],
                         WL0[2, dy + 1, dx + 1])
        wq1[0:64, j, 0:64] = W0t
        wq1[64:128, j, 0:64] = W1t
        wq1[64:128, j, 64:128] = W0t
        wq2[0:64, j, 0:64] = W2t
        wq2[0:64, j, 64:128] = W1t
        wq2[64:128, j, 64:128] = W2t

    w1p, w1l = pack_pair(Ws[2])   # L1 conv1 64->96
    w1l = np.concatenate([w1l, w1l], axis=0)  # [128, 9, 96]

    def pack_generic(W):  # -> list of [kchunk, 27, co] arrays
        cin, co = W.shape[3], W.shape[4]
        wf = W.reshape(27, cin, co)
        out = []
        for k0 in range(0, cin, 128):
            kc = min(128, cin - k0)
            out.append(np.ascontiguousarray(
                wf[:, k0:k0 + kc, :].transpose(1, 0, 2)))  # [kc, 27, co]
        return out

    gen_w = {}
    for li, wi in [("w1c2", 3), ("w2c1", 4), ("w2c2", 5), ("w3c1", 6),
                   ("w3c2", 7), ("w4c1", 8), ("w4c2", 9), ("w5c1", 10),
                   ("w5c2", 11)]:
        gen_w[li] = pack_generic(Ws[wi])
    # L6: center tap only (1^3 grid)
    for li, wi in [("w6c1", 12), ("w6c2", 13)]:
        W = Ws[wi]
        cin, co = W.shape[3], W.shape[4]
        wc = W[1, 1, 1]  # [cin, co]
        gen_w[li] = [np.ascontiguousarray(wc[k0:k0 + min(128, cin - k0)][:, None, :])
                     for k0 in range(0, cin, 128)]

    # ---- per-core mask arrays ----
    # L0 maskneg for conv2-evict: [8, 4096]
    mn0 = [((masks[0][8 * k:8 * k + 8] - 1.0) * BIG).reshape(8, -1).astype(np.float32)
           for k in range(NC)]
    # L0 pool-out multiply: m1 on core's L1 slices [4, 1024]
    m1p = [masks[1][4 * k:4 * k + 4].reshape(4, -1).astype(np.float32)
           for k in range(NC)]

    def slab_mask(mask, z0, nsl):
        D2 = mask.shape[1] * mask.shape[2]
        out = np.zeros((nsl, D2), np.float32)
        for i in range(nsl):
            zg = z0 + i
            if 0 <= zg < mask.shape[0]:
                out[i] = mask[zg].reshape(-1)
        return out

    # L1 conv1-evict multiply mask (m1 x ingrid): slices [4k-1, 4k+5)
    m1mul = [slab_mask(masks[1], 4 * k - 1, 6) for k in range(NC)]
    # L1 conv2-evict maskneg: slices [4k, 4k+4)
    mn1 = [((slab_mask(masks[1], 4 * k, 4) - 1.0) * BIG).astype(np.float32)
           for k in range(NC)]
    # L1 pool-out multiply: m2 on core's L2 slices [2, 256]
    m2p = [slab_mask(masks[2], 2 * k, 2) for k in range(NC)]
    # L2 conv1-evict multiply (m2 x ingrid): slices [2k-1, 2k+3)
    m2mul = [slab_mask(masks[2], 2 * k - 1, 4) for k in range(NC)]
    # L2 conv2-evict maskneg: slices [2k, 2k+2)
    mn2 = [((slab_mask(masks[2], 2 * k, 2) - 1.0) * BIG).astype(np.float32)
           for k in range(NC)]
    # L2 pool-out multiply: m3 on core's L3 slice [1, 64]
    m3p = [slab_mask(masks[3], k, 1) for k in range(NC)]
    # L3 (replicated): conv1-evict mul (m3 x ingrid) slices [-1, 9)
    m3mul_r = slab_mask(masks[3], -1, 10)
    mn3_r = ((slab_mask(masks[3], 0, 8) - 1.0) * BIG).astype(np.float32)
    m4p_r = slab_mask(masks[4], 0, 4)       # [4, 16]
    m4mul_r = slab_mask(masks[4], 0, 4)     # L4 out all valid (full grid)
    mn4_r = ((slab_mask(masks[4], 0, 4) - 1.0) * BIG).astype(np.float32)
    m5p_r = slab_mask(masks[5], 0, 2)
    m5mul_r = slab_mask(masks[5], 0, 2)
    mn5_r = ((slab_mask(masks[5], 0, 2) - 1.0) * BIG).astype(np.float32)
    m6p_r = slab_mask(masks[6], 0, 1)

    meta = {
        "mask_flags": {
            # whether the real mask (not just ingrid) has zeros at each level
            1: not np.all(masks[1] == 1.0),
            2: not np.all(masks[2] == 1.0),
            3: not np.all(masks[3] == 1.0),
            4: not np.all(masks[4] == 1.0),
            5: not np.all(masks[5] == 1.0),
            6: not np.all(masks[6] == 1.0),
        },
    }

    in_maps = []
    for k in range(NC):
        im = {
            "x1col": x1cols[k],
            "w1col": w1col,
            "wq1": wq1, "wq2": wq2, "w1p": w1p, "w1l": w1l,
            "mn0": mn0[k], "m1p": m1p[k],
            "m1mul": m1mul[k], "mn1": mn1[k], "m2p": m2p[k],
            "m2mul": m2mul[k], "mn2": mn2[k], "m3p": m3p[k],
            "m3mul": m3mul_r, "mn3": mn3_r, "m4p": m4p_r,
            "m4mul": m4mul_r, "mn4": mn4_r, "m5p": m5p_r,
            "m5mul": m5mul_r, "mn5": mn5_r, "m6p": m6p_r,
        }
        for name, chunks in gen_w.items():
            for ci, arr in enumerate(chunks):
                im[f"{name}_{ci}"] = arr
        parts = []
        for name, sh in MANIFEST:
            a = np.ascontiguousarray(im[name], np.float32)
            assert a.shape == sh, (name, a.shape, sh)
            parts.append(a.reshape(-1))
        in_maps.append({"blob": np.concatenate(parts)})
    return in_maps, meta


def build_kernel(meta):
    import contextlib
    nc = bacc.Bacc("TRN2", target_bir_lowering=False, debug=False, num_devices=NC)
    mf = meta["mask_flags"]

    # ---------- DRAM I/O declarations ----------
    # single flat input blob; every constant is an AP view into it
    blob_d = nc.dram_tensor("blob", [BLOB_SIZE], F32, kind="ExternalInput")
    views = {}
    off = 0
    for name, sh in MANIFEST:
        n = int(np.prod(sh))
        v = blob_d[off:off + n]
        if len(sh) == 2:
            v = v.rearrange("(a b) -> a b", a=sh[0], b=sh[1])
        elif len(sh) == 3:
            v = v.rearrange("(a b c) -> a b c", a=sh[0], b=sh[1], c=sh[2])
        views[name] = v
        off += n

    x1col = views["x1col"]
    w1col_d = views["w1col"]
    wq1_d = views["wq1"]; wq2_d = views["wq2"]
    w1p_d = views["w1p"]; w1l_d = views["w1l"]
    mn0_d = views["mn0"]; m1p_d = views["m1p"]
    m1mul_d = views["m1mul"]; mn1_d = views["mn1"]
    m2p_d = views["m2p"]
    m2mul_d = views["m2mul"]; mn2_d = views["mn2"]
    m3p_d = views["m3p"]
    m3mul_d = views["m3mul"]; mn3_d = views["mn3"]
    m4p_d = views["m4p"]; m4mul_d = views["m4mul"]
    mn4_d = views["mn4"]
    m5p_d = views["m5p"]; m5mul_d = views["m5mul"]
    mn5_d = views["mn5"]; m6p_d = views["m6p"]

    genw_d = {name: [views[f"{name}_{i}"] for i in range(len(shl))]
              for name, shl in GENW_SHAPES.items()}

    out_d = nc.dram_tensor("out", [1, 256], F32, kind="ExternalOutput")
    import os as _os
    DBG = bool(_os.environ.get("K_DEBUG"))
    dbg_d = {}
    if DBG:
        for nm, sh in [("dP0", (64, 4, 1156)), ("dA1", (128, 8, 1156)),
                       ("dB1", (96, 6, 1156)), ("dC1", (96, 4, 1024)),
                       ("dP1", (96, 2, 324)), ("dA2", (96, 6, 324)),
                       ("dB2", (128, 4, 324)), ("dC2", (128, 2, 256)),
                       ("dP2", (128, 1, 100)), ("dA3", (128, 12, 100)),
                       ("dB3a", (128, 10, 100)), ("dC3a", (128, 512)),
                       ("dP4a", (128, 216)), ("dB4a", (128, 216)),
                       ("dC4a", (128, 64)), ("dP5a", (128, 64)),
                       ("dB5a", (128, 64)), ("dP6a", (128, 27)),
                       ("dX6a", (128, 1)), ("dC0", (128, 4096))]:
            dbg_d[nm] = nc.dram_tensor(nm, list(sh), F32, kind="ExternalOutput")

    with tile.TileContext(nc) as tc:
        ctx = contextlib.ExitStack()
        with ctx:
            pst = ctx.enter_context(tc.tile_pool(name="ps", bufs=4, space="PSUM"))
            drm = ctx.enter_context(tc.tile_pool(name="dram", bufs=1, space="DRAM"))
            glob = ctx.enter_context(tc.tile_pool(name="glob", bufs=1))

            pid = nc.sync.partition_id()

            _wl_ctr = [0]

            def wload(pool, d, shape=None, name=None, dt=F32R):
                sh = shape or d.shape
                _wl_ctr[0] += 1
                t = pool.tile(list(sh), dt,
                              name=name or f"sb_w{_wl_ctr[0]}")
                nc.sync.dma_start(t[:], d[:].bitcast(dt) if dt is F32R else d[:])
                return t

            # zero tile for G-pad zeroing
            zt = glob.tile([128, 1156], F32)
            nc.vector.memset(zt[:], 0.0)

            # DRAM gather buffers. G1/G2 are [zp, core(+pad), c, v] so each
            # per-slice AllGather writes a contiguous [8, c, v] block; the
            # core axis is padded by one on each side for the z halo.
            c1_d = drm.tile([4, 64, 1156], F32)
            G1 = drm.tile([4, 10, 64, 1156], F32)
            c2_d = drm.tile([2, 96, 324], F32)
            G2 = drm.tile([2, 10, 96, 324], F32)
            c3_d = drm.tile([1, 128, 100], F32)
            G3 = drm.tile([12, 128, 100], F32)
            gpad_insts = []
            for dst, csz in ((G1[2, 0], (64, 1156)), (G1[3, 0], (64, 1156)),
                             (G1[0, 9], (64, 1156)), (G1[1, 9], (64, 1156)),
                             (G2[0, 0], (96, 324)), (G2[1, 0], (96, 324)),
                             (G2[0, 9], (96, 324)), (G2[1, 9], (96, 324))):
                gpad_insts.append(
                    nc.sync.dma_start(dst, zt[0:csz[0], 0:csz[1]]))
            for s in (0, 1, 10, 11):
                gpad_insts.append(
                    nc.sync.dma_start(G3[s], zt[0:128, 0:100]))

            # persistent tail tensors (small; cross level boundaries)
            P4a = glob.tile([128, 216], F32); P4b = glob.tile([32, 216], F32)
            P5a = glob.tile([128, 64], F32); P5b = glob.tile([64, 64], F32)
            P6a = glob.tile([128, 27], F32); P6b = glob.tile([96, 27], F32)
            X6a = glob.tile([128, 1], F32); X6b = glob.tile([128, 1], F32)
            outt = glob.tile([128, 2], F32)
            for t in (P4a, P4b, P5a, P5b, P6a, P6b):
                nc.vector.memset(t[:].bitcast(F32), 0.0)

            # ================ LEVEL 0 ================
            with tc.tile_pool(name="l0w", bufs=1) as wp, \
                 tc.tile_pool(name="l0p", bufs=1) as pp, \
                 tc.tile_pool(name="l0s", bufs=2) as ss, \
                 tc.tile_pool(name="l0m", bufs=4) as sm:
                w1col_t = wload(wp, w1col_d, name="w1col_t")
                wq1_t = wload(wp, wq1_d, name="wq1_t")
                wq2_t = wload(wp, wq2_d, name="wq2_t")

                A0 = pp.tile([128, 4, 4356], F32R)
                C0 = pp.tile([128, 4096], F32R)   # rows0=out z, rows64=out z+1
                P0 = pp.tile([64, 4, 1156], F32R)
                for _s in range(4):
                    nc.vector.memset(A0[:, _s, :].bitcast(F32), 0.0)
                nc.vector.memset(P0[:].bitcast(F32), 0.0)

                def l0_conv1(sl):
                    # psum M=128 (w1col cols are co duplicated); evict the two
                    # h1 copies from the matching psum partition halves so no
                    # cross-partition copy is needed.
                    xs = ss.tile([81, 4096], F32R, tag="x1s")
                    nc.sync.dma_start(xs[:], x1col[sl].bitcast(F32R))
                    for chunk in range(8):
                        ps = pst.tile([128, 512], F32, tag="ps")
                        nc.tensor.matmul(ps[:], w1col_t[:, :],
                                         xs[:, chunk * 512:chunk * 512 + 512],
                                         start=True, stop=True)
                        r0, r1 = sl % 4, (sl - 1) % 4
                        yb = chunk * 8
                        src = ps[:].rearrange("p (a b) -> p a b", b=64)
                        d0 = A0[0:64, r0, :].rearrange("p (a b) -> p a b", b=66)
                        d1 = A0[64:128, r1, :].rearrange("p (a b) -> p a b", b=66)
                        nc.scalar.copy(d0[:, yb + 1:yb + 9, 1:65], src[0:64])
                        nc.vector.tensor_copy(d1[:, yb + 1:yb + 9, 1:65],
                                              src[64:128])

                def l0_conv2(z):
                    # output pair (z, z+1) in one M=128 psum: two K=128 block
                    # matmuls per tap. Ring slot r: rows0=h1[r], rows64=h1[r+1].
                    s1 = z % 4         # pair (h1[z],   h1[z+1])
                    s2 = (z + 2) % 4   # pair (h1[z+2], h1[z+3])
                    w1v = A0[:, s1, :].rearrange("p (a b) -> p a b", b=66)
                    w2v = A0[:, s2, :].rearrange("p (a b) -> p a b", b=66)
                    for chunk in range(8):
                        yb = chunk * 8
                        ps = pst.tile([128, 512], F32, tag="ps")
                        v = ps[:].rearrange("p (a b) -> p a b", b=64)
                        for j, (dy, dx) in enumerate(DYDX):
                            ys = slice(yb + 1 + dy, yb + 9 + dy)
                            xsl = slice(1 + dx, 65 + dx)
                            nc.tensor.matmul(v, wq1_t[:, j, :],
                                             w1v[:, ys, xsl],
                                             start=(j == 0), stop=False)
                            nc.tensor.matmul(v, wq2_t[:, j, :],
                                             w2v[:, ys, xsl],
                                             start=False, stop=(j == 8))
                        mt = sm.tile([128, 512], F32, tag="mn0")
                        nc.sync.dma_start(
                            mt[0:64, :], mn0_d[z, yb * 64:yb * 64 + 512]
                            .unsqueeze(0).to_broadcast((64, 512)))
                        nc.sync.dma_start(
                            mt[64:128, :], mn0_d[z + 1, yb * 64:yb * 64 + 512]
                            .unsqueeze(0).to_broadcast((64, 512)))
                        nc.vector.tensor_add(
                            C0[:, yb * 64:yb * 64 + 512], ps[:], mt[:])

                ag1s = []

                def l0_pool(z):
                    zp = z // 2
                    # bring the odd output slice down to partitions 0:64
                    E0 = ss.tile([64, 4096], F32R, tag="pool0e", bufs=1)
                    nc.sync.dma_start(E0[:], C0[64:128, :])
                    nc.vector.tensor_max(E0[:], C0[0:64, :], E0[:])
                    v = E0[:].rearrange("p (a b) -> p a b", b=64)
                    t2 = ss.tile([64, 32, 64], F32R, tag="pool0b", bufs=1)
                    nc.vector.tensor_max(t2[:], v[:, 0::2, :], v[:, 1::2, :])
                    t3 = ss.tile([64, 32, 32], F32R, tag="pool0c", bufs=1)
                    nc.vector.tensor_max(t3[:], t2[:, :, 0::2], t2[:, :, 1::2])
                    mt = sm.tile([64, 1024], F32, tag="m1p", bufs=2)
                    nc.sync.dma_start(mt[:], m1p_d[zp].unsqueeze(0)
                                      .to_broadcast((64, 1024)))
                    dst = P0[:, zp, :].rearrange("p (a b) -> p a b", b=34)
                    nc.vector.tensor_mul(
                        dst[:, 1:33, 1:33], t3[:],
                        mt[:].rearrange("p (a b) -> p a b", b=32))
                    # gather this slice now so the collective overlaps the
                    # remaining L0 compute; core k's slice zp lands at
                    # G1 row 2 + 4k + zp.
                    nc.sync.dma_start(c1_d[zp], P0[:, zp, :].bitcast(F32))
                    ag = nc.gpsimd.collective_compute(
                        "AllGather", mybir.AluOpType.bypass,
                        replica_groups=[list(range(NC))],
                        ins=[c1_d[zp].opt()],
                        outs=[G1[zp, 1:9].opt()])
                    for gi in gpad_insts:
                        add_dep_helper(ag.ins, gi.ins,
                                       reason="G pads zeroed before gathers")
                    ag1s.append(ag)

                for sl in range(10):
                    l0_conv1(sl)
                    if sl >= 3 and (sl - 3) % 2 == 0:
                        zz = sl - 3
                        l0_conv2(zz)
                        l0_pool(zz)

                if DBG:
                    nc.sync.dma_start(dbg_d["dP0"][:], P0[:].bitcast(F32))
                    nc.sync.dma_start(dbg_d["dC0"][:], C0[:].bitcast(F32))

            # ================ LEVEL 1 ================
            with tc.tile_pool(name="l1w", bufs=1) as wp, \
                 tc.tile_pool(name="l1p", bufs=1) as pp, \
                 tc.tile_pool(name="l1s", bufs=2) as ss, \
                 tc.tile_pool(name="l1m", bufs=4) as sm:
                w1p_t = wload(wp, w1p_d)
                w1l_t = wload(wp, w1l_d)
                w1c2_t = wload(wp, genw_d["w1c2"][0])

                A1 = pp.tile([128, 8, 1156], F32R)
                B1 = pp.tile([96, 6, 1156], F32R)
                C1 = pp.tile([96, 4, 1024], F32R)
                P1 = pp.tile([96, 2, 324], F32R)
                nc.vector.memset(B1[:].bitcast(F32), 0.0)
                nc.vector.memset(P1[:].bitcast(F32), 0.0)
                # halo reads from the [zp, core] gather layout: rows0 i holds
                # x1[4k-2+i] (i=0..7), rows64 i holds x1[4k-1+i] (i=0..6);
                # global z maps to G1[z % 4, z // 4 + 1].
                def g1rd(dst, zp0, zp1, coreoff):
                    src = G1[zp0:zp1, bass.ds(pid + coreoff, 1)].squeeze(1)
                    return nc.sync.dma_start(
                        dst, src.rearrange("z c v -> c z v").bitcast(F32R))

                _rds = [
                    g1rd(A1[0:64, 0:2, :], 2, 4, 0),
                    g1rd(A1[0:64, 2:6, :], 0, 4, 1),
                    g1rd(A1[0:64, 6:8, :], 0, 2, 2),
                    g1rd(A1[64:128, 0:1, :], 3, 4, 0),
                    g1rd(A1[64:128, 1:5, :], 0, 4, 1),
                    g1rd(A1[64:128, 5:7, :], 0, 2, 2),
                ]
                for ag in ag1s:
                    for _r in _rds:
                        add_dep_helper(_r.ins, ag.ins,
                                       reason="gather before dynamic read")

                def l1_conv1(sl):
                    # A1 rows0 idx i = x1[4k-2+i]; rows64 idx i = x1[4k-1+i].
                    # out sl (global 4k-1+sl): pair = A1[:, sl] (dz=-1,0);
                    # leftover dz=+1 = rows64 idx sl+1 == rows0 idx sl+2.
                    mt = sm.tile([96, 1024], F32, tag="m1mul")
                    nc.sync.dma_start(mt[:], m1mul_d[sl].unsqueeze(0)
                                      .to_broadcast((96, 1024)))
                    pss = [pst.tile([96, 512], F32, tag="ps", name=f"ps_l1_{sl}_{_c}") for _c in range(2)]
                    wA = A1[:, sl, :].rearrange("p (a b) -> p a b", b=34)
                    wB = A1[64:128, sl + 1, :].rearrange("p (a b) -> p a b", b=34)
                    wC = A1[0:64, sl + 2, :].rearrange("p (a b) -> p a b", b=34)
                    for j, (dy, dx) in enumerate(DYDX):
                        xsl = slice(1 + dx, 33 + dx)
                        for chunk in range(2):
                            yb = chunk * 16
                            ys = slice(yb + 1 + dy, yb + 17 + dy)
                            nc.tensor.matmul(
                                pss[chunk][:].rearrange("p (a b) -> p a b", b=32),
                                w1p_t[:, j, :], wA[:, ys, xsl],
                                start=(j == 0), stop=False)
                        # row-paired leftovers: chunk0 on rows 64:128,
                        # chunk1 on rows 0:64 (concurrent row groups)
                        ys0 = slice(1 + dy, 17 + dy)
                        ys1 = slice(17 + dy, 33 + dy)
                        nc.tensor.matmul(
                            pss[0][:].rearrange("p (a b) -> p a b", b=32),
                            w1l_t[64:128, j, :], wB[:, ys0, xsl],
                            start=False, stop=(j == 8))
                        nc.tensor.matmul(
                            pss[1][:].rearrange("p (a b) -> p a b", b=32),
                            w1l_t[0:64, j, :], wC[:, ys1, xsl],
                            start=False, stop=(j == 8))
                    for chunk in range(2):
                        yb = chunk * 16
                        dst = B1[:, sl, :].rearrange("p (a b) -> p a b", b=34)
                        nc.vector.tensor_mul(
                            dst[:, yb + 1:yb + 17, 1:33],
                            pss[chunk][:].rearrange("p (a b) -> p a b", b=32),
                            mt[:, yb * 32:yb * 32 + 512].rearrange(
                                "p (a b) -> p a b", b=32))

                def l1_conv2(sl):
                    mt = sm.tile([96, 1024], F32, tag="mn1")
                    nc.sync.dma_start(mt[:], mn1_d[sl].unsqueeze(0)
                                      .to_broadcast((96, 1024)))
                    for chunk in range(2):
                        yb = chunk * 16
                        ps = pst.tile([96, 512], F32, tag="ps")
                        for o, (dz, dy, dx) in enumerate(OFFSETS):
                            w = B1[:, sl + 1 + dz, :].rearrange(
                                "p (a b) -> p a b", b=34)
                            nc.tensor.matmul(
                                ps[:].rearrange("p (a b) -> p a b", b=32),
                                w1c2_t[:, o, :],
                                w[:, yb + 1 + dy:yb + 17 + dy, 1 + dx:33 + dx],
                                start=(o == 0), stop=(o == 26))
                        nc.vector.tensor_add(C1[:, sl, yb * 32:yb * 32 + 512],
                                             ps[:],
                                             mt[:, yb * 32:yb * 32 + 512])

                ag2s = []

                def l1_pool(zz):
                    zp = zz // 2
                    nc.vector.tensor_max(C1[:, zz, :], C1[:, zz, :], C1[:, zz + 1, :])
                    v = C1[:, zz, :].rearrange("p (a b) -> p a b", b=32)
                    t2 = ss.tile([96, 16, 32], F32R, tag="pool1b")
                    nc.vector.tensor_max(t2[:], v[:, 0::2, :], v[:, 1::2, :])
                    t3 = ss.tile([96, 16, 16], F32R, tag="pool1c")
                    nc.vector.tensor_max(t3[:], t2[:, :, 0::2], t2[:, :, 1::2])
                    mt = sm.tile([96, 256], F32, tag="m2p")
                    nc.sync.dma_start(mt[:], m2p_d[zp].unsqueeze(0)
                                      .to_broadcast((96, 256)))
                    dst = P1[:, zp, :].rearrange("p (a b) -> p a b", b=18)
                    nc.vector.tensor_mul(
                        dst[:, 1:17, 1:17], t3[:],
                        mt[:].rearrange("p (a b) -> p a b", b=16))
                    nc.sync.dma_start(c2_d[zp], P1[:, zp, :].bitcast(F32))
                    ag = nc.gpsimd.collective_compute(
                        "AllGather", mybir.AluOpType.bypass,
                        replica_groups=[list(range(NC))],
                        ins=[c2_d[zp].opt()],
                        outs=[G2[zp, 1:9].opt()])
                    for gi in gpad_insts:
                        add_dep_helper(ag.ins, gi.ins,
                                       reason="G pads zeroed before gathers")
                    ag2s.append(ag)

                for sl in range(6):
                    l1_conv1(sl)
                    if sl >= 2:
                        l1_conv2(sl - 2)
                        if sl >= 3 and (sl - 3) % 2 == 0:
                            l1_pool(sl - 3)

                if DBG:
                    nc.sync.dma_start(dbg_d["dA1"][:], A1[:].bitcast(F32))
                    nc.sync.dma_start(dbg_d["dB1"][:], B1[:].bitcast(F32))
                    nc.sync.dma_start(dbg_d["dC1"][:], C1[:].bitcast(F32))
                    nc.sync.dma_start(dbg_d["dP1"][:], P1[:].bitcast(F32))

            # ================ LEVEL 2 ================
            with tc.tile_pool(name="l2w", bufs=1) as wp, \
                 tc.tile_pool(name="l2p", bufs=1) as pp, \
                 tc.tile_pool(name="l2s", bufs=2) as ss, \
                 tc.tile_pool(name="l2m", bufs=4) as sm:
                w2c1_t = wload(wp, genw_d["w2c1"][0])
                w2c2_t = wload(wp, genw_d["w2c2"][0])
                A2 = pp.tile([96, 6, 324], F32R)
                B2 = pp.tile([128, 4, 324], F32R)
                C2 = pp.tile([128, 2, 256], F32R)
                P2 = pp.tile([128, 1, 100], F32R)
                nc.vector.memset(B2[:].bitcast(F32), 0.0)
                nc.vector.memset(P2[:].bitcast(F32), 0.0)
                # A2 slices i=0..5 hold x2[2k-2+i]; z -> G2[z % 2, z // 2 + 1]
                _rds2 = []
                for i, coreoff in enumerate((0, 1, 2)):
                    src = G2[0:2, bass.ds(pid + coreoff, 1)].squeeze(1)
                    _rds2.append(nc.sync.dma_start(
                        A2[:, 2 * i:2 * i + 2, :],
                        src.rearrange("z c v -> c z v").bitcast(F32R)))
                for ag in ag2s:
                    for _r in _rds2:
                        add_dep_helper(_r.ins, ag.ins,
                                       reason="gather before dynamic read")

                for s0 in (0, 2):
                    ps = pst.tile([128, 512], F32, tag="ps")
                    for o, (dz, dy, dx) in enumerate(OFFSETS):
                        w = A2[:].rearrange("p z (a b) -> p z a b", b=18)
                        nc.tensor.matmul(
                            ps[:].rearrange("p (z a b) -> p z a b", z=2, a=16),
                            w2c1_t[:, o, :],
                            w[:, s0 + dz + 1:s0 + dz + 3,
                              1 + dy:17 + dy, 1 + dx:17 + dx],
                            start=(o == 0), stop=(o == 26))
                    mt = sm.tile([128, 512], F32, tag="m2mul")
                    nc.sync.dma_start(
                        mt[:], m2mul_d[s0:s0 + 2].flatten().unsqueeze(0)
                        .to_broadcast((128, 512)))
                    dst = B2[:].rearrange("p z (a b) -> p z a b", b=18)
                    nc.vector.tensor_mul(
                        dst[:, s0:s0 + 2, 1:17, 1:17],
                        ps[:].rearrange("p (z a b) -> p z a b", z=2, a=16),
                        mt[:].rearrange("p (z a b) -> p z a b", z=2, a=16))

                ps = pst.tile([128, 512], F32, tag="ps")
                for o, (dz, dy, dx) in enumerate(OFFSETS):
                    w = B2[:].rearrange("p z (a b) -> p z a b", b=18)
                    nc.tensor.matmul(
                        ps[:].rearrange("p (z a b) -> p z a b", z=2, a=16),
                        w2c2_t[:, o, :],
                        w[:, dz + 1:dz + 3, 1 + dy:17 + dy, 1 + dx:17 + dx],
                        start=(o == 0), stop=(o == 26))
                if mf[2]:
                    mt = sm.tile([128, 512], F32, tag="mn2")
                    nc.sync.dma_start(mt[:], mn2_d[:].flatten().unsqueeze(0)
                                      .to_broadcast((128, 512)))
                    nc.vector.tensor_add(C2[:].rearrange("p a b -> p (a b)"),
                                         ps[:], mt[:])
                else:
                    nc.scalar.copy(C2[:].rearrange("p a b -> p (a b)"), ps[:])

                # L2 pool
                nc.vector.tensor_max(C2[:, 0, :], C2[:, 0, :], C2[:, 1, :])
                v = C2[:, 0, :].rearrange("p (a b) -> p a b", b=16)
                t2 = ss.tile([128, 8, 16], F32R, tag="pool2b")
                nc.vector.tensor_max(t2[:], v[:, 0::2, :], v[:, 1::2, :])
                dst = P2[:, 0, :].rearrange("p (a b) -> p a b", b=10)
                if mf[3]:
                    t3 = ss.tile([128, 8, 8], F32R, tag="pool2c")
                    nc.vector.tensor_max(t3[:], t2[:, :, 0::2], t2[:, :, 1::2])
                    mt = sm.tile([128, 64], F32, tag="m3p")
                    nc.sync.dma_start(mt[:], m3p_d[0].unsqueeze(0)
                                      .to_broadcast((128, 64)))
                    nc.vector.tensor_mul(
                        dst[:, 1:9, 1:9], t3[:],
                        mt[:].rearrange("p (a b) -> p a b", b=8))
                else:
                    nc.vector.tensor_max(dst[:, 1:9, 1:9],
                                         t2[:, :, 0::2], t2[:, :, 1::2])

                if DBG:
                    nc.sync.dma_start(dbg_d["dA2"][:], A2[:].bitcast(F32))
                    nc.sync.dma_start(dbg_d["dB2"][:], B2[:].bitcast(F32))
                    nc.sync.dma_start(dbg_d["dC2"][:], C2[:].bitcast(F32))
                    nc.sync.dma_start(dbg_d["dP2"][:], P2[:].bitcast(F32))
                nc.sync.dma_start(c3_d[:].rearrange("z c v -> c z v"),
                                  P2[:].bitcast(F32))

            # ---- AllGather L2 -> L3 ----
            ag3 = nc.gpsimd.collective_compute(
                "AllGather", mybir.AluOpType.bypass,
                replica_groups=[list(range(NC))],
                ins=[c3_d[:].opt()], outs=[G3[2:10].opt()])
            for gi in gpad_insts:
                add_dep_helper(ag3.ins, gi.ins, reason="G pads zeroed before gathers")

            # ================ LEVEL 3 (replicated) ================
            with tc.tile_pool(name="l3w", bufs=1) as wp, \
                 tc.tile_pool(name="l3p", bufs=1) as pp, \
                 tc.tile_pool(name="l3s", bufs=2) as ss, \
                 tc.tile_pool(name="l3m", bufs=4) as sm:
                w3c1_t = wload(wp, genw_d["w3c1"][0])
                w3c2_t = [wload(wp, d) for d in genw_d["w3c2"]]
                A3 = pp.tile([128, 12, 100], F32R)
                B3a = pp.tile([128, 10, 100], F32R)
                B3b = pp.tile([32, 10, 100], F32R)
                C3a = pp.tile([128, 512], F32R)
                C3b = pp.tile([32, 512], F32R)
                nc.vector.memset(B3a[:].bitcast(F32), 0.0)
                nc.vector.memset(B3b[:].bitcast(F32), 0.0)
                _r4 = nc.sync.dma_start(A3[:], G3[:].rearrange("z c v -> c z v").bitcast(F32R))
                add_dep_helper(_r4.ins, ag3.ins, reason="gather before read")

                # conv1
                for (z0, nz) in ((0, 8), (2, 8)):
                    N = nz * 64
                    for (c0, co_n) in ((0, 128), (128, 32)):
                        ps = pst.tile([co_n, 512], F32, tag="ps")
                        for o, (dz, dy, dx) in enumerate(OFFSETS):
                            w = A3[:].rearrange("p z (a b) -> p z a b", b=10)
                            nc.tensor.matmul(
                                ps[:, 0:N].rearrange(
                                    "p (z a b) -> p z a b", z=nz, a=8),
                                w3c1_t[:, o, c0:c0 + co_n],
                                w[:, z0 + dz + 1:z0 + dz + 1 + nz,
                                  1 + dy:9 + dy, 1 + dx:9 + dx],
                                start=(o == 0), stop=(o == 26))
                        mt = sm.tile([co_n, 512], F32, tag="m3mul")
                        nc.sync.dma_start(
                            mt[:, 0:N],
                            m3mul_d[z0:z0 + nz].flatten().unsqueeze(0)
                            .to_broadcast((co_n, N)))
                        B3 = B3a if c0 == 0 else B3b
                        dst = B3[:].rearrange("p z (a b) -> p z a b", b=10)
                        nc.vector.tensor_mul(
                            dst[:, z0:z0 + nz, 1:9, 1:9],
                            ps[:, 0:N].rearrange(
                                "p (z a b) -> p z a b", z=nz, a=8),
                            mt[:, 0:N].rearrange(
                                "p (z a b) -> p z a b", z=nz, a=8))

                # conv2
                for (c0, co_n) in ((0, 128), (128, 32)):
                    ps = pst.tile([co_n, 512], F32, tag="ps")
                    for o, (dz, dy, dx) in enumerate(OFFSETS):
                        for ki, B3 in enumerate((B3a, B3b)):
                            w = B3[:].rearrange("p z (a b) -> p z a b", b=10)
                            nc.tensor.matmul(
                                ps[:].rearrange("p (z a b) -> p z a b",
                                                z=8, a=8),
                                w3c2_t[ki][:, o, c0:c0 + co_n],
                                w[:, dz + 1:dz + 9, 1 + dy:9 + dy,
                                  1 + dx:9 + dx],
                                start=(o == 0 and ki == 0),
                                stop=(o == 26 and ki == 1))
                    C3 = C3a if c0 == 0 else C3b
                    if mf[3]:
                        mt = sm.tile([co_n, 512], F32, tag="mn3")
                        nc.sync.dma_start(mt[:], mn3_d[:].flatten().unsqueeze(0)
                                          .to_broadcast((co_n, 512)))
                        nc.vector.tensor_add(C3[:], ps[:], mt[:])
                    else:
                        nc.scalar.copy(C3[:], ps[:])

                # pool -> P4
                for C3, P4, cn in ((C3a, P4a, 128), (C3b, P4b, 32)):
                    v = C3[:].rearrange("p (z v) -> p z v", v=64)
                    t1 = ss.tile([cn, 4, 64], F32R, tag="pool3a")
                    nc.vector.tensor_max(t1[:], v[:, 0::2, :], v[:, 1::2, :])
                    u = t1[:].rearrange("p z (a b) -> p z a b", b=8)
                    t2 = ss.tile([cn, 4, 4, 8], F32R, tag="pool3b")
                    nc.vector.tensor_max(t2[:], u[:, :, 0::2, :],
                                         u[:, :, 1::2, :])
                    dst = P4[:].rearrange("p (z a b) -> p z a b", z=6, a=6)
                    if mf[4]:
                        t3 = ss.tile([cn, 4, 4, 4], F32R, tag="pool3c")
                        nc.vector.tensor_max(t3[:], t2[:, :, :, 0::2],
                                             t2[:, :, :, 1::2])
                        mt = sm.tile([cn, 64], F32, tag="m4p")
                        nc.sync.dma_start(mt[:], m4p_d[:].flatten().unsqueeze(0)
                                          .to_broadcast((cn, 64)))
                        nc.vector.tensor_mul(
                            dst[:, 1:5, 1:5, 1:5], t3[:],
                            mt[:].rearrange("p (z a b) -> p z a b", z=4, a=4))
                    else:
                        nc.vector.tensor_max(dst[:, 1:5, 1:5, 1:5],
                                             t2[:, :, :, 0::2],
                                             t2[:, :, :, 1::2])

                if DBG:
                    nc.sync.dma_start(dbg_d["dA3"][:], A3[:].bitcast(F32))
                    nc.sync.dma_start(dbg_d["dB3a"][:], B3a[:].bitcast(F32))
                    nc.sync.dma_start(dbg_d["dC3a"][:], C3a[:].bitcast(F32))

            # ================ TAIL (levels 4-6, replicated) ================
            def tail_conv(sm, wts, ins, outs, pg, og, mode, mdram, mname):
                N = og * og * og
                noff = wts[0].shape[1]
                offs = OFFSETS if noff == 27 else [(0, 0, 0)]
                for (ot, c0, co_n, padded) in outs:
                    ps = pst.tile([co_n, max(N, 8)], F32, tag="ps")
                    nmm = len(offs) * len(ins)
                    i = 0
                    for o, (dz, dy, dx) in enumerate(offs):
                        for ki, it in enumerate(ins):
                            w = it[:].rearrange("p (z a b) -> p z a b",
                                                z=pg, a=pg)
                            nc.tensor.matmul(
                                ps[:, 0:N].rearrange(
                                    "p (z a b) -> p z a b", z=og, a=og),
                                wts[ki][:, o, c0:c0 + co_n],
                                w[:, 1 + dz:1 + dz + og, 1 + dy:1 + dy + og,
                                  1 + dx:1 + dx + og],
                                start=(i == 0), stop=(i == nmm - 1))
                            i += 1
                    if padded:
                        opg = og + 2
                        dst = ot[:].rearrange("p (z a b) -> p z a b",
                                              z=opg, a=opg)[:, 1:1 + og,
                                                            1:1 + og, 1:1 + og]
                    else:
                        dst = ot[:, 0:N].rearrange("p (z a b) -> p z a b",
                                                   z=og, a=og)
                    src = ps[:, 0:N].rearrange("p (z a b) -> p z a b",
                                               z=og, a=og)
                    if mode == "copy":
                        nc.scalar.copy(dst, src)
                    else:
                        mt = sm.tile([co_n, N], F32, tag=mname)
                        nc.sync.dma_start(
                            mt[:], mdram[:].flatten().unsqueeze(0)
                            .to_broadcast((co_n, N)))
                        mm = mt[:].rearrange("p (z a b) -> p z a b", z=og, a=og)
                        if mode == "mul":
                            nc.vector.tensor_mul(dst, src, mm)
                        else:
                            nc.vector.tensor_add(dst, src, mm)

            def tail_pool(sm, ss, cs, ps_out, g, has_mask, mdram):
                go = g // 2
                for (ct, cn), (pt, _) in zip(cs, ps_out):
                    v = ct[:, 0:g * g * g].rearrange("p (z v) -> p z v",
                                                     v=g * g)
                    t1 = ss.tile([cn, go, g * g], F32, tag=f"tp{g}a")
                    nc.vector.tensor_max(t1[:], v[:, 0::2, :], v[:, 1::2, :])
                    u = t1[:].rearrange("p z (a b) -> p z a b", b=g)
                    t2 = ss.tile([cn, go, go, g], F32, tag=f"tp{g}b")
                    nc.vector.tensor_max(t2[:], u[:, :, 0::2, :],
                                         u[:, :, 1::2, :])
                    gp = go + 2
                    dst = pt[:].rearrange("p (z a b) -> p z a b", z=gp, a=gp)
                    if has_mask:
                        t3 = ss.tile([cn, go, go, go], F32, tag=f"tp{g}c")
                        nc.vector.tensor_max(t3[:], t2[:, :, :, 0::2],
                                             t2[:, :, :, 1::2])
                        mt = sm.tile([cn, go * go * go], F32, tag=f"tp{g}m")
                        nc.sync.dma_start(
                            mt[:], mdram[:].flatten().unsqueeze(0)
                            .to_broadcast((cn, go * go * go)))
                        nc.vector.tensor_mul(
                            dst[:, 1:1 + go, 1:1 + go, 1:1 + go], t3[:],
                            mt[:].rearrange("p (z a b) -> p z a b",
                                            z=go, a=go))
                    else:
                        nc.vector.tensor_max(
                            dst[:, 1:1 + go, 1:1 + go, 1:1 + go],
                            t2[:, :, :, 0::2], t2[:, :, :, 1::2])

            # ---- L4 ----
            with tc.tile_pool(name="l4w", bufs=1) as wp, \
                 tc.tile_pool(name="l4p", bufs=1) as pp, \
                 tc.tile_pool(name="l4s", bufs=2) as ss, \
                 tc.tile_pool(name="l4m", bufs=2) as sm:
                w4c1_t = [wload(wp, d, dt=F32) for d in genw_d["w4c1"]]
                w4c2_t = [wload(wp, d, dt=F32) for d in genw_d["w4c2"]]
                B4a = pp.tile([128, 216], F32); B4b = pp.tile([64, 216], F32)
                C4a = pp.tile([128, 64], F32); C4b = pp.tile([64, 64], F32)
                nc.vector.memset(B4a[:].bitcast(F32), 0.0)
                nc.vector.memset(B4b[:].bitcast(F32), 0.0)
                tail_conv(sm, w4c1_t, [P4a, P4b],
                          [(B4a, 0, 128, True), (B4b, 128, 64, True)], 6, 4,
                          "mul" if mf[4] else "copy", m4mul_d, "m4mul")
                tail_conv(sm, w4c2_t, [B4a, B4b],
                          [(C4a, 0, 128, False), (C4b, 128, 64, False)], 6, 4,
                          "add" if mf[4] else "copy", mn4_d, "mn4")
                tail_pool(sm, ss, [(C4a, 128), (C4b, 64)],
                          [(P5a, 128), (P5b, 64)], 4, mf[5], m5p_d)

                if DBG:
                    nc.sync.dma_start(dbg_d["dP4a"][:], P4a[:])
                    nc.sync.dma_start(dbg_d["dB4a"][:], B4a[:])
                    nc.sync.dma_start(dbg_d["dC4a"][:], C4a[:])

            # ---- L5 ----
            with tc.tile_pool(name="l5w", bufs=1) as wp, \
                 tc.tile_pool(name="l5p", bufs=1) as pp, \
                 tc.tile_pool(name="l5s", bufs=2) as ss, \
                 tc.tile_pool(name="l5m", bufs=2) as sm:
                w5c1_t = [wload(wp, d, dt=F32) for d in genw_d["w5c1"]]
                w5c2_t = [wload(wp, d, dt=F32) for d in genw_d["w5c2"]]
                B5a = pp.tile([128, 64], F32); B5b = pp.tile([96, 64], F32)
                C5a = pp.tile([128, 8], F32); C5b = pp.tile([96, 8], F32)
                nc.vector.memset(B5a[:].bitcast(F32), 0.0)
                nc.vector.memset(B5b[:].bitcast(F32), 0.0)
                tail_conv(sm, w5c1_t, [P5a, P5b],
                          [(B5a, 0, 128, True), (B5b, 128, 96, True)], 4, 2,
                          "mul" if mf[5] else "copy", m5mul_d, "m5mul")
                tail_conv(sm, w5c2_t, [B5a, B5b],
                          [(C5a, 0, 128, False), (C5b, 128, 96, False)], 4, 2,
                          "add" if mf[5] else "copy", mn5_d, "mn5")
                tail_pool(sm, ss, [(C5a, 128), (C5b, 96)],
                          [(P6a, 128), (P6b, 96)], 2, mf[6], m6p_d)

                if DBG:
                    nc.sync.dma_start(dbg_d["dP5a"][:], P5a[:])
                    nc.sync.dma_start(dbg_d["dB5a"][:], B5a[:])
                    nc.sync.dma_start(dbg_d["dP6a"][:], P6a[:])

            # ---- L6 (1^3, center tap only) ----
            with tc.tile_pool(name="l6w", bufs=1) as wp:
                w6c1_t = [wload(wp, d, dt=F32) for d in genw_d["w6c1"]]
                w6c2_t = [wload(wp, d, dt=F32) for d in genw_d["w6c2"]]
                for (ot, c0) in ((X6a, 0), (X6b, 128)):
                    ps = pst.tile([128, 8], F32, tag="ps")
                    nc.tensor.matmul(ps[:, 0:1], w6c1_t[0][:, 0, c0:c0 + 128],
                                     P6a[:, 13:14], start=True, stop=False)
                    nc.tensor.matmul(ps[:, 0:1], w6c1_t[1][:, 0, c0:c0 + 128],
                                     P6b[:, 13:14], start=False, stop=True)
                    nc.vector.tensor_copy(ot[:], ps[:, 0:1])
                for i, c0 in enumerate((0, 128)):
                    ps = pst.tile([128, 8], F32, tag="ps")
                    nc.tensor.matmul(ps[:, 0:1], w6c2_t[0][:, 0, c0:c0 + 128],
                                     X6a[:], start=True, stop=False)
                    nc.tensor.matmul(ps[:, 0:1], w6c2_t[1][:, 0, c0:c0 + 128],
                                     X6b[:], start=False, stop=True)
                    nc.scalar.copy(outt[:, i:i + 1], ps[:, 0:1])
            if DBG:
                nc.sync.dma_start(dbg_d["dX6a"][:], X6a[:])
            nc.sync.dma_start(out_d[0, 0:128], outt[:, 0])
            nc.sync.dma_start(out_d[0, 128:256], outt[:, 1])

    nc.compile()
    return nc



_CACHE = {}
_RUNNERS = {}


def kernel(features, coors, W0, W1, W2, W3, W4, W5, W6, W7, W8, W9, W10, W11,
           W12, W13):
    features = np.asarray(features, np.float32)
    coors = np.asarray(coors, np.int32)
    Ws = [np.asarray(w, np.float32) for w in
          (W0, W1, W2, W3, W4, W5, W6, W7, W8, W9, W10, W11, W12, W13)]
    in_maps, meta = build_host_inputs(features, coors, Ws)
    key = tuple(sorted(meta["mask_flags"].items()))
    if key not in _CACHE:
        _CACHE[key] = build_kernel(meta)
    nc = _CACHE[key]
    try:
        if key not in _RUNNERS:
            _RUNNERS[key] = _Runner(nc, NC)
        r = _RUNNERS[key]
        r.place(in_maps)
        outs = r.run()
        out = r.result(outs, "out").reshape(256)
    except Exception:
        res = run_bass_kernel_spmd(nc, in_maps, core_ids=list(range(NC)))
        out = res.results[0]["out"].reshape(256)
    return out.reshape(1, 1, 1, 1, 256).astype(np.float32)


if __name__ == "__main__":
    pass

